# revision 29
# baseline (speedup 1.0000x reference)
"""GAT 2-layer kernel for 8 Trainium2 NeuronCores (bf16 pipeline).

Strategy (edge-parallel over dst-sorted edges, node-range sharded): host
appends self-loops, sorts edges by dst, gives each core a contiguous 6250-dst
range. Dsts are greedily packed into windows of <=128 dsts whose edges fit a
fixed 9-tile budget (4 "lo" + 5 "hi" tiles of 128 slots, split by src index so
int16 dma_gather indices reach the whole node table). The per-window dst
RANGES vary per core (host data) while the tile geometry is shared, so one
SPMD program serves all 8 cores with ~6% fewer gathered slots than a fixed
10-tile layout. Per-slot attention logits alpha = a_src[src] + a_dst[dst] are
host-expanded (bf16), like all index prep.

  - Launch T: [xh | a_src | a_dst] = x^T-tiles @ [W1P | W1A] per core from a
    host-pretransposed bf16 xT; psums grouped 3 tiles per bank, psum->SBUF
    copies alternate ACT/DVE, one DMA in / six piece DMAs out.
  - Launch E1 (heads=8): small chunks (2-4 windows, DVE-paced); per chunk,
    dma_gather of bf16 xh rows (256B); e = exp(leaky(alpha)) on ACT;
    msg = xh[src] * e on
    DVE (2x, c-major head broadcast); one-hot S per tile (tensor_scalar
    is_equal, 4x); segment sums via S^T @ [msg | e] matmuls accumulated in
    PSUM; ACT copies psums to a bf16 chunk buffer; the normalize + bias + ELU
    epilogue runs batched (in <=5-window halves for PSUM pressure) one chunk
    behind (software pipelining); batched PE transposes + [h@W2 | h@W2A]
    matmuls; per-chunk output DMAs. Tapered tail chunks keep the pipeline
    drain after the last gather short. Host reassembles the layer-2 table
    between launches.
  - Launch E2 (heads=1): e2 is folded into the selection matrix (S_e = e2 *
    one_hot via fused is_equal+mult), the gathered 512B rows carry a trailing
    1.0 so one matmul yields [agg | s]; divide-by-s is fused into the ACT psum
    copy as a per-partition scale; + b2; window-major dump, host scatters rows
    back to node order.

Sharding note (vs the edge-parallel hint): edges are sharded by dst range so
all segment reductions stay core-local in PSUM - no cross-core all-reduce is
needed; the small weights are folded/replicated on the host side.
"""

import os
import sys

sys.path.insert(0, "/opt/trn_rl_repo")

import numpy as np
import ml_dtypes

import concourse.bass as bass
import concourse.bacc as bacc
import concourse.mybir as mybir
import concourse.tile as tile
from concourse.bass_utils import run_bass_kernel_spmd

F32 = mybir.dt.float32
BF16 = mybir.dt.bfloat16
I16 = mybir.dt.int16
ALU = mybir.AluOpType
ACTF = mybir.ActivationFunctionType
BF = ml_dtypes.bfloat16

# Problem constants (hardcoded per harness contract).
N = 50000
E = 400000
FIN = 128
H1, C1 = 8, 16          # layer-1 heads / channels
FMID = H1 * C1          # 128
FOUT = 128
NEG_SLOPE = 0.2

NCORES = 8
NPC = N // NCORES       # 6250 nodes per core
LOT = 4                 # lo tiles per window (src < 32768 reachable)
HIT = 5                 # hi tiles per window (src >= HI_BASE reachable)
TPW = LOT + HIT         # 9 tiles of 128 slots per window
SENT = -1               # sentinel dst_rel for padding slots
HI_BASE = N - 32768     # 17232: hi gather covers rows [HI_BASE, N)
NT_T = (NPC + 127) // 128  # x tiles per core in launch T (49)
NPC_PAD = NT_T * 128
TCOLS = FMID + 2 * H1   # 144: [xh | a_src | a_dst] in launch T

GATHER_TILES = 8        # tiles (128 idxs each) per dma_gather call
                        # (1024 idxs = 64 descs/engine packet, HW limit)

_CACHE = {}


# ----------------------------------------------------------------------------
# Host-side graph preprocessing
# ----------------------------------------------------------------------------

def _wrap16(idx):
    """int16 index array [n] -> dma_gather wrapped layout [16, n//16]."""
    n = idx.shape[0]
    return np.ascontiguousarray(idx.reshape(n // 16, 16).T.astype(np.int16))


class Geom:
    """Shared launch geometry: W windows of TPW tiles, chunk window counts."""

    def __init__(self, wins, chunks=None):
        self.WINS = wins
        if chunks is None:
            full, rem = divmod(wins, 10)
            chunks = [10] * full + ([rem] if rem else [])
            if chunks[-1] > 4:                  # short drain after last gather
                chunks = chunks[:-1] + [chunks[-1] - 3, 3]
        assert sum(chunks) == wins
        self.CHUNKS = chunks
        self.NTILES = wins * TPW
        self.TPC_MAX = max(chunks) * TPW
        # cumulative offsets per chunk (windows / tiles / lo+hi idx columns)
        w0 = [0]
        for cw in chunks:
            w0.append(w0[-1] + cw)
        self.w0 = w0
        self.t0 = [w * TPW for w in w0]
        self.lo_c0 = [w * LOT * 128 // 16 for w in w0]
        self.hi_c0 = [w * HIT * 128 // 16 for w in w0]


def _plan_windows(counts_core, ml_core, mh_core):
    """Greedy dst packing: <=128 dsts, <=LOT*128 lo slots, <=HIT*128 hi
    slots, <=TPW*128 total edges per window. Returns [(dst0, ndst)]."""
    wins = []
    n = counts_core.shape[0]
    d = 0
    cap_t, cap_l, cap_h = TPW * 128, LOT * 128, HIT * 128
    while d < n:
        d0 = d
        tot = ml = mh = 0
        while d < n and d - d0 < 128:
            k, l, h = counts_core[d], ml_core[d], mh_core[d]
            if tot + k > cap_t or ml + l > cap_l or mh + h > cap_h:
                break
            tot += k
            ml += l
            mh += h
            d += 1
        assert d > d0, "single dst exceeds window caps"
        wins.append((d0, d - d0))
    return wins


def _e1_chunks(wins):
    """Fine-grained chunks (4 windows) with small warm-up and taper: E1 is
    DVE-paced, so small chunks pipeline the gathers and epilogue tightly."""
    rem = wins - 10
    assert rem > 0
    return [2, 3] + [4] * (rem // 4) + ([rem % 4] if rem % 4 else []) + [3, 2]


def _plan_all(src, dst):
    """Sort edges by dst, plan shared windows. Returns the plan tuple."""
    s_all = np.concatenate([src, np.arange(N, dtype=np.int64)])
    d_all = np.concatenate([dst, np.arange(N, dtype=np.int64)])
    order = np.argsort(d_all, kind="stable")
    s_all = s_all[order]
    d_all = d_all[order]
    counts = np.bincount(d_all, minlength=N)
    starts = np.concatenate([[0], np.cumsum(counts)])
    # per-dst mandatory-lo / mandatory-hi counts
    ml_all = np.bincount(d_all[s_all < HI_BASE], minlength=N)
    mh_all = np.bincount(d_all[s_all >= 32768], minlength=N)

    core_wins = []
    for c in range(NCORES):
        r = slice(c * NPC, (c + 1) * NPC)
        wins = _plan_windows(counts[r], ml_all[r], mh_all[r])
        core_wins.append([(c * NPC + d0, nd) for d0, nd in wins])
    W = max(len(w) for w in core_wins)
    return (s_all, d_all, starts, core_wins, W)


def _fill_cores(plan, geom):
    """Per-core device index arrays + host slot maps for one chunking."""
    s_all, d_all, starts, core_wins, W = plan
    cores = []
    for c in range(NCORES):
        wmap = core_wins[c] + [(c * NPC, 0)] * (W - len(core_wins[c]))
        slot_src = np.zeros((geom.NTILES, 128), np.int64)
        slot_dst = np.zeros((geom.NTILES, 128), np.int64)
        slot_rel = np.full((geom.NTILES, 128), SENT, np.int64)
        ilo_cols = []
        ihi_cols = []
        for ci, cw in enumerate(geom.CHUNKS):
            lo_flat = np.zeros(cw * LOT * 128, np.int64)
            hi_flat = np.zeros(cw * HIT * 128, np.int64)
            for wi in range(cw):
                w = geom.w0[ci] + wi
                base, nd = wmap[w]
                e0, e1 = starts[base], starts[base + nd]
                ss, dd = s_all[e0:e1], d_all[e0:e1]
                must_lo = ss < HI_BASE
                must_hi = ss >= 32768
                free = ~must_lo & ~must_hi
                cap_lo = LOT * 128
                n_lo = min(cap_lo, int(e1 - e0) - int(must_hi.sum()))
                sel_lo = must_lo.copy()
                free_idx = np.where(free)[0]
                sel_lo[free_idx[:n_lo - int(must_lo.sum())]] = True
                sel_hi = ~sel_lo
                nl, nh = int(sel_lo.sum()), int(sel_hi.sum())
                assert nl <= cap_lo and nh <= HIT * 128, (nl, nh)
                # lo block
                ls = np.zeros(cap_lo, np.int64)
                ld = np.full(cap_lo, base, np.int64)
                lr = np.full(cap_lo, SENT, np.int64)
                ls[:nl] = ss[sel_lo]
                ld[:nl] = dd[sel_lo]
                lr[:nl] = dd[sel_lo] - base
                lo_flat[wi * cap_lo:(wi + 1) * cap_lo] = ls
                g0 = geom.t0[ci] + wi * LOT
                slot_src[g0:g0 + LOT] = ls.reshape(LOT, 128)
                slot_dst[g0:g0 + LOT] = ld.reshape(LOT, 128)
                slot_rel[g0:g0 + LOT] = lr.reshape(LOT, 128)
                # hi block
                cap_hi = HIT * 128
                hs = np.full(cap_hi, HI_BASE, np.int64)
                hd = np.full(cap_hi, base, np.int64)
                hr = np.full(cap_hi, SENT, np.int64)
                hs[:nh] = ss[sel_hi]
                hd[:nh] = dd[sel_hi]
                hr[:nh] = dd[sel_hi] - base
                hi_flat[wi * cap_hi:(wi + 1) * cap_hi] = hs
                g1 = geom.t0[ci] + cw * LOT + wi * HIT
                slot_src[g1:g1 + HIT] = hs.reshape(HIT, 128)
                slot_dst[g1:g1 + HIT] = hd.reshape(HIT, 128)
                slot_rel[g1:g1 + HIT] = hr.reshape(HIT, 128)
            ilo_cols.append(_wrap16(lo_flat))
            ihi_cols.append(_wrap16(hi_flat - HI_BASE))
        idx_lo = np.concatenate(ilo_cols, axis=1)
        idx_hi = np.concatenate(ihi_cols, axis=1)
        cores.append({
            "idx_lo": np.ascontiguousarray(np.tile(idx_lo, (8, 1))),
            "idx_hi": np.ascontiguousarray(np.tile(idx_hi, (8, 1))),
            "slot_src": np.ascontiguousarray(slot_src.T),   # [128, n_tiles]
            "slot_dst": np.ascontiguousarray(slot_dst.T),
            "drel": np.ascontiguousarray(slot_rel.T.astype(np.float32)),
            "wmap": wmap,
        })
    return cores


def _perm_cmajor():
    """Column permutation h*16+c -> c*8+h for layer-1 features."""
    p = np.zeros(FMID, np.int64)
    for h in range(H1):
        for c in range(C1):
            p[c * H1 + h] = h * C1 + c
    return p


# ----------------------------------------------------------------------------
# Bass program builders
# ----------------------------------------------------------------------------

def _new_nc():
    return bacc.Bacc("TRN2", target_bir_lowering=False, debug=False,
                     num_devices=NCORES)


def build_T():
    """Table launch: [xh | a_src | a_dst] = xT^T @ [W1P | W1A] per core."""
    nc = _new_nc()
    xt_in = nc.declare_dram_parameter("xt", [128, NPC_PAD], BF16, isOutput=False)
    w_in = nc.declare_dram_parameter("w", [FIN, TCOLS], BF16, isOutput=False)
    dump_out = nc.declare_dram_parameter("dump", [128, NT_T * TCOLS], BF16,
                                         isOutput=True)

    with tile.TileContext(nc) as tc:
        with (
            tc.tile_pool(name="const", bufs=1) as cpool,
            tc.tile_pool(name="ps", bufs=6, space="PSUM") as pspool,
        ):
            xt = cpool.tile([128, NPC_PAD], BF16)
            w = cpool.tile([FIN, TCOLS], BF16)
            acc = cpool.tile([128, NT_T, TCOLS], BF16)
            # split xT load so tile-0 compute starts early
            nc.sync.dma_start(out=w[:], in_=w_in[:, :])
            q = [0, 6 * 128, 18 * 128, 34 * 128, NPC_PAD]
            for i in range(4):
                nc.sync.dma_start(out=xt[:, q[i]:q[i + 1]],
                                  in_=xt_in[:, q[i]:q[i + 1]])
            for g0 in range(0, NT_T, 3):
                gn = min(3, NT_T - g0)
                ps = pspool.tile([128, 3, TCOLS], F32, space="PSUM")
                for j in range(gn):
                    t = g0 + j
                    nc.tensor.matmul(out=ps[:, j, :],
                                     lhsT=xt[:, t * 128:(t + 1) * 128],
                                     rhs=w[:], start=True, stop=True)
                if (g0 // 3) % 2 == 0:
                    nc.scalar.copy(out=acc[:, g0:g0 + gn, :], in_=ps[:, 0:gn, :])
                else:
                    nc.vector.tensor_copy(out=acc[:, g0:g0 + gn, :],
                                          in_=ps[:, 0:gn, :])
                if g0 + gn in (9, 18, 27, 36, 42, NT_T):
                    marks = [0, 9, 18, 27, 36, 42, NT_T]
                    d0 = marks[marks.index(g0 + gn) - 1] * TCOLS
                    d1 = (g0 + gn) * TCOLS
                    nc.sync.dma_start(out=dump_out[:, d0:d1],
                                      in_=acc[:, d0 // TCOLS:(g0 + gn), :])
    nc.compile()
    return nc


def _emit_gathers(nc, G, table_in, idx, base_tile, n_tiles, idx_col0):
    done = 0
    while done < n_tiles:
        piece = min(GATHER_TILES, n_tiles - done)
        nidx = piece * 128
        c0 = idx_col0 + done * 8
        nc.gpsimd.dma_gather(
            out_ap=G[:, base_tile + done:base_tile + done + piece, :],
            in_ap=table_in, idxs_ap=idx[:, c0:c0 + nidx // 16],
            num_idxs=nidx, num_idxs_reg=nidx,
            elem_size=table_in.shape[-1])
        done += piece


def _emit_gathers_il(nc, G, lo_ap, hi_ap, ilo, ihi, nlo_t, nhi_t,
                     lo_c0, hi_c0):
    """Interleave lo/hi gather pieces so each window's full tile set (its lo
    AND hi block) lands as early as possible."""
    lo_done = hi_done = 0
    while lo_done < nlo_t or hi_done < nhi_t:
        for ap, idx, done, n_t, c0, base in (
                (lo_ap, ilo, lo_done, nlo_t, lo_c0, 0),
                (hi_ap, ihi, hi_done, nhi_t, hi_c0, nlo_t)):
            if done >= n_t:
                continue
            piece = min(GATHER_TILES, n_t - done)
            nidx = piece * 128
            cc = c0 + done * 8
            nc.gpsimd.dma_gather(
                out_ap=G[:, base + done:base + done + piece, :],
                in_ap=ap, idxs_ap=idx[:, cc:cc + nidx // 16],
                num_idxs=nidx, num_idxs_reg=nidx,
                elem_size=ap.shape[-1])
        lo_done = min(nlo_t, lo_done + GATHER_TILES)
        hi_done = min(nhi_t, hi_done + GATHER_TILES)


def _tile_of(ci, cw, wi, t, t0):
    """Global tile id for tile t of window wi in chunk ci (lo block first)."""
    if t < LOT:
        return t0 + wi * LOT + t
    return t0 + cw * LOT + wi * HIT + (t - LOT)


def build_E1(geom, deep_bufs=False, pool_s=0, fill_chunks=6):
    W, NTILES = geom.WINS, geom.NTILES
    LO_COLS = W * LOT * 128 // 16
    HI_COLS = W * HIT * 128 // 16
    nc = _new_nc()
    table_in = nc.declare_dram_parameter("table", [N, 128], BF16, isOutput=False)
    ae_in = nc.declare_dram_parameter("ae", [128, NTILES, H1], BF16,
                                      isOutput=False)
    ilo_in = nc.declare_dram_parameter("ilo", [128, LO_COLS], I16,
                                       isOutput=False)
    ihi_in = nc.declare_dram_parameter("ihi", [128, HI_COLS], I16,
                                       isOutput=False)
    drel_in = nc.declare_dram_parameter("drel", [128, NTILES], F32, isOutput=False)
    cst_in = nc.declare_dram_parameter("cst", [128, 258], BF16, isOutput=False)
    dump_out = nc.declare_dram_parameter("dump", [128, W * (FOUT + 2)], BF16,
                                         isOutput=True)

    with tile.TileContext(nc) as tc:
        with (
            tc.tile_pool(name="const", bufs=1) as cpool,
            tc.tile_pool(name="gat", bufs=3 if deep_bufs else 2) as gpool,
            tc.tile_pool(name="alp", bufs=3 if deep_bufs else 2) as apool,
            tc.tile_pool(name="rhs", bufs=3 if deep_bufs else 2) as rpool,
            tc.tile_pool(name="sel", bufs=48) as spool,
            tc.tile_pool(name="psw", bufs=2, space="PSUM") as ppool,
            tc.tile_pool(name="accp", bufs=2) as accppool,
            tc.tile_pool(name="acca", bufs=2) as accapool,
            tc.tile_pool(name="epi", bufs=2) as epool,
            tc.tile_pool(name="hel", bufs=2) as hpool,
            tc.tile_pool(name="ht", bufs=2) as htpool,
            tc.tile_pool(name="psep", bufs=1, space="PSUM") as peppool,
        ):
            ilo = cpool.tile([128, LO_COLS], I16)
            ihi = cpool.tile([128, HI_COLS], I16)
            drel = cpool.tile([128, NTILES], F32)
            ae = cpool.tile([128, NTILES, H1], BF16)
            cst = cpool.tile([128, 258], BF16)
            ident = cst[:, 0:128]
            w2c = cst[:, 128:258]
            iota_t = cpool.tile([128, 128], BF16)
            iota = iota_t[:]
            # iota built on-device (Pool, ~0.2us): the one-hot S builds then
            # depend only on drel, starting ~1us earlier in the fill
            nc.gpsimd.iota(out=iota, pattern=[[1, 128]], base=0,
                           channel_multiplier=0,
                           allow_small_or_imprecise_dtypes=True)

            # per-chunk JIT input loads: chunks 0/1 up front, chunk ch+1
            # during chunk ch, remainder all at once
            def load_chunk_inputs(c, cend=None):
                cend = c + 1 if cend is None else cend
                l0, l1 = geom.lo_c0[c], geom.lo_c0[cend]
                h0, h1 = geom.hi_c0[c], geom.hi_c0[cend]
                t0, t1 = geom.t0[c], geom.t0[cend]
                nc.sync.dma_start(out=ilo[:, l0:l1], in_=ilo_in[:, l0:l1])
                nc.sync.dma_start(out=ihi[:, h0:h1], in_=ihi_in[:, h0:h1])
                nc.sync.dma_start(out=drel[:, t0:t1], in_=drel_in[:, t0:t1])
                nc.sync.dma_start(out=ae[:, t0:t1, :], in_=ae_in[:, t0:t1, :])

            nchunks = len(geom.CHUNKS)
            load_chunk_inputs(0, min(2, nchunks))
            nc.sync.dma_start(out=cst[:], in_=cst_in[:, :])
            if nchunks > 2:
                load_chunk_inputs(2, nchunks)

            def epilogue(ci, accP, w0, w1):
                # batched normalize + bias + ELU over windows [w0, w1)
                # (fixed 5-window tile shapes so pool tags stay unified)
                nw = w1 - w0
                sEps = epool.tile([128, 5, H1], F32, name="sEps")
                nc.scalar.activation(out=sEps[:, 0:nw, :],
                                     in_=accP[:, w0:w1, 128:128 + H1],
                                     func=ACTF.Copy, bias=1e-30)
                rec = epool.tile([128, 5, H1], BF16, name="rec")
                with nc.allow_low_precision(reason="coef normalize in bf16"):
                    nc.vector.reciprocal(out=rec[:, 0:nw, :],
                                         in_=sEps[:, 0:nw, :])
                # b1 is host-folded into the table rows: since coefs sum
                # to 1, (sum e*(xh+b1))/s = (sum e*xh)/s + b1, landing the
                # bias before the ELU exactly as the reference does
                hB = epool.tile([128, 5, 128], BF16, name="hB")
                nc.vector.tensor_tensor(
                    out=hB[:, 0:nw, :].rearrange("p w (c h) -> p w c h", h=H1),
                    in0=accP[:, w0:w1, 0:128].rearrange(
                        "p w (c h) -> p w c h", h=H1),
                    in1=rec[:, 0:nw, :].unsqueeze(2).broadcast_to(
                        [128, nw, C1, H1]),
                    op=ALU.mult)
                # exp(min(x,0)) = exp(-relu(-x)): both steps on ACT
                tmp = epool.tile([128, 5, 128], BF16, name="tmp")
                nc.scalar.activation(out=tmp[:, 0:nw, :], in_=hB[:, 0:nw, :],
                                     func=ACTF.Relu, scale=-1.0)
                nc.scalar.activation(out=tmp[:, 0:nw, :], in_=tmp[:, 0:nw, :],
                                     func=ACTF.Exp, scale=-1.0)
                helu = hpool.tile([128, 5, 128], BF16, name="helu")
                nc.vector.tensor_scalar(out=helu[:, 0:nw, :],
                                        in0=tmp[:, 0:nw, :],
                                        scalar1=-1.0, scalar2=None, op0=ALU.add)
                nc.vector.tensor_tensor(out=helu[:, 0:nw, :],
                                        in0=helu[:, 0:nw, :],
                                        in1=hB[:, 0:nw, :], op=ALU.max)
                # layer-2 features: [h @ W2 | h @ W2A] via batched PE transpose
                accA = accapool.tile([128, 5, FOUT + 2], BF16, name="accA")
                psT = peppool.tile([128, 5, 128], BF16, space="PSUM",
                                   name="psT")
                for wi in range(nw):
                    nc.tensor.transpose(out=psT[:, wi, :], in_=helu[:, wi, :],
                                        identity=ident)
                hT = htpool.tile([128, 5, 128], BF16, name="hT")
                nc.scalar.copy(out=hT[:, 0:nw, :], in_=psT[:, 0:nw, :])
                n1 = (nw + 1) // 2
                psA1 = peppool.tile([128, 3, FOUT + 2], F32, space="PSUM",
                                    name="psA1")
                psA2 = peppool.tile([128, 2, FOUT + 2], F32,
                                    space="PSUM", name="psA2")
                for wi in range(nw):
                    pa = psA1[:, wi, :] if wi < n1 else psA2[:, wi - n1, :]
                    nc.tensor.matmul(out=pa, lhsT=hT[:, wi, :], rhs=w2c,
                                     start=True, stop=True)
                nc.scalar.copy(out=accA[:, 0:n1, :], in_=psA1[:, 0:n1, :])
                if nw > n1:
                    nc.scalar.copy(out=accA[:, n1:nw, :],
                                   in_=psA2[:, 0:nw - n1, :])
                c0 = (geom.w0[ci] + w0) * (FOUT + 2)
                c1 = (geom.w0[ci] + w1) * (FOUT + 2)
                nc.sync.dma_start(out=dump_out[:, c0:c1],
                                  in_=accA[:, 0:nw, :])

            def emit_exp(ci, cw):
                # e = exp(leaky_relu(alpha)) on ACT (alpha host-preadded);
                # emitted one chunk ahead so the in-order ACT queue never
                # stalls it behind the current chunk's psum copies
                tpc = cw * TPW
                t0 = geom.t0[ci]
                A2 = apool.tile([128, geom.TPC_MAX, H1], BF16, name="A2")
                RHS = rpool.tile([128, geom.TPC_MAX, 128 + H1], BF16,
                                 name="RHS")
                nc.scalar.activation(out=A2[:, 0:tpc, :],
                                     in_=ae[:, t0:t0 + tpc, :],
                                     func=ACTF.Prelu, alpha=NEG_SLOPE)
                nc.scalar.activation(out=RHS[:, 0:tpc, 128:128 + H1],
                                     in_=A2[:, 0:tpc, :], func=ACTF.Exp)
                return RHS

            prev = None
            RHS_cur = None
            for ci, cw in enumerate(geom.CHUNKS):
                t0 = geom.t0[ci]
                tpc = cw * TPW
                nlo_t = cw * LOT
                G = gpool.tile([128, geom.TPC_MAX, 128], BF16, name="G")
                _emit_gathers(nc, G, table_in[:, :], ilo, 0, nlo_t,
                              geom.lo_c0[ci])
                _emit_gathers(nc, G, table_in[HI_BASE:, :], ihi, nlo_t,
                              tpc - nlo_t, geom.hi_c0[ci])
                if ci == 0:
                    RHS_cur = emit_exp(0, cw)
                RHS_nxt = (emit_exp(ci + 1, geom.CHUNKS[ci + 1])
                           if ci + 1 < len(geom.CHUNKS) else None)
                RHS = RHS_cur

                def emit_prev_epilogue():
                    if prev is not None:
                        pci, paccP, pcw = prev
                        for e0 in range(0, pcw, 5):
                            epilogue(pci, paccP, e0, min(e0 + 5, pcw))

                # steady state: previous chunk's epilogue first (its deps are
                # long done, so the in-order DVE queue never stalls on it and
                # it fills DVE while this chunk's gathers land). During the
                # fill (ci < 4) deps complete in order S -> epilogue -> msgs,
                # so emit in that order instead to avoid head-of-line blocks.
                if ci >= fill_chunks:
                    emit_prev_epilogue()

                def emit_msg(lo0, n):
                    # msg = xh[src] * e (broadcast over channels; c-major)
                    in0 = G[:, lo0:lo0 + n, :].rearrange(
                        "p t (c h) -> p t c h", h=H1)
                    in1 = RHS[:, lo0:lo0 + n, 128:128 + H1].unsqueeze(
                        2).broadcast_to([128, n, C1, H1])
                    out0 = RHS[:, lo0:lo0 + n, 0:128].rearrange(
                        "p t (c h) -> p t c h", h=H1)
                    nc.vector.tensor_tensor(out=out0, in0=in0, in1=in1,
                                            op=ALU.mult)

                def build_S(wi, t):
                    gl = _tile_of(ci, cw, wi, t, 0)
                    S = spool.tile([128, 128], BF16, name="S")
                    eng = (nc.gpsimd if (wi == cw - 1 and t < pool_s)
                           else nc.vector)
                    eng.tensor_scalar(
                        out=S[:], in0=iota,
                        scalar1=drel[:, t0 + gl:t0 + gl + 1], scalar2=None,
                        op0=ALU.is_equal)
                    return (gl, S)

                accP = accppool.tile([128, max(geom.CHUNKS), 128 + H1], BF16,
                                     name="accP")
                if ci < fill_chunks:
                    # warm-up chunks: S builds first (no gather dep), so DVE
                    # starts ~2.5us before the first gather lands
                    Sw = [[build_S(wi, t) for t in range(TPW)]
                          for wi in range(cw)]
                    emit_prev_epilogue()
                    for m0 in range(0, tpc, GATHER_TILES):
                        emit_msg(m0, min(GATHER_TILES, tpc - m0))
                    for wi in range(cw):
                        psum = ppool.tile([128, 128 + H1], F32, space="PSUM",
                                          name="psum")
                        for t, (gl, S) in enumerate(Sw[wi]):
                            nc.tensor.matmul(out=psum[:], lhsT=S[:],
                                             rhs=RHS[:, gl, :],
                                             start=(t == 0),
                                             stop=(t == TPW - 1))
                        nc.scalar.copy(out=accP[:, wi, :], in_=psum[:])
                else:
                    for m0 in range(0, nlo_t, GATHER_TILES):
                        emit_msg(m0, min(GATHER_TILES, nlo_t - m0))
                    for wi in range(cw):
                        Ss = [build_S(wi, t) for t in range(TPW)]
                        if wi == 0:
                            for m0 in range(nlo_t, tpc, GATHER_TILES):
                                emit_msg(m0, min(GATHER_TILES, tpc - m0))
                        psum = ppool.tile([128, 128 + H1], F32, space="PSUM",
                                          name="psum")
                        for t, (gl, S) in enumerate(Ss):
                            nc.tensor.matmul(out=psum[:], lhsT=S[:],
                                             rhs=RHS[:, gl, :],
                                             start=(t == 0),
                                             stop=(t == TPW - 1))
                        nc.scalar.copy(out=accP[:, wi, :], in_=psum[:])
                prev = (ci, accP, cw)
                RHS_cur = RHS_nxt
            pci, paccP, pcw = prev
            for e0 in range(0, pcw, 5):
                epilogue(pci, paccP, e0, min(e0 + 5, pcw))
    nc.compile()
    return nc


def build_E2(geom):
    W, NTILES = geom.WINS, geom.NTILES
    LO_COLS = W * LOT * 128 // 16
    HI_COLS = W * HIT * 128 // 16
    nc = _new_nc()
    table_in = nc.declare_dram_parameter("table", [N, 256], BF16, isOutput=False)
    ae_in = nc.declare_dram_parameter("ae", [128, NTILES, 1], BF16,
                                      isOutput=False)
    ilo_in = nc.declare_dram_parameter("ilo", [128, LO_COLS], I16,
                                       isOutput=False)
    ihi_in = nc.declare_dram_parameter("ihi", [128, HI_COLS], I16,
                                       isOutput=False)
    drel_in = nc.declare_dram_parameter("drel", [128, NTILES], F32, isOutput=False)

    dump_out = nc.declare_dram_parameter("dump", [128, W * FOUT], BF16,
                                         isOutput=True)

    with tile.TileContext(nc) as tc:
        with (
            tc.tile_pool(name="const", bufs=1) as cpool,
            tc.tile_pool(name="gat", bufs=2) as gpool,
            tc.tile_pool(name="alp", bufs=2) as apool,
            tc.tile_pool(name="sel", bufs=24) as spool,
            tc.tile_pool(name="psw", bufs=7, space="PSUM") as ppool,
            tc.tile_pool(name="agg", bufs=2) as aggpool,
            tc.tile_pool(name="rc", bufs=4) as rcpool,
        ):
            ilo = cpool.tile([128, LO_COLS], I16)
            ihi = cpool.tile([128, HI_COLS], I16)
            drel = cpool.tile([128, NTILES], F32)
            ae = cpool.tile([128, NTILES, 1], BF16)
            iota_t = cpool.tile([128, 128], BF16)
            iota = iota_t[:]
            nc.gpsimd.iota(out=iota, pattern=[[1, 128]], base=0,
                           channel_multiplier=0,
                           allow_small_or_imprecise_dtypes=True)

            def load_chunk_inputs(c, cend=None):
                cend = c + 1 if cend is None else cend
                l0, l1 = geom.lo_c0[c], geom.lo_c0[cend]
                h0, h1 = geom.hi_c0[c], geom.hi_c0[cend]
                t0, t1 = geom.t0[c], geom.t0[cend]
                nc.sync.dma_start(out=ilo[:, l0:l1], in_=ilo_in[:, l0:l1])
                nc.sync.dma_start(out=ihi[:, h0:h1], in_=ihi_in[:, h0:h1])
                nc.sync.dma_start(out=drel[:, t0:t1], in_=drel_in[:, t0:t1])
                nc.sync.dma_start(out=ae[:, t0:t1, :], in_=ae_in[:, t0:t1, :])

            nchunks = len(geom.CHUNKS)
            load_chunk_inputs(0, min(2, nchunks))
            if nchunks > 2:
                load_chunk_inputs(2, nchunks)

            for ci, cw in enumerate(geom.CHUNKS):
                t0 = geom.t0[ci]
                tpc = cw * TPW
                nlo_t = cw * LOT
                G = gpool.tile([128, geom.TPC_MAX, 256], BF16, name="G")
                _emit_gathers_il(nc, G, table_in[:, :], table_in[HI_BASE:, :],
                                 ilo, ihi, nlo_t, tpc - nlo_t,
                                 geom.lo_c0[ci], geom.hi_c0[ci])
                A = apool.tile([128, geom.TPC_MAX, 1], BF16, name="A")
                A2 = apool.tile([128, geom.TPC_MAX, 1], F32, name="A2")
                nc.scalar.activation(out=A[:, 0:tpc, :],
                                     in_=ae[:, t0:t0 + tpc, :],
                                     func=ACTF.Prelu, alpha=NEG_SLOPE)
                nc.scalar.activation(out=A2[:, 0:tpc, :], in_=A[:, 0:tpc, :],
                                     func=ACTF.Exp)
                aggN = aggpool.tile([128, max(geom.CHUNKS), FOUT], BF16,
                                    name="aggN")
                for wi in range(cw):
                    Ss = []
                    for t in range(TPW):
                        g = _tile_of(ci, cw, wi, t, 0)
                        S = spool.tile([128, 128], BF16, name="S")
                        nc.vector.tensor_scalar(
                            out=S[:], in0=iota,
                            scalar1=drel[:, t0 + g:t0 + g + 1],
                            scalar2=A2[:, g, 0:1],
                            op0=ALU.is_equal, op1=ALU.mult)
                        Ss.append((g, S))
                    psum = ppool.tile([128, 129], F32, space="PSUM",
                                      name="psum")
                    for t, (g, S) in enumerate(Ss):
                        nc.tensor.matmul(out=psum[:], lhsT=S[:],
                                         rhs=G[:, g, 0:129],
                                         start=(t == 0), stop=(t == TPW - 1))
                    # out = agg / s: fold 1/s into the ACT psum copy as a
                    # per-partition scale
                    sEps = rcpool.tile([128, 1], F32, name="sEps")
                    nc.scalar.activation(out=sEps[:], in_=psum[:, 128:129],
                                         func=ACTF.Copy, bias=1e-30)
                    rec = rcpool.tile([128, 1], F32, name="rec")
                    nc.vector.reciprocal(out=rec[:], in_=sEps[:])
                    nc.scalar.activation(out=aggN[:, wi, :], in_=psum[:, 0:128],
                                         func=ACTF.Copy, scale=rec[:])
                # b2 is host-folded into the table rows (coefs sum to 1)
                c0 = geom.w0[ci] * FOUT
                c1 = geom.w0[ci + 1] * FOUT
                nc.sync.dma_start(out=dump_out[:, c0:c1],
                                  in_=aggN[:, 0:cw, :])
    nc.compile()
    return nc


# ----------------------------------------------------------------------------
# Host orchestration
# ----------------------------------------------------------------------------

def _run(nc, in_maps, tag):
    trace = os.environ.get("KERNEL_TRACE", "0") == "1"
    res = run_bass_kernel_spmd(nc, in_maps, list(range(NCORES)), trace=trace)
    if trace:
        _CACHE.setdefault("profiles", {})[tag] = res
    return res.results


def _expand_ae(cores, a_src, a_dst):
    """Host-expanded per-slot alpha = a_src[src] + a_dst[dst] per core."""
    a_src = a_src.astype(np.float32)
    a_dst = a_dst.astype(np.float32)
    return [np.ascontiguousarray(
        (a_src[cd["slot_src"]] + a_dst[cd["slot_dst"]]).astype(BF))
        for cd in cores]


def kernel(x, src, dst, W1, att_src1, att_dst1, b1, W2, att_src2, att_dst2, b2):
    x = np.asarray(x, np.float32)
    src = np.asarray(src, np.int64)
    dst = np.asarray(dst, np.int64)
    W1 = np.asarray(W1, np.float32)
    W2 = np.asarray(W2, np.float32)
    att_src1 = np.asarray(att_src1, np.float32)
    att_dst1 = np.asarray(att_dst1, np.float32)
    att_src2 = np.asarray(att_src2, np.float32)
    att_dst2 = np.asarray(att_dst2, np.float32)
    b1 = np.asarray(b1, np.float32)
    b2 = np.asarray(b2, np.float32)

    ekey = ("edges", hash(src.tobytes()), hash(dst.tobytes()))
    if ekey not in _CACHE:
        plan = _plan_all(src, dst)
        W = plan[4]
        geom1 = Geom(W, _e1_chunks(W))
        geom2 = Geom(W)
        _CACHE[ekey] = (geom1, geom2, _fill_cores(plan, geom1),
                        _fill_cores(plan, geom2))
    geom1, geom2, cores1, cores2 = _CACHE[ekey]

    pkey = ("progs_geom", geom1.WINS, tuple(geom1.CHUNKS),
            tuple(geom2.CHUNKS))
    if pkey not in _CACHE:
        _CACHE[pkey] = (build_T(), build_E1(geom1), build_E2(geom2))
        _CACHE["progs"] = _CACHE[pkey]
    ncT, ncE1, ncE2 = _CACHE[pkey]

    perm = _perm_cmajor()
    W1P = np.ascontiguousarray(W1[:, perm])
    W1A_src = np.einsum("fhc,hc->fh", W1.reshape(FIN, H1, C1), att_src1)
    W1A_dst = np.einsum("fhc,hc->fh", W1.reshape(FIN, H1, C1), att_dst1)
    WT = np.concatenate([W1P, W1A_src, W1A_dst], axis=1).astype(BF)  # [128,144]
    b1P = b1[perm].astype(np.float32)
    W2P = np.ascontiguousarray(W2[perm, :])
    att2cat = np.stack([att_src2[0], att_dst2[0]], axis=1).astype(np.float32)
    W2A = (W2P @ att2cat).astype(np.float32)  # [128, 2] in permuted row space
    W2C = np.concatenate([W2P, W2A], axis=1).astype(BF)  # [128, 130]

    ident = np.eye(128, dtype=np.float32).astype(BF)
    iota = np.tile(np.arange(128, dtype=np.float32), (128, 1)).astype(BF)
    b1rep = np.tile(b1P, (128, 1)).astype(BF)
    b2rep = np.tile(b2, (128, 1)).astype(BF)
    cst1 = np.ascontiguousarray(
        np.concatenate([ident, W2C], axis=1))               # [128, 258]

    # ---- Launch T: per-core [xh | a_src | a_dst] -------------------------
    xtpad = np.zeros((NCORES, 128, NPC_PAD), BF)
    for c in range(NCORES):
        xtpad[c, :, :NPC] = x[c * NPC:(c + 1) * NPC].T.astype(BF)
    in_maps = [{"xt": xtpad[c], "w": WT} for c in range(NCORES)]
    resT = _run(ncT, in_maps, "T")
    parts = []
    for c in range(NCORES):
        d = resT[c]["dump"].reshape(128, NT_T, TCOLS)
        parts.append(d.transpose(1, 0, 2).reshape(NPC_PAD, TCOLS)[:NPC])
    ta = np.concatenate(parts)                      # [N, 144] bf16
    # fold b1 into the rows: (sum e*(xh+b1))/s = (sum e*xh)/s + b1
    table1 = np.ascontiguousarray(
        (ta[:, 0:FMID].astype(np.float32) + b1P).astype(BF))  # [N, 128]
    a1_src = np.ascontiguousarray(ta[:, FMID:FMID + H1])
    a1_dst = np.ascontiguousarray(ta[:, FMID + H1:FMID + 2 * H1])
    ae1 = _expand_ae(cores1, a1_src, a1_dst)

    # ---- Launch E1 --------------------------------------------------------
    in_maps = [{"table": table1, "ae": ae1[c], "ilo": cores1[c]["idx_lo"],
                "ihi": cores1[c]["idx_hi"], "drel": cores1[c]["drel"],
                "cst": cst1}
               for c in range(NCORES)]
    resE1 = _run(ncE1, in_maps, "E1")
    ha = np.zeros((N, FOUT + 2), BF)
    for c in range(NCORES):
        d = resE1[c]["dump"].reshape(128, geom1.WINS, FOUT + 2)
        for w, (base, nd) in enumerate(cores1[c]["wmap"]):
            if nd:
                ha[base:base + nd] = d[0:nd, w, :]
    table2 = np.zeros((N, 256), BF)                 # [xh2+b2 | 1.0 | pad]
    table2[:, 0:FOUT] = (ha[:, 0:FOUT].astype(np.float32) + b2).astype(BF)
    table2[:, FOUT] = BF(1.0)
    a2_src = np.ascontiguousarray(ha[:, FOUT:FOUT + 1])
    a2_dst = np.ascontiguousarray(ha[:, FOUT + 1:FOUT + 2])
    ae2 = _expand_ae(cores2, a2_src, a2_dst)

    # ---- Launch E2 --------------------------------------------------------
    in_maps = [{"table": table2, "ae": ae2[c], "ilo": cores2[c]["idx_lo"],
                "ihi": cores2[c]["idx_hi"], "drel": cores2[c]["drel"]}
               for c in range(NCORES)]
    resE2 = _run(ncE2, in_maps, "E2")
    out = np.zeros((N, FOUT), np.float32)
    for c in range(NCORES):
        d = resE2[c]["dump"].reshape(128, geom2.WINS, FOUT).astype(np.float32)
        for w, (base, nd) in enumerate(cores2[c]["wmap"]):
            if nd:
                out[base:base + nd] = d[0:nd, w, :]
    return np.ascontiguousarray(out)


# revision 31
# speedup vs baseline: 1.0020x; 1.0020x over previous
"""GAT 2-layer kernel for 8 Trainium2 NeuronCores (bf16 pipeline).

Strategy (edge-parallel over dst-sorted edges, node-range sharded): host
appends self-loops, sorts edges by dst, gives each core a contiguous 6250-dst
range. Dsts are greedily packed into windows of <=128 dsts whose edges fit a
fixed 9-tile budget (4 "lo" + 5 "hi" tiles of 128 slots, split by src index so
int16 dma_gather indices reach the whole node table). The per-window dst
RANGES vary per core (host data) while the tile geometry is shared, so one
SPMD program serves all 8 cores with ~6% fewer gathered slots than a fixed
10-tile layout. Per-slot attention logits alpha = a_src[src] + a_dst[dst] are
host-expanded (bf16), like all index prep.

  - Launch T: [xh | a_src | a_dst] = x^T-tiles @ [W1P | W1A] per core from a
    host-pretransposed bf16 xT; psums grouped 3 tiles per bank, psum->SBUF
    copies alternate ACT/DVE, one DMA in / six piece DMAs out.
  - Launch E1 (heads=8): small chunks (2-4 windows, DVE-paced); per chunk,
    dma_gather of bf16 xh rows (256B); e = exp(leaky(alpha)) on ACT;
    msg = xh[src] * e on
    DVE (2x, c-major head broadcast); one-hot S per tile (tensor_scalar
    is_equal, 4x); segment sums via S^T @ [msg | e] matmuls accumulated in
    PSUM; ACT copies psums to a bf16 chunk buffer; the normalize + bias + ELU
    epilogue runs batched (in <=5-window halves for PSUM pressure) one chunk
    behind (software pipelining); batched PE transposes + [h@W2 | h@W2A]
    matmuls; per-chunk output DMAs. Tapered tail chunks keep the pipeline
    drain after the last gather short. Host reassembles the layer-2 table
    between launches.
  - Launch E2 (heads=1): e2 is folded into the selection matrix (S_e = e2 *
    one_hot via fused is_equal+mult), the gathered 512B rows carry a trailing
    1.0 so one matmul yields [agg | s]; divide-by-s is fused into the ACT psum
    copy as a per-partition scale; + b2; window-major dump, host scatters rows
    back to node order.

Sharding note (vs the edge-parallel hint): edges are sharded by dst range so
all segment reductions stay core-local in PSUM - no cross-core all-reduce is
needed; the small weights are folded/replicated on the host side.
"""

import os
import sys

sys.path.insert(0, "/opt/trn_rl_repo")

import numpy as np
import ml_dtypes

import concourse.bass as bass
import concourse.bacc as bacc
import concourse.mybir as mybir
import concourse.tile as tile
from concourse.bass_utils import run_bass_kernel_spmd

F32 = mybir.dt.float32
BF16 = mybir.dt.bfloat16
I16 = mybir.dt.int16
ALU = mybir.AluOpType
ACTF = mybir.ActivationFunctionType
BF = ml_dtypes.bfloat16

# Problem constants (hardcoded per harness contract).
N = 50000
E = 400000
FIN = 128
H1, C1 = 8, 16          # layer-1 heads / channels
FMID = H1 * C1          # 128
FOUT = 128
NEG_SLOPE = 0.2

NCORES = 8
NPC = N // NCORES       # 6250 nodes per core
LOT = 4                 # lo tiles per window (src < 32768 reachable)
HIT = 5                 # hi tiles per window (src >= HI_BASE reachable)
TPW = LOT + HIT         # 9 tiles of 128 slots per window
SENT = -1               # sentinel dst_rel for padding slots
HI_BASE = N - 32768     # 17232: hi gather covers rows [HI_BASE, N)
NT_T = (NPC + 127) // 128  # x tiles per core in launch T (49)
NPC_PAD = NT_T * 128
TCOLS = FMID + 2 * H1   # 144: [xh | a_src | a_dst] in launch T

GATHER_TILES = 8        # tiles (128 idxs each) per dma_gather call
                        # (1024 idxs = 64 descs/engine packet, HW limit)

_CACHE = {}


# ----------------------------------------------------------------------------
# Host-side graph preprocessing
# ----------------------------------------------------------------------------

def _wrap16(idx):
    """int16 index array [n] -> dma_gather wrapped layout [16, n//16]."""
    n = idx.shape[0]
    return np.ascontiguousarray(idx.reshape(n // 16, 16).T.astype(np.int16))


class Geom:
    """Shared launch geometry: W windows of TPW tiles, chunk window counts."""

    def __init__(self, wins, chunks=None):
        self.WINS = wins
        if chunks is None:
            full, rem = divmod(wins, 10)
            chunks = [10] * full + ([rem] if rem else [])
            if chunks[-1] > 4:                  # short drain after last gather
                chunks = chunks[:-1] + [chunks[-1] - 3, 3]
        assert sum(chunks) == wins
        self.CHUNKS = chunks
        self.NTILES = wins * TPW
        self.TPC_MAX = max(chunks) * TPW
        # cumulative offsets per chunk (windows / tiles / lo+hi idx columns)
        w0 = [0]
        for cw in chunks:
            w0.append(w0[-1] + cw)
        self.w0 = w0
        self.t0 = [w * TPW for w in w0]
        self.lo_c0 = [w * LOT * 128 // 16 for w in w0]
        self.hi_c0 = [w * HIT * 128 // 16 for w in w0]


def _plan_windows(counts_core, ml_core, mh_core):
    """Greedy dst packing: <=128 dsts, <=LOT*128 lo slots, <=HIT*128 hi
    slots, <=TPW*128 total edges per window. Returns [(dst0, ndst)]."""
    wins = []
    n = counts_core.shape[0]
    d = 0
    cap_t, cap_l, cap_h = TPW * 128, LOT * 128, HIT * 128
    while d < n:
        d0 = d
        tot = ml = mh = 0
        while d < n and d - d0 < 128:
            k, l, h = counts_core[d], ml_core[d], mh_core[d]
            if tot + k > cap_t or ml + l > cap_l or mh + h > cap_h:
                break
            tot += k
            ml += l
            mh += h
            d += 1
        assert d > d0, "single dst exceeds window caps"
        wins.append((d0, d - d0))
    return wins


def _e1_chunks(wins):
    """Fine-grained chunks (4 windows) with small warm-up and taper: E1 is
    DVE-paced, so small chunks pipeline the gathers and epilogue tightly."""
    rem = wins - 10
    assert rem > 0
    return [2, 3] + [4] * (rem // 4) + ([rem % 4] if rem % 4 else []) + [3, 2]


def _plan_all(src, dst):
    """Sort edges by dst, plan shared windows. Returns the plan tuple."""
    s_all = np.concatenate([src, np.arange(N, dtype=np.int64)])
    d_all = np.concatenate([dst, np.arange(N, dtype=np.int64)])
    order = np.argsort(d_all, kind="stable")
    s_all = s_all[order]
    d_all = d_all[order]
    counts = np.bincount(d_all, minlength=N)
    starts = np.concatenate([[0], np.cumsum(counts)])
    # per-dst mandatory-lo / mandatory-hi counts
    ml_all = np.bincount(d_all[s_all < HI_BASE], minlength=N)
    mh_all = np.bincount(d_all[s_all >= 32768], minlength=N)

    core_wins = []
    for c in range(NCORES):
        r = slice(c * NPC, (c + 1) * NPC)
        wins = _plan_windows(counts[r], ml_all[r], mh_all[r])
        core_wins.append([(c * NPC + d0, nd) for d0, nd in wins])
    W = max(len(w) for w in core_wins)
    return (s_all, d_all, starts, core_wins, W)


def _fill_cores(plan, geom):
    """Per-core device index arrays + host slot maps for one chunking."""
    s_all, d_all, starts, core_wins, W = plan
    cores = []
    for c in range(NCORES):
        wmap = core_wins[c] + [(c * NPC, 0)] * (W - len(core_wins[c]))
        slot_src = np.zeros((geom.NTILES, 128), np.int64)
        slot_dst = np.zeros((geom.NTILES, 128), np.int64)
        slot_rel = np.full((geom.NTILES, 128), SENT, np.int64)
        ilo_cols = []
        ihi_cols = []
        for ci, cw in enumerate(geom.CHUNKS):
            lo_flat = np.zeros(cw * LOT * 128, np.int64)
            hi_flat = np.zeros(cw * HIT * 128, np.int64)
            for wi in range(cw):
                w = geom.w0[ci] + wi
                base, nd = wmap[w]
                e0, e1 = starts[base], starts[base + nd]
                ss, dd = s_all[e0:e1], d_all[e0:e1]
                must_lo = ss < HI_BASE
                must_hi = ss >= 32768
                free = ~must_lo & ~must_hi
                cap_lo = LOT * 128
                n_lo = min(cap_lo, int(e1 - e0) - int(must_hi.sum()))
                sel_lo = must_lo.copy()
                free_idx = np.where(free)[0]
                sel_lo[free_idx[:n_lo - int(must_lo.sum())]] = True
                sel_hi = ~sel_lo
                nl, nh = int(sel_lo.sum()), int(sel_hi.sum())
                assert nl <= cap_lo and nh <= HIT * 128, (nl, nh)
                # lo block
                ls = np.zeros(cap_lo, np.int64)
                ld = np.full(cap_lo, base, np.int64)
                lr = np.full(cap_lo, SENT, np.int64)
                ls[:nl] = ss[sel_lo]
                ld[:nl] = dd[sel_lo]
                lr[:nl] = dd[sel_lo] - base
                lo_flat[wi * cap_lo:(wi + 1) * cap_lo] = ls
                g0 = geom.t0[ci] + wi * LOT
                slot_src[g0:g0 + LOT] = ls.reshape(LOT, 128)
                slot_dst[g0:g0 + LOT] = ld.reshape(LOT, 128)
                slot_rel[g0:g0 + LOT] = lr.reshape(LOT, 128)
                # hi block
                cap_hi = HIT * 128
                hs = np.full(cap_hi, HI_BASE, np.int64)
                hd = np.full(cap_hi, base, np.int64)
                hr = np.full(cap_hi, SENT, np.int64)
                hs[:nh] = ss[sel_hi]
                hd[:nh] = dd[sel_hi]
                hr[:nh] = dd[sel_hi] - base
                hi_flat[wi * cap_hi:(wi + 1) * cap_hi] = hs
                g1 = geom.t0[ci] + cw * LOT + wi * HIT
                slot_src[g1:g1 + HIT] = hs.reshape(HIT, 128)
                slot_dst[g1:g1 + HIT] = hd.reshape(HIT, 128)
                slot_rel[g1:g1 + HIT] = hr.reshape(HIT, 128)
            ilo_cols.append(_wrap16(lo_flat))
            ihi_cols.append(_wrap16(hi_flat - HI_BASE))
        idx_lo = np.concatenate(ilo_cols, axis=1)
        idx_hi = np.concatenate(ihi_cols, axis=1)
        cores.append({
            "idx_lo": np.ascontiguousarray(np.tile(idx_lo, (8, 1))),
            "idx_hi": np.ascontiguousarray(np.tile(idx_hi, (8, 1))),
            "slot_src": np.ascontiguousarray(slot_src.T),   # [128, n_tiles]
            "slot_dst": np.ascontiguousarray(slot_dst.T),
            "drel": np.ascontiguousarray(slot_rel.T.astype(np.float32)),
            "wmap": wmap,
        })
    return cores


def _perm_cmajor():
    """Column permutation h*16+c -> c*8+h for layer-1 features."""
    p = np.zeros(FMID, np.int64)
    for h in range(H1):
        for c in range(C1):
            p[c * H1 + h] = h * C1 + c
    return p


# ----------------------------------------------------------------------------
# Bass program builders
# ----------------------------------------------------------------------------

def _new_nc():
    return bacc.Bacc("TRN2", target_bir_lowering=False, debug=False,
                     num_devices=NCORES)


def build_T():
    """Table launch: [xh | a_src | a_dst] = xT^T @ [W1P | W1A] per core."""
    nc = _new_nc()
    xt_in = nc.declare_dram_parameter("xt", [128, NPC_PAD], BF16, isOutput=False)
    w_in = nc.declare_dram_parameter("w", [FIN, TCOLS], BF16, isOutput=False)
    dump_out = nc.declare_dram_parameter("dump", [128, NT_T * TCOLS], BF16,
                                         isOutput=True)

    with tile.TileContext(nc) as tc:
        with (
            tc.tile_pool(name="const", bufs=1) as cpool,
            tc.tile_pool(name="ps", bufs=6, space="PSUM") as pspool,
        ):
            xt = cpool.tile([128, NPC_PAD], BF16)
            w = cpool.tile([FIN, TCOLS], BF16)
            acc = cpool.tile([128, NT_T, TCOLS], BF16)
            # split xT load so tile-0 compute starts early
            nc.sync.dma_start(out=w[:], in_=w_in[:, :])
            q = [0, 6 * 128, 18 * 128, 34 * 128, NPC_PAD]
            for i in range(4):
                nc.sync.dma_start(out=xt[:, q[i]:q[i + 1]],
                                  in_=xt_in[:, q[i]:q[i + 1]])
            for g0 in range(0, NT_T, 3):
                gn = min(3, NT_T - g0)
                ps = pspool.tile([128, 3, TCOLS], F32, space="PSUM")
                for j in range(gn):
                    t = g0 + j
                    nc.tensor.matmul(out=ps[:, j, :],
                                     lhsT=xt[:, t * 128:(t + 1) * 128],
                                     rhs=w[:], start=True, stop=True)
                if (g0 // 3) % 2 == 0:
                    nc.scalar.copy(out=acc[:, g0:g0 + gn, :], in_=ps[:, 0:gn, :])
                else:
                    nc.vector.tensor_copy(out=acc[:, g0:g0 + gn, :],
                                          in_=ps[:, 0:gn, :])
                if g0 + gn in (9, 18, 27, 36, 42, NT_T):
                    marks = [0, 9, 18, 27, 36, 42, NT_T]
                    d0 = marks[marks.index(g0 + gn) - 1] * TCOLS
                    d1 = (g0 + gn) * TCOLS
                    nc.sync.dma_start(out=dump_out[:, d0:d1],
                                      in_=acc[:, d0 // TCOLS:(g0 + gn), :])
    nc.compile()
    return nc


def _emit_gathers(nc, G, table_in, idx, base_tile, n_tiles, idx_col0):
    done = 0
    while done < n_tiles:
        piece = min(GATHER_TILES, n_tiles - done)
        nidx = piece * 128
        c0 = idx_col0 + done * 8
        nc.gpsimd.dma_gather(
            out_ap=G[:, base_tile + done:base_tile + done + piece, :],
            in_ap=table_in, idxs_ap=idx[:, c0:c0 + nidx // 16],
            num_idxs=nidx, num_idxs_reg=nidx,
            elem_size=table_in.shape[-1])
        done += piece


def _emit_gathers_il(nc, G, lo_ap, hi_ap, ilo, ihi, nlo_t, nhi_t,
                     lo_c0, hi_c0):
    """Interleave lo/hi gather pieces so each window's full tile set (its lo
    AND hi block) lands as early as possible."""
    lo_done = hi_done = 0
    while lo_done < nlo_t or hi_done < nhi_t:
        for ap, idx, done, n_t, c0, base in (
                (lo_ap, ilo, lo_done, nlo_t, lo_c0, 0),
                (hi_ap, ihi, hi_done, nhi_t, hi_c0, nlo_t)):
            if done >= n_t:
                continue
            piece = min(GATHER_TILES, n_t - done)
            nidx = piece * 128
            cc = c0 + done * 8
            nc.gpsimd.dma_gather(
                out_ap=G[:, base + done:base + done + piece, :],
                in_ap=ap, idxs_ap=idx[:, cc:cc + nidx // 16],
                num_idxs=nidx, num_idxs_reg=nidx,
                elem_size=ap.shape[-1])
        lo_done = min(nlo_t, lo_done + GATHER_TILES)
        hi_done = min(nhi_t, hi_done + GATHER_TILES)


def _tile_of(ci, cw, wi, t, t0):
    """Global tile id for tile t of window wi in chunk ci (lo block first)."""
    if t < LOT:
        return t0 + wi * LOT + t
    return t0 + cw * LOT + wi * HIT + (t - LOT)


def build_E1(geom, deep_bufs=False, pool_s=0, fill_chunks=6):
    W, NTILES = geom.WINS, geom.NTILES
    LO_COLS = W * LOT * 128 // 16
    HI_COLS = W * HIT * 128 // 16
    nc = _new_nc()
    table_in = nc.declare_dram_parameter("table", [N, 128], BF16, isOutput=False)
    ae_in = nc.declare_dram_parameter("ae", [128, NTILES, H1], BF16,
                                      isOutput=False)
    ilo_in = nc.declare_dram_parameter("ilo", [128, LO_COLS], I16,
                                       isOutput=False)
    ihi_in = nc.declare_dram_parameter("ihi", [128, HI_COLS], I16,
                                       isOutput=False)
    drel_in = nc.declare_dram_parameter("drel", [128, NTILES], F32, isOutput=False)
    cst_in = nc.declare_dram_parameter("cst", [128, 258], BF16, isOutput=False)
    dump_out = nc.declare_dram_parameter("dump", [128, W * (FOUT + 2)], BF16,
                                         isOutput=True)

    with tile.TileContext(nc) as tc:
        with (
            tc.tile_pool(name="const", bufs=1) as cpool,
            tc.tile_pool(name="gat", bufs=3 if deep_bufs else 2) as gpool,
            tc.tile_pool(name="alp", bufs=3 if deep_bufs else 2) as apool,
            tc.tile_pool(name="rhs", bufs=3 if deep_bufs else 2) as rpool,
            tc.tile_pool(name="sel", bufs=48) as spool,
            tc.tile_pool(name="psw", bufs=2, space="PSUM") as ppool,
            tc.tile_pool(name="accp", bufs=2) as accppool,
            tc.tile_pool(name="acca", bufs=2) as accapool,
            tc.tile_pool(name="epi", bufs=2) as epool,
            tc.tile_pool(name="hel", bufs=2) as hpool,
            tc.tile_pool(name="ht", bufs=2) as htpool,
            tc.tile_pool(name="psep", bufs=1, space="PSUM") as peppool,
        ):
            ilo = cpool.tile([128, LO_COLS], I16)
            ihi = cpool.tile([128, HI_COLS], I16)
            drel = cpool.tile([128, NTILES], F32)
            ae = cpool.tile([128, NTILES, H1], BF16)
            cst = cpool.tile([128, 258], BF16)
            ident = cst[:, 0:128]
            w2c = cst[:, 128:258]
            accAll = cpool.tile([128, W, FOUT + 2], BF16)
            iota_t = cpool.tile([128, 128], BF16)
            iota = iota_t[:]
            # iota built on-device (Pool, ~0.2us): the one-hot S builds then
            # depend only on drel, starting ~1us earlier in the fill
            nc.gpsimd.iota(out=iota, pattern=[[1, 128]], base=0,
                           channel_multiplier=0,
                           allow_small_or_imprecise_dtypes=True)

            # per-chunk JIT input loads: chunks 0/1 up front, chunk ch+1
            # during chunk ch, remainder all at once
            def load_chunk_inputs(c, cend=None):
                cend = c + 1 if cend is None else cend
                l0, l1 = geom.lo_c0[c], geom.lo_c0[cend]
                h0, h1 = geom.hi_c0[c], geom.hi_c0[cend]
                t0, t1 = geom.t0[c], geom.t0[cend]
                nc.sync.dma_start(out=ilo[:, l0:l1], in_=ilo_in[:, l0:l1])
                nc.sync.dma_start(out=ihi[:, h0:h1], in_=ihi_in[:, h0:h1])
                nc.sync.dma_start(out=drel[:, t0:t1], in_=drel_in[:, t0:t1])
                nc.sync.dma_start(out=ae[:, t0:t1, :], in_=ae_in[:, t0:t1, :])

            nchunks = len(geom.CHUNKS)
            load_chunk_inputs(0, min(2, nchunks))
            nc.sync.dma_start(out=cst[:], in_=cst_in[:, :])
            if nchunks > 2:
                load_chunk_inputs(2, nchunks)

            def epilogue(ci, accP, w0, w1):
                # batched normalize + bias + ELU over windows [w0, w1)
                # (fixed 5-window tile shapes so pool tags stay unified)
                nw = w1 - w0
                sEps = epool.tile([128, 5, H1], F32, name="sEps")
                nc.scalar.activation(out=sEps[:, 0:nw, :],
                                     in_=accP[:, w0:w1, 128:128 + H1],
                                     func=ACTF.Copy, bias=1e-30)
                rec = epool.tile([128, 5, H1], BF16, name="rec")
                with nc.allow_low_precision(reason="coef normalize in bf16"):
                    nc.vector.reciprocal(out=rec[:, 0:nw, :],
                                         in_=sEps[:, 0:nw, :])
                # b1 is host-folded into the table rows: since coefs sum
                # to 1, (sum e*(xh+b1))/s = (sum e*xh)/s + b1, landing the
                # bias before the ELU exactly as the reference does
                hB = epool.tile([128, 5, 128], BF16, name="hB")
                nc.vector.tensor_tensor(
                    out=hB[:, 0:nw, :].rearrange("p w (c h) -> p w c h", h=H1),
                    in0=accP[:, w0:w1, 0:128].rearrange(
                        "p w (c h) -> p w c h", h=H1),
                    in1=rec[:, 0:nw, :].unsqueeze(2).broadcast_to(
                        [128, nw, C1, H1]),
                    op=ALU.mult)
                # exp(min(x,0)) = exp(-relu(-x)): both steps on ACT
                tmp = epool.tile([128, 5, 128], BF16, name="tmp")
                nc.scalar.activation(out=tmp[:, 0:nw, :], in_=hB[:, 0:nw, :],
                                     func=ACTF.Relu, scale=-1.0)
                nc.scalar.activation(out=tmp[:, 0:nw, :], in_=tmp[:, 0:nw, :],
                                     func=ACTF.Exp, scale=-1.0)
                helu = hpool.tile([128, 5, 128], BF16, name="helu")
                nc.vector.tensor_scalar(out=helu[:, 0:nw, :],
                                        in0=tmp[:, 0:nw, :],
                                        scalar1=-1.0, scalar2=None, op0=ALU.add)
                nc.vector.tensor_tensor(out=helu[:, 0:nw, :],
                                        in0=helu[:, 0:nw, :],
                                        in1=hB[:, 0:nw, :], op=ALU.max)
                # layer-2 features: [h @ W2 | h @ W2A] via batched PE transpose
                gw0 = geom.w0[ci] + w0
                psT = peppool.tile([128, 5, 128], BF16, space="PSUM",
                                   name="psT")
                for wi in range(nw):
                    nc.tensor.transpose(out=psT[:, wi, :], in_=helu[:, wi, :],
                                        identity=ident)
                hT = htpool.tile([128, 5, 128], BF16, name="hT")
                nc.scalar.copy(out=hT[:, 0:nw, :], in_=psT[:, 0:nw, :])
                n1 = (nw + 1) // 2
                psA1 = peppool.tile([128, 3, FOUT + 2], F32, space="PSUM",
                                    name="psA1")
                psA2 = peppool.tile([128, 2, FOUT + 2], F32,
                                    space="PSUM", name="psA2")
                for wi in range(nw):
                    pa = psA1[:, wi, :] if wi < n1 else psA2[:, wi - n1, :]
                    nc.tensor.matmul(out=pa, lhsT=hT[:, wi, :], rhs=w2c,
                                     start=True, stop=True)
                # results land in the persistent accAll; the dumps are
                # deferred past the last gather so they never steal body
                # DMA slots from the gather stream (the launch pacer)
                nc.scalar.copy(out=accAll[:, gw0:gw0 + n1, :],
                               in_=psA1[:, 0:n1, :])
                if nw > n1:
                    nc.scalar.copy(out=accAll[:, gw0 + n1:gw0 + nw, :],
                                   in_=psA2[:, 0:nw - n1, :])

            def emit_exp(ci, cw):
                # e = exp(leaky_relu(alpha)) on ACT (alpha host-preadded);
                # emitted one chunk ahead so the in-order ACT queue never
                # stalls it behind the current chunk's psum copies
                tpc = cw * TPW
                t0 = geom.t0[ci]
                A2 = apool.tile([128, geom.TPC_MAX, H1], BF16, name="A2")
                RHS = rpool.tile([128, geom.TPC_MAX, 128 + H1], BF16,
                                 name="RHS")
                nc.scalar.activation(out=A2[:, 0:tpc, :],
                                     in_=ae[:, t0:t0 + tpc, :],
                                     func=ACTF.Prelu, alpha=NEG_SLOPE)
                nc.scalar.activation(out=RHS[:, 0:tpc, 128:128 + H1],
                                     in_=A2[:, 0:tpc, :], func=ACTF.Exp)
                return RHS

            prev = None
            RHS_cur = None
            for ci, cw in enumerate(geom.CHUNKS):
                t0 = geom.t0[ci]
                tpc = cw * TPW
                nlo_t = cw * LOT
                G = gpool.tile([128, geom.TPC_MAX, 128], BF16, name="G")
                _emit_gathers(nc, G, table_in[:, :], ilo, 0, nlo_t,
                              geom.lo_c0[ci])
                _emit_gathers(nc, G, table_in[HI_BASE:, :], ihi, nlo_t,
                              tpc - nlo_t, geom.hi_c0[ci])
                if ci == 0:
                    RHS_cur = emit_exp(0, cw)
                RHS_nxt = (emit_exp(ci + 1, geom.CHUNKS[ci + 1])
                           if ci + 1 < len(geom.CHUNKS) else None)
                RHS = RHS_cur

                def emit_prev_epilogue():
                    if prev is not None:
                        pci, paccP, pcw = prev
                        for e0 in range(0, pcw, 5):
                            epilogue(pci, paccP, e0, min(e0 + 5, pcw))

                # steady state: previous chunk's epilogue first (its deps are
                # long done, so the in-order DVE queue never stalls on it and
                # it fills DVE while this chunk's gathers land). During the
                # fill (ci < 4) deps complete in order S -> epilogue -> msgs,
                # so emit in that order instead to avoid head-of-line blocks.
                if ci >= fill_chunks:
                    emit_prev_epilogue()

                def emit_msg(lo0, n):
                    # msg = xh[src] * e (broadcast over channels; c-major)
                    in0 = G[:, lo0:lo0 + n, :].rearrange(
                        "p t (c h) -> p t c h", h=H1)
                    in1 = RHS[:, lo0:lo0 + n, 128:128 + H1].unsqueeze(
                        2).broadcast_to([128, n, C1, H1])
                    out0 = RHS[:, lo0:lo0 + n, 0:128].rearrange(
                        "p t (c h) -> p t c h", h=H1)
                    nc.vector.tensor_tensor(out=out0, in0=in0, in1=in1,
                                            op=ALU.mult)

                def build_S(wi, t):
                    gl = _tile_of(ci, cw, wi, t, 0)
                    S = spool.tile([128, 128], BF16, name="S")
                    eng = (nc.gpsimd if (wi == cw - 1 and t < pool_s)
                           else nc.vector)
                    eng.tensor_scalar(
                        out=S[:], in0=iota,
                        scalar1=drel[:, t0 + gl:t0 + gl + 1], scalar2=None,
                        op0=ALU.is_equal)
                    return (gl, S)

                accP = accppool.tile([128, max(geom.CHUNKS), 128 + H1], BF16,
                                     name="accP")
                if ci < fill_chunks:
                    # warm-up chunks: S builds first (no gather dep), so DVE
                    # starts ~2.5us before the first gather lands
                    Sw = [[build_S(wi, t) for t in range(TPW)]
                          for wi in range(cw)]
                    emit_prev_epilogue()
                    for m0 in range(0, tpc, GATHER_TILES):
                        emit_msg(m0, min(GATHER_TILES, tpc - m0))
                    for wi in range(cw):
                        psum = ppool.tile([128, 128 + H1], F32, space="PSUM",
                                          name="psum")
                        for t, (gl, S) in enumerate(Sw[wi]):
                            nc.tensor.matmul(out=psum[:], lhsT=S[:],
                                             rhs=RHS[:, gl, :],
                                             start=(t == 0),
                                             stop=(t == TPW - 1))
                        nc.scalar.copy(out=accP[:, wi, :], in_=psum[:])
                else:
                    for m0 in range(0, nlo_t, GATHER_TILES):
                        emit_msg(m0, min(GATHER_TILES, nlo_t - m0))
                    for wi in range(cw):
                        Ss = [build_S(wi, t) for t in range(TPW)]
                        if wi == 0:
                            for m0 in range(nlo_t, tpc, GATHER_TILES):
                                emit_msg(m0, min(GATHER_TILES, tpc - m0))
                        psum = ppool.tile([128, 128 + H1], F32, space="PSUM",
                                          name="psum")
                        for t, (gl, S) in enumerate(Ss):
                            nc.tensor.matmul(out=psum[:], lhsT=S[:],
                                             rhs=RHS[:, gl, :],
                                             start=(t == 0),
                                             stop=(t == TPW - 1))
                        nc.scalar.copy(out=accP[:, wi, :], in_=psum[:])
                prev = (ci, accP, cw)
                RHS_cur = RHS_nxt
            pci, paccP, pcw = prev
            for e0 in range(0, pcw, 5):
                epilogue(pci, paccP, e0, min(e0 + 5, pcw))
            # deferred output dumps: pieces ordered by epilogue completion
            cuts = [0, W // 3, 2 * W // 3, geom.w0[-3], geom.w0[-2], W]
            for a, b in zip(cuts, cuts[1:]):
                if b > a:
                    nc.sync.dma_start(
                        out=dump_out[:, a * (FOUT + 2):b * (FOUT + 2)],
                        in_=accAll[:, a:b, :])
    nc.compile()
    return nc


def build_E2(geom):
    W, NTILES = geom.WINS, geom.NTILES
    LO_COLS = W * LOT * 128 // 16
    HI_COLS = W * HIT * 128 // 16
    nc = _new_nc()
    table_in = nc.declare_dram_parameter("table", [N, 256], BF16, isOutput=False)
    ae_in = nc.declare_dram_parameter("ae", [128, NTILES, 1], BF16,
                                      isOutput=False)
    ilo_in = nc.declare_dram_parameter("ilo", [128, LO_COLS], I16,
                                       isOutput=False)
    ihi_in = nc.declare_dram_parameter("ihi", [128, HI_COLS], I16,
                                       isOutput=False)
    drel_in = nc.declare_dram_parameter("drel", [128, NTILES], F32, isOutput=False)

    dump_out = nc.declare_dram_parameter("dump", [128, W * FOUT], BF16,
                                         isOutput=True)

    with tile.TileContext(nc) as tc:
        with (
            tc.tile_pool(name="const", bufs=1) as cpool,
            tc.tile_pool(name="gat", bufs=2) as gpool,
            tc.tile_pool(name="alp", bufs=2) as apool,
            tc.tile_pool(name="sel", bufs=24) as spool,
            tc.tile_pool(name="psw", bufs=7, space="PSUM") as ppool,
            tc.tile_pool(name="agg", bufs=2) as aggpool,
            tc.tile_pool(name="rc", bufs=4) as rcpool,
        ):
            ilo = cpool.tile([128, LO_COLS], I16)
            ihi = cpool.tile([128, HI_COLS], I16)
            drel = cpool.tile([128, NTILES], F32)
            ae = cpool.tile([128, NTILES, 1], BF16)
            accAll = cpool.tile([128, W, FOUT + 2], BF16)
            iota_t = cpool.tile([128, 128], BF16)
            iota = iota_t[:]
            nc.gpsimd.iota(out=iota, pattern=[[1, 128]], base=0,
                           channel_multiplier=0,
                           allow_small_or_imprecise_dtypes=True)

            def load_chunk_inputs(c, cend=None):
                cend = c + 1 if cend is None else cend
                l0, l1 = geom.lo_c0[c], geom.lo_c0[cend]
                h0, h1 = geom.hi_c0[c], geom.hi_c0[cend]
                t0, t1 = geom.t0[c], geom.t0[cend]
                nc.sync.dma_start(out=ilo[:, l0:l1], in_=ilo_in[:, l0:l1])
                nc.sync.dma_start(out=ihi[:, h0:h1], in_=ihi_in[:, h0:h1])
                nc.sync.dma_start(out=drel[:, t0:t1], in_=drel_in[:, t0:t1])
                nc.sync.dma_start(out=ae[:, t0:t1, :], in_=ae_in[:, t0:t1, :])

            nchunks = len(geom.CHUNKS)
            load_chunk_inputs(0, min(2, nchunks))
            if nchunks > 2:
                load_chunk_inputs(2, nchunks)

            for ci, cw in enumerate(geom.CHUNKS):
                t0 = geom.t0[ci]
                tpc = cw * TPW
                nlo_t = cw * LOT
                G = gpool.tile([128, geom.TPC_MAX, 256], BF16, name="G")
                _emit_gathers_il(nc, G, table_in[:, :], table_in[HI_BASE:, :],
                                 ilo, ihi, nlo_t, tpc - nlo_t,
                                 geom.lo_c0[ci], geom.hi_c0[ci])
                A = apool.tile([128, geom.TPC_MAX, 1], BF16, name="A")
                A2 = apool.tile([128, geom.TPC_MAX, 1], F32, name="A2")
                nc.scalar.activation(out=A[:, 0:tpc, :],
                                     in_=ae[:, t0:t0 + tpc, :],
                                     func=ACTF.Prelu, alpha=NEG_SLOPE)
                nc.scalar.activation(out=A2[:, 0:tpc, :], in_=A[:, 0:tpc, :],
                                     func=ACTF.Exp)
                aggN = aggpool.tile([128, max(geom.CHUNKS), FOUT], BF16,
                                    name="aggN")
                for wi in range(cw):
                    Ss = []
                    for t in range(TPW):
                        g = _tile_of(ci, cw, wi, t, 0)
                        S = spool.tile([128, 128], BF16, name="S")
                        nc.vector.tensor_scalar(
                            out=S[:], in0=iota,
                            scalar1=drel[:, t0 + g:t0 + g + 1],
                            scalar2=A2[:, g, 0:1],
                            op0=ALU.is_equal, op1=ALU.mult)
                        Ss.append((g, S))
                    psum = ppool.tile([128, 129], F32, space="PSUM",
                                      name="psum")
                    for t, (g, S) in enumerate(Ss):
                        nc.tensor.matmul(out=psum[:], lhsT=S[:],
                                         rhs=G[:, g, 0:129],
                                         start=(t == 0), stop=(t == TPW - 1))
                    # out = agg / s: fold 1/s into the ACT psum copy as a
                    # per-partition scale
                    sEps = rcpool.tile([128, 1], F32, name="sEps")
                    nc.scalar.activation(out=sEps[:], in_=psum[:, 128:129],
                                         func=ACTF.Copy, bias=1e-30)
                    rec = rcpool.tile([128, 1], F32, name="rec")
                    nc.vector.reciprocal(out=rec[:], in_=sEps[:])
                    nc.scalar.activation(out=aggN[:, wi, :], in_=psum[:, 0:128],
                                         func=ACTF.Copy, scale=rec[:])
                # b2 is host-folded into the table rows (coefs sum to 1)
                c0 = geom.w0[ci] * FOUT
                c1 = geom.w0[ci + 1] * FOUT
                nc.sync.dma_start(out=dump_out[:, c0:c1],
                                  in_=aggN[:, 0:cw, :])
    nc.compile()
    return nc


# ----------------------------------------------------------------------------
# Host orchestration
# ----------------------------------------------------------------------------

def _run(nc, in_maps, tag):
    trace = os.environ.get("KERNEL_TRACE", "0") == "1"
    res = run_bass_kernel_spmd(nc, in_maps, list(range(NCORES)), trace=trace)
    if trace:
        _CACHE.setdefault("profiles", {})[tag] = res
    return res.results


def _expand_ae(cores, a_src, a_dst):
    """Host-expanded per-slot alpha = a_src[src] + a_dst[dst] per core."""
    a_src = a_src.astype(np.float32)
    a_dst = a_dst.astype(np.float32)
    return [np.ascontiguousarray(
        (a_src[cd["slot_src"]] + a_dst[cd["slot_dst"]]).astype(BF))
        for cd in cores]


def kernel(x, src, dst, W1, att_src1, att_dst1, b1, W2, att_src2, att_dst2, b2):
    x = np.asarray(x, np.float32)
    src = np.asarray(src, np.int64)
    dst = np.asarray(dst, np.int64)
    W1 = np.asarray(W1, np.float32)
    W2 = np.asarray(W2, np.float32)
    att_src1 = np.asarray(att_src1, np.float32)
    att_dst1 = np.asarray(att_dst1, np.float32)
    att_src2 = np.asarray(att_src2, np.float32)
    att_dst2 = np.asarray(att_dst2, np.float32)
    b1 = np.asarray(b1, np.float32)
    b2 = np.asarray(b2, np.float32)

    ekey = ("edges", hash(src.tobytes()), hash(dst.tobytes()))
    if ekey not in _CACHE:
        plan = _plan_all(src, dst)
        W = plan[4]
        geom1 = Geom(W, _e1_chunks(W))
        geom2 = Geom(W)
        _CACHE[ekey] = (geom1, geom2, _fill_cores(plan, geom1),
                        _fill_cores(plan, geom2))
    geom1, geom2, cores1, cores2 = _CACHE[ekey]

    pkey = ("progs_geom", geom1.WINS, tuple(geom1.CHUNKS),
            tuple(geom2.CHUNKS))
    if pkey not in _CACHE:
        _CACHE[pkey] = (build_T(), build_E1(geom1), build_E2(geom2))
        _CACHE["progs"] = _CACHE[pkey]
    ncT, ncE1, ncE2 = _CACHE[pkey]

    perm = _perm_cmajor()
    W1P = np.ascontiguousarray(W1[:, perm])
    W1A_src = np.einsum("fhc,hc->fh", W1.reshape(FIN, H1, C1), att_src1)
    W1A_dst = np.einsum("fhc,hc->fh", W1.reshape(FIN, H1, C1), att_dst1)
    WT = np.concatenate([W1P, W1A_src, W1A_dst], axis=1).astype(BF)  # [128,144]
    b1P = b1[perm].astype(np.float32)
    W2P = np.ascontiguousarray(W2[perm, :])
    att2cat = np.stack([att_src2[0], att_dst2[0]], axis=1).astype(np.float32)
    W2A = (W2P @ att2cat).astype(np.float32)  # [128, 2] in permuted row space
    W2C = np.concatenate([W2P, W2A], axis=1).astype(BF)  # [128, 130]

    ident = np.eye(128, dtype=np.float32).astype(BF)
    iota = np.tile(np.arange(128, dtype=np.float32), (128, 1)).astype(BF)
    b1rep = np.tile(b1P, (128, 1)).astype(BF)
    b2rep = np.tile(b2, (128, 1)).astype(BF)
    cst1 = np.ascontiguousarray(
        np.concatenate([ident, W2C], axis=1))               # [128, 258]

    # ---- Launch T: per-core [xh | a_src | a_dst] -------------------------
    xtpad = np.zeros((NCORES, 128, NPC_PAD), BF)
    for c in range(NCORES):
        xtpad[c, :, :NPC] = x[c * NPC:(c + 1) * NPC].T.astype(BF)
    in_maps = [{"xt": xtpad[c], "w": WT} for c in range(NCORES)]
    resT = _run(ncT, in_maps, "T")
    parts = []
    for c in range(NCORES):
        d = resT[c]["dump"].reshape(128, NT_T, TCOLS)
        parts.append(d.transpose(1, 0, 2).reshape(NPC_PAD, TCOLS)[:NPC])
    ta = np.concatenate(parts)                      # [N, 144] bf16
    # fold b1 into the rows: (sum e*(xh+b1))/s = (sum e*xh)/s + b1
    table1 = np.ascontiguousarray(
        (ta[:, 0:FMID].astype(np.float32) + b1P).astype(BF))  # [N, 128]
    a1_src = np.ascontiguousarray(ta[:, FMID:FMID + H1])
    a1_dst = np.ascontiguousarray(ta[:, FMID + H1:FMID + 2 * H1])
    ae1 = _expand_ae(cores1, a1_src, a1_dst)

    # ---- Launch E1 --------------------------------------------------------
    in_maps = [{"table": table1, "ae": ae1[c], "ilo": cores1[c]["idx_lo"],
                "ihi": cores1[c]["idx_hi"], "drel": cores1[c]["drel"],
                "cst": cst1}
               for c in range(NCORES)]
    resE1 = _run(ncE1, in_maps, "E1")
    ha = np.zeros((N, FOUT + 2), BF)
    for c in range(NCORES):
        d = resE1[c]["dump"].reshape(128, geom1.WINS, FOUT + 2)
        for w, (base, nd) in enumerate(cores1[c]["wmap"]):
            if nd:
                ha[base:base + nd] = d[0:nd, w, :]
    table2 = np.zeros((N, 256), BF)                 # [xh2+b2 | 1.0 | pad]
    table2[:, 0:FOUT] = (ha[:, 0:FOUT].astype(np.float32) + b2).astype(BF)
    table2[:, FOUT] = BF(1.0)
    a2_src = np.ascontiguousarray(ha[:, FOUT:FOUT + 1])
    a2_dst = np.ascontiguousarray(ha[:, FOUT + 1:FOUT + 2])
    ae2 = _expand_ae(cores2, a2_src, a2_dst)

    # ---- Launch E2 --------------------------------------------------------
    in_maps = [{"table": table2, "ae": ae2[c], "ilo": cores2[c]["idx_lo"],
                "ihi": cores2[c]["idx_hi"], "drel": cores2[c]["drel"]}
               for c in range(NCORES)]
    resE2 = _run(ncE2, in_maps, "E2")
    out = np.zeros((N, FOUT), np.float32)
    for c in range(NCORES):
        d = resE2[c]["dump"].reshape(128, geom2.WINS, FOUT).astype(np.float32)
        for w, (base, nd) in enumerate(cores2[c]["wmap"]):
            if nd:
                out[base:base + nd] = d[0:nd, w, :]
    return np.ascontiguousarray(out)


# revision 32
# speedup vs baseline: 1.0041x; 1.0021x over previous
"""GAT 2-layer kernel for 8 Trainium2 NeuronCores (bf16 pipeline).

Strategy (edge-parallel over dst-sorted edges, node-range sharded): host
appends self-loops, sorts edges by dst, gives each core a contiguous 6250-dst
range. Dsts are greedily packed into windows of <=128 dsts whose edges fit a
fixed 9-tile budget (4 "lo" + 5 "hi" tiles of 128 slots, split by src index so
int16 dma_gather indices reach the whole node table). The per-window dst
RANGES vary per core (host data) while the tile geometry is shared, so one
SPMD program serves all 8 cores with ~6% fewer gathered slots than a fixed
10-tile layout. Per-slot attention logits alpha = a_src[src] + a_dst[dst] are
host-expanded (bf16), like all index prep.

  - Launch T: [xh | a_src | a_dst] = x^T-tiles @ [W1P | W1A] per core from a
    host-pretransposed bf16 xT; psums grouped 3 tiles per bank, psum->SBUF
    copies alternate ACT/DVE, one DMA in / six piece DMAs out.
  - Launch E1 (heads=8): small chunks (2-4 windows, DVE-paced); per chunk,
    dma_gather of bf16 xh rows (256B); e = exp(leaky(alpha)) on ACT;
    msg = xh[src] * e on
    DVE (2x, c-major head broadcast); one-hot S per tile (tensor_scalar
    is_equal, 4x); segment sums via S^T @ [msg | e] matmuls accumulated in
    PSUM; ACT copies psums to a bf16 chunk buffer; the normalize + bias + ELU
    epilogue runs batched (in <=5-window halves for PSUM pressure) one chunk
    behind (software pipelining); batched PE transposes + [h@W2 | h@W2A]
    matmuls; per-chunk output DMAs. Tapered tail chunks keep the pipeline
    drain after the last gather short. Host reassembles the layer-2 table
    between launches.
  - Launch E2 (heads=1): e2 is folded into the selection matrix (S_e = e2 *
    one_hot via fused is_equal+mult), the gathered 512B rows carry a trailing
    1.0 so one matmul yields [agg | s]; divide-by-s is fused into the ACT psum
    copy as a per-partition scale; + b2; window-major dump, host scatters rows
    back to node order.

Sharding note (vs the edge-parallel hint): edges are sharded by dst range so
all segment reductions stay core-local in PSUM - no cross-core all-reduce is
needed; the small weights are folded/replicated on the host side.
"""

import os
import sys

sys.path.insert(0, "/opt/trn_rl_repo")

import numpy as np
import ml_dtypes

import concourse.bass as bass
import concourse.bacc as bacc
import concourse.mybir as mybir
import concourse.tile as tile
from concourse.bass_utils import run_bass_kernel_spmd

F32 = mybir.dt.float32
BF16 = mybir.dt.bfloat16
I16 = mybir.dt.int16
ALU = mybir.AluOpType
ACTF = mybir.ActivationFunctionType
BF = ml_dtypes.bfloat16

# Problem constants (hardcoded per harness contract).
N = 50000
E = 400000
FIN = 128
H1, C1 = 8, 16          # layer-1 heads / channels
FMID = H1 * C1          # 128
FOUT = 128
NEG_SLOPE = 0.2

NCORES = 8
NPC = N // NCORES       # 6250 nodes per core
LOT = 4                 # lo tiles per window (src < 32768 reachable)
HIT = 5                 # hi tiles per window (src >= HI_BASE reachable)
TPW = LOT + HIT         # 9 tiles of 128 slots per window
SENT = -1               # sentinel dst_rel for padding slots
HI_BASE = N - 32768     # 17232: hi gather covers rows [HI_BASE, N)
NT_T = (NPC + 127) // 128  # x tiles per core in launch T (49)
NPC_PAD = NT_T * 128
TCOLS = FMID + 2 * H1   # 144: [xh | a_src | a_dst] in launch T

GATHER_TILES = 8        # tiles (128 idxs each) per dma_gather call
                        # (1024 idxs = 64 descs/engine packet, HW limit)

_CACHE = {}


# ----------------------------------------------------------------------------
# Host-side graph preprocessing
# ----------------------------------------------------------------------------

def _wrap16(idx):
    """int16 index array [n] -> dma_gather wrapped layout [16, n//16]."""
    n = idx.shape[0]
    return np.ascontiguousarray(idx.reshape(n // 16, 16).T.astype(np.int16))


class Geom:
    """Shared launch geometry: W windows of TPW tiles, chunk window counts."""

    def __init__(self, wins, chunks=None):
        self.WINS = wins
        if chunks is None:
            full, rem = divmod(wins, 10)
            chunks = [10] * full + ([rem] if rem else [])
            if chunks[-1] > 4:                  # short drain after last gather
                chunks = chunks[:-1] + [chunks[-1] - 3, 3]
        assert sum(chunks) == wins
        self.CHUNKS = chunks
        self.NTILES = wins * TPW
        self.TPC_MAX = max(chunks) * TPW
        # cumulative offsets per chunk (windows / tiles / lo+hi idx columns)
        w0 = [0]
        for cw in chunks:
            w0.append(w0[-1] + cw)
        self.w0 = w0
        self.t0 = [w * TPW for w in w0]
        self.lo_c0 = [w * LOT * 128 // 16 for w in w0]
        self.hi_c0 = [w * HIT * 128 // 16 for w in w0]


def _plan_windows(counts_core, ml_core, mh_core):
    """Greedy dst packing: <=128 dsts, <=LOT*128 lo slots, <=HIT*128 hi
    slots, <=TPW*128 total edges per window. Returns [(dst0, ndst)]."""
    wins = []
    n = counts_core.shape[0]
    d = 0
    cap_t, cap_l, cap_h = TPW * 128, LOT * 128, HIT * 128
    while d < n:
        d0 = d
        tot = ml = mh = 0
        while d < n and d - d0 < 128:
            k, l, h = counts_core[d], ml_core[d], mh_core[d]
            if tot + k > cap_t or ml + l > cap_l or mh + h > cap_h:
                break
            tot += k
            ml += l
            mh += h
            d += 1
        assert d > d0, "single dst exceeds window caps"
        wins.append((d0, d - d0))
    return wins


def _e1_chunks(wins):
    """Fine-grained chunks (4 windows) with small warm-up and taper: E1 is
    DVE-paced, so small chunks pipeline the gathers and epilogue tightly."""
    rem = wins - 10
    assert rem > 0
    return [2, 3] + [4] * (rem // 4) + ([rem % 4] if rem % 4 else []) + [3, 2]


def _plan_all(src, dst):
    """Sort edges by dst, plan shared windows. Returns the plan tuple."""
    s_all = np.concatenate([src, np.arange(N, dtype=np.int64)])
    d_all = np.concatenate([dst, np.arange(N, dtype=np.int64)])
    order = np.argsort(d_all, kind="stable")
    s_all = s_all[order]
    d_all = d_all[order]
    counts = np.bincount(d_all, minlength=N)
    starts = np.concatenate([[0], np.cumsum(counts)])
    # per-dst mandatory-lo / mandatory-hi counts
    ml_all = np.bincount(d_all[s_all < HI_BASE], minlength=N)
    mh_all = np.bincount(d_all[s_all >= 32768], minlength=N)

    core_wins = []
    for c in range(NCORES):
        r = slice(c * NPC, (c + 1) * NPC)
        wins = _plan_windows(counts[r], ml_all[r], mh_all[r])
        core_wins.append([(c * NPC + d0, nd) for d0, nd in wins])
    W = max(len(w) for w in core_wins)
    return (s_all, d_all, starts, core_wins, W)


def _fill_cores(plan, geom):
    """Per-core device index arrays + host slot maps for one chunking."""
    s_all, d_all, starts, core_wins, W = plan
    cores = []
    for c in range(NCORES):
        wmap = core_wins[c] + [(c * NPC, 0)] * (W - len(core_wins[c]))
        slot_src = np.zeros((geom.NTILES, 128), np.int64)
        slot_dst = np.zeros((geom.NTILES, 128), np.int64)
        slot_rel = np.full((geom.NTILES, 128), SENT, np.int64)
        ilo_cols = []
        ihi_cols = []
        for ci, cw in enumerate(geom.CHUNKS):
            lo_flat = np.zeros(cw * LOT * 128, np.int64)
            hi_flat = np.zeros(cw * HIT * 128, np.int64)
            for wi in range(cw):
                w = geom.w0[ci] + wi
                base, nd = wmap[w]
                e0, e1 = starts[base], starts[base + nd]
                ss, dd = s_all[e0:e1], d_all[e0:e1]
                must_lo = ss < HI_BASE
                must_hi = ss >= 32768
                free = ~must_lo & ~must_hi
                cap_lo = LOT * 128
                n_lo = min(cap_lo, int(e1 - e0) - int(must_hi.sum()))
                sel_lo = must_lo.copy()
                free_idx = np.where(free)[0]
                sel_lo[free_idx[:n_lo - int(must_lo.sum())]] = True
                sel_hi = ~sel_lo
                nl, nh = int(sel_lo.sum()), int(sel_hi.sum())
                assert nl <= cap_lo and nh <= HIT * 128, (nl, nh)
                # lo block
                ls = np.zeros(cap_lo, np.int64)
                ld = np.full(cap_lo, base, np.int64)
                lr = np.full(cap_lo, SENT, np.int64)
                ls[:nl] = ss[sel_lo]
                ld[:nl] = dd[sel_lo]
                lr[:nl] = dd[sel_lo] - base
                lo_flat[wi * cap_lo:(wi + 1) * cap_lo] = ls
                g0 = geom.t0[ci] + wi * LOT
                slot_src[g0:g0 + LOT] = ls.reshape(LOT, 128)
                slot_dst[g0:g0 + LOT] = ld.reshape(LOT, 128)
                slot_rel[g0:g0 + LOT] = lr.reshape(LOT, 128)
                # hi block
                cap_hi = HIT * 128
                hs = np.full(cap_hi, HI_BASE, np.int64)
                hd = np.full(cap_hi, base, np.int64)
                hr = np.full(cap_hi, SENT, np.int64)
                hs[:nh] = ss[sel_hi]
                hd[:nh] = dd[sel_hi]
                hr[:nh] = dd[sel_hi] - base
                hi_flat[wi * cap_hi:(wi + 1) * cap_hi] = hs
                g1 = geom.t0[ci] + cw * LOT + wi * HIT
                slot_src[g1:g1 + HIT] = hs.reshape(HIT, 128)
                slot_dst[g1:g1 + HIT] = hd.reshape(HIT, 128)
                slot_rel[g1:g1 + HIT] = hr.reshape(HIT, 128)
            ilo_cols.append(_wrap16(lo_flat))
            ihi_cols.append(_wrap16(hi_flat - HI_BASE))
        idx_lo = np.concatenate(ilo_cols, axis=1)
        idx_hi = np.concatenate(ihi_cols, axis=1)
        cores.append({
            "idx_lo": np.ascontiguousarray(np.tile(idx_lo, (8, 1))),
            "idx_hi": np.ascontiguousarray(np.tile(idx_hi, (8, 1))),
            "slot_src": np.ascontiguousarray(slot_src.T),   # [128, n_tiles]
            "slot_dst": np.ascontiguousarray(slot_dst.T),
            "drel": np.ascontiguousarray(slot_rel.T.astype(np.int16)),
            "wmap": wmap,
        })
    return cores


def _perm_cmajor():
    """Column permutation h*16+c -> c*8+h for layer-1 features."""
    p = np.zeros(FMID, np.int64)
    for h in range(H1):
        for c in range(C1):
            p[c * H1 + h] = h * C1 + c
    return p


# ----------------------------------------------------------------------------
# Bass program builders
# ----------------------------------------------------------------------------

def _new_nc():
    return bacc.Bacc("TRN2", target_bir_lowering=False, debug=False,
                     num_devices=NCORES)


def build_T():
    """Table launch: [xh | a_src | a_dst] = xT^T @ [W1P | W1A] per core."""
    nc = _new_nc()
    xt_in = nc.declare_dram_parameter("xt", [128, NPC_PAD], BF16, isOutput=False)
    w_in = nc.declare_dram_parameter("w", [FIN, TCOLS], BF16, isOutput=False)
    dump_out = nc.declare_dram_parameter("dump", [128, NT_T * TCOLS], BF16,
                                         isOutput=True)

    with tile.TileContext(nc) as tc:
        with (
            tc.tile_pool(name="const", bufs=1) as cpool,
            tc.tile_pool(name="ps", bufs=6, space="PSUM") as pspool,
        ):
            xt = cpool.tile([128, NPC_PAD], BF16)
            w = cpool.tile([FIN, TCOLS], BF16)
            acc = cpool.tile([128, NT_T, TCOLS], BF16)
            # split xT load so tile-0 compute starts early
            nc.sync.dma_start(out=w[:], in_=w_in[:, :])
            q = [0, 6 * 128, 18 * 128, 34 * 128, NPC_PAD]
            for i in range(4):
                nc.sync.dma_start(out=xt[:, q[i]:q[i + 1]],
                                  in_=xt_in[:, q[i]:q[i + 1]])
            for g0 in range(0, NT_T, 3):
                gn = min(3, NT_T - g0)
                ps = pspool.tile([128, 3, TCOLS], F32, space="PSUM")
                for j in range(gn):
                    t = g0 + j
                    nc.tensor.matmul(out=ps[:, j, :],
                                     lhsT=xt[:, t * 128:(t + 1) * 128],
                                     rhs=w[:], start=True, stop=True)
                if (g0 // 3) % 2 == 0:
                    nc.scalar.copy(out=acc[:, g0:g0 + gn, :], in_=ps[:, 0:gn, :])
                else:
                    nc.vector.tensor_copy(out=acc[:, g0:g0 + gn, :],
                                          in_=ps[:, 0:gn, :])
                if g0 + gn in (9, 18, 27, 36, 42, NT_T):
                    marks = [0, 9, 18, 27, 36, 42, NT_T]
                    d0 = marks[marks.index(g0 + gn) - 1] * TCOLS
                    d1 = (g0 + gn) * TCOLS
                    nc.sync.dma_start(out=dump_out[:, d0:d1],
                                      in_=acc[:, d0 // TCOLS:(g0 + gn), :])
    nc.compile()
    return nc


def _emit_gathers(nc, G, table_in, idx, base_tile, n_tiles, idx_col0):
    done = 0
    while done < n_tiles:
        piece = min(GATHER_TILES, n_tiles - done)
        nidx = piece * 128
        c0 = idx_col0 + done * 8
        nc.gpsimd.dma_gather(
            out_ap=G[:, base_tile + done:base_tile + done + piece, :],
            in_ap=table_in, idxs_ap=idx[:, c0:c0 + nidx // 16],
            num_idxs=nidx, num_idxs_reg=nidx,
            elem_size=table_in.shape[-1])
        done += piece


def _emit_gathers_il(nc, G, lo_ap, hi_ap, ilo, ihi, nlo_t, nhi_t,
                     lo_c0, hi_c0):
    """Interleave lo/hi gather pieces so each window's full tile set (its lo
    AND hi block) lands as early as possible."""
    lo_done = hi_done = 0
    while lo_done < nlo_t or hi_done < nhi_t:
        for ap, idx, done, n_t, c0, base in (
                (lo_ap, ilo, lo_done, nlo_t, lo_c0, 0),
                (hi_ap, ihi, hi_done, nhi_t, hi_c0, nlo_t)):
            if done >= n_t:
                continue
            piece = min(GATHER_TILES, n_t - done)
            nidx = piece * 128
            cc = c0 + done * 8
            nc.gpsimd.dma_gather(
                out_ap=G[:, base + done:base + done + piece, :],
                in_ap=ap, idxs_ap=idx[:, cc:cc + nidx // 16],
                num_idxs=nidx, num_idxs_reg=nidx,
                elem_size=ap.shape[-1])
        lo_done = min(nlo_t, lo_done + GATHER_TILES)
        hi_done = min(nhi_t, hi_done + GATHER_TILES)


def _tile_of(ci, cw, wi, t, t0):
    """Global tile id for tile t of window wi in chunk ci (lo block first)."""
    if t < LOT:
        return t0 + wi * LOT + t
    return t0 + cw * LOT + wi * HIT + (t - LOT)


def build_E1(geom, deep_bufs=False, pool_s=0, fill_chunks=6):
    W, NTILES = geom.WINS, geom.NTILES
    LO_COLS = W * LOT * 128 // 16
    HI_COLS = W * HIT * 128 // 16
    nc = _new_nc()
    table_in = nc.declare_dram_parameter("table", [N, 128], BF16, isOutput=False)
    ae_in = nc.declare_dram_parameter("ae", [128, NTILES, H1], BF16,
                                      isOutput=False)
    ilo_in = nc.declare_dram_parameter("ilo", [128, LO_COLS], I16,
                                       isOutput=False)
    ihi_in = nc.declare_dram_parameter("ihi", [128, HI_COLS], I16,
                                       isOutput=False)
    drel_in = nc.declare_dram_parameter("drel", [128, NTILES], I16, isOutput=False)
    cst_in = nc.declare_dram_parameter("cst", [128, 258], BF16, isOutput=False)
    dump_out = nc.declare_dram_parameter("dump", [128, W * (FOUT + 2)], BF16,
                                         isOutput=True)

    with tile.TileContext(nc) as tc:
        with (
            tc.tile_pool(name="const", bufs=1) as cpool,
            tc.tile_pool(name="gat", bufs=3 if deep_bufs else 2) as gpool,
            tc.tile_pool(name="alp", bufs=3 if deep_bufs else 2) as apool,
            tc.tile_pool(name="rhs", bufs=3 if deep_bufs else 2) as rpool,
            tc.tile_pool(name="sel", bufs=48) as spool,
            tc.tile_pool(name="psw", bufs=2, space="PSUM") as ppool,
            tc.tile_pool(name="accp", bufs=2) as accppool,
            tc.tile_pool(name="acca", bufs=2) as accapool,
            tc.tile_pool(name="epi", bufs=2) as epool,
            tc.tile_pool(name="hel", bufs=2) as hpool,
            tc.tile_pool(name="ht", bufs=2) as htpool,
            tc.tile_pool(name="psep", bufs=1, space="PSUM") as peppool,
        ):
            ilo = cpool.tile([128, LO_COLS], I16)
            ihi = cpool.tile([128, HI_COLS], I16)
            drel = cpool.tile([128, NTILES], F32)
            drel16 = cpool.tile([128, NTILES], I16)
            ae = cpool.tile([128, NTILES, H1], BF16)
            cst = cpool.tile([128, 258], BF16)
            ident = cst[:, 0:128]
            w2c = cst[:, 128:258]
            accAll = cpool.tile([128, W, FOUT + 2], BF16)
            iota_t = cpool.tile([128, 128], BF16)
            iota = iota_t[:]
            # iota built on-device (Pool, ~0.2us): the one-hot S builds then
            # depend only on drel, starting ~1us earlier in the fill
            nc.gpsimd.iota(out=iota, pattern=[[1, 128]], base=0,
                           channel_multiplier=0,
                           allow_small_or_imprecise_dtypes=True)

            # per-chunk JIT input loads: chunks 0/1 up front, chunk ch+1
            # during chunk ch, remainder all at once
            def load_chunk_inputs(c, cend=None):
                cend = c + 1 if cend is None else cend
                l0, l1 = geom.lo_c0[c], geom.lo_c0[cend]
                h0, h1 = geom.hi_c0[c], geom.hi_c0[cend]
                t0, t1 = geom.t0[c], geom.t0[cend]
                nc.sync.dma_start(out=ilo[:, l0:l1], in_=ilo_in[:, l0:l1])
                nc.sync.dma_start(out=ihi[:, h0:h1], in_=ihi_in[:, h0:h1])
                nc.sync.dma_start(out=drel16[:, t0:t1], in_=drel_in[:, t0:t1])
                nc.sync.dma_start(out=ae[:, t0:t1, :], in_=ae_in[:, t0:t1, :])

            nchunks = len(geom.CHUNKS)
            load_chunk_inputs(0, min(2, nchunks))
            nc.sync.dma_start(out=cst[:], in_=cst_in[:, :])
            if nchunks > 2:
                load_chunk_inputs(2, nchunks)
            tcv = geom.t0[min(2, nchunks)]
            nc.vector.tensor_copy(out=drel[:, 0:tcv], in_=drel16[:, 0:tcv])

            def epilogue(ci, accP, w0, w1):
                # batched normalize + bias + ELU over windows [w0, w1)
                # (fixed 5-window tile shapes so pool tags stay unified)
                nw = w1 - w0
                sEps = epool.tile([128, 5, H1], F32, name="sEps")
                nc.scalar.activation(out=sEps[:, 0:nw, :],
                                     in_=accP[:, w0:w1, 128:128 + H1],
                                     func=ACTF.Copy, bias=1e-30)
                rec = epool.tile([128, 5, H1], BF16, name="rec")
                with nc.allow_low_precision(reason="coef normalize in bf16"):
                    nc.vector.reciprocal(out=rec[:, 0:nw, :],
                                         in_=sEps[:, 0:nw, :])
                # b1 is host-folded into the table rows: since coefs sum
                # to 1, (sum e*(xh+b1))/s = (sum e*xh)/s + b1, landing the
                # bias before the ELU exactly as the reference does
                hB = epool.tile([128, 5, 128], BF16, name="hB")
                nc.vector.tensor_tensor(
                    out=hB[:, 0:nw, :].rearrange("p w (c h) -> p w c h", h=H1),
                    in0=accP[:, w0:w1, 0:128].rearrange(
                        "p w (c h) -> p w c h", h=H1),
                    in1=rec[:, 0:nw, :].unsqueeze(2).broadcast_to(
                        [128, nw, C1, H1]),
                    op=ALU.mult)
                # exp(min(x,0)) = exp(-relu(-x)): both steps on ACT
                tmp = epool.tile([128, 5, 128], BF16, name="tmp")
                nc.scalar.activation(out=tmp[:, 0:nw, :], in_=hB[:, 0:nw, :],
                                     func=ACTF.Relu, scale=-1.0)
                nc.scalar.activation(out=tmp[:, 0:nw, :], in_=tmp[:, 0:nw, :],
                                     func=ACTF.Exp, scale=-1.0)
                helu = hpool.tile([128, 5, 128], BF16, name="helu")
                nc.vector.tensor_scalar(out=helu[:, 0:nw, :],
                                        in0=tmp[:, 0:nw, :],
                                        scalar1=-1.0, scalar2=None, op0=ALU.add)
                nc.vector.tensor_tensor(out=helu[:, 0:nw, :],
                                        in0=helu[:, 0:nw, :],
                                        in1=hB[:, 0:nw, :], op=ALU.max)
                # layer-2 features: [h @ W2 | h @ W2A] via batched PE transpose
                gw0 = geom.w0[ci] + w0
                psT = peppool.tile([128, 5, 128], BF16, space="PSUM",
                                   name="psT")
                for wi in range(nw):
                    nc.tensor.transpose(out=psT[:, wi, :], in_=helu[:, wi, :],
                                        identity=ident)
                hT = htpool.tile([128, 5, 128], BF16, name="hT")
                nc.scalar.copy(out=hT[:, 0:nw, :], in_=psT[:, 0:nw, :])
                n1 = (nw + 1) // 2
                psA1 = peppool.tile([128, 3, FOUT + 2], F32, space="PSUM",
                                    name="psA1")
                psA2 = peppool.tile([128, 2, FOUT + 2], F32,
                                    space="PSUM", name="psA2")
                for wi in range(nw):
                    pa = psA1[:, wi, :] if wi < n1 else psA2[:, wi - n1, :]
                    nc.tensor.matmul(out=pa, lhsT=hT[:, wi, :], rhs=w2c,
                                     start=True, stop=True)
                # results land in the persistent accAll; the dumps are
                # deferred past the last gather so they never steal body
                # DMA slots from the gather stream (the launch pacer)
                nc.scalar.copy(out=accAll[:, gw0:gw0 + n1, :],
                               in_=psA1[:, 0:n1, :])
                if nw > n1:
                    nc.scalar.copy(out=accAll[:, gw0 + n1:gw0 + nw, :],
                                   in_=psA2[:, 0:nw - n1, :])

            def emit_exp(ci, cw):
                # e = exp(leaky_relu(alpha)) on ACT (alpha host-preadded);
                # emitted one chunk ahead so the in-order ACT queue never
                # stalls it behind the current chunk's psum copies
                tpc = cw * TPW
                t0 = geom.t0[ci]
                A2 = apool.tile([128, geom.TPC_MAX, H1], BF16, name="A2")
                RHS = rpool.tile([128, geom.TPC_MAX, 128 + H1], BF16,
                                 name="RHS")
                nc.scalar.activation(out=A2[:, 0:tpc, :],
                                     in_=ae[:, t0:t0 + tpc, :],
                                     func=ACTF.Prelu, alpha=NEG_SLOPE)
                nc.scalar.activation(out=RHS[:, 0:tpc, 128:128 + H1],
                                     in_=A2[:, 0:tpc, :], func=ACTF.Exp)
                return RHS

            prev = None
            RHS_cur = None
            for ci, cw in enumerate(geom.CHUNKS):
                t0 = geom.t0[ci]
                tpc = cw * TPW
                nlo_t = cw * LOT
                G = gpool.tile([128, geom.TPC_MAX, 128], BF16, name="G")
                _emit_gathers(nc, G, table_in[:, :], ilo, 0, nlo_t,
                              geom.lo_c0[ci])
                _emit_gathers(nc, G, table_in[HI_BASE:, :], ihi, nlo_t,
                              tpc - nlo_t, geom.hi_c0[ci])
                if ci == 0:
                    RHS_cur = emit_exp(0, cw)
                RHS_nxt = (emit_exp(ci + 1, geom.CHUNKS[ci + 1])
                           if ci + 1 < len(geom.CHUNKS) else None)
                RHS = RHS_cur
                if ci == 1 and len(geom.CHUNKS) > 2:
                    # bulk drel int16->f32 (its DMA landed during chunk 0)
                    nc.vector.tensor_copy(out=drel[:, geom.t0[2]:],
                                          in_=drel16[:, geom.t0[2]:])

                def emit_prev_epilogue():
                    if prev is not None:
                        pci, paccP, pcw = prev
                        for e0 in range(0, pcw, 5):
                            epilogue(pci, paccP, e0, min(e0 + 5, pcw))

                # steady state: previous chunk's epilogue first (its deps are
                # long done, so the in-order DVE queue never stalls on it and
                # it fills DVE while this chunk's gathers land). During the
                # fill (ci < 4) deps complete in order S -> epilogue -> msgs,
                # so emit in that order instead to avoid head-of-line blocks.
                if ci >= fill_chunks:
                    emit_prev_epilogue()

                def emit_msg(lo0, n):
                    # msg = xh[src] * e (broadcast over channels; c-major)
                    in0 = G[:, lo0:lo0 + n, :].rearrange(
                        "p t (c h) -> p t c h", h=H1)
                    in1 = RHS[:, lo0:lo0 + n, 128:128 + H1].unsqueeze(
                        2).broadcast_to([128, n, C1, H1])
                    out0 = RHS[:, lo0:lo0 + n, 0:128].rearrange(
                        "p t (c h) -> p t c h", h=H1)
                    nc.vector.tensor_tensor(out=out0, in0=in0, in1=in1,
                                            op=ALU.mult)

                def build_S(wi, t):
                    gl = _tile_of(ci, cw, wi, t, 0)
                    S = spool.tile([128, 128], BF16, name="S")
                    eng = (nc.gpsimd if (wi == cw - 1 and t < pool_s)
                           else nc.vector)
                    eng.tensor_scalar(
                        out=S[:], in0=iota,
                        scalar1=drel[:, t0 + gl:t0 + gl + 1], scalar2=None,
                        op0=ALU.is_equal)
                    return (gl, S)

                accP = accppool.tile([128, max(geom.CHUNKS), 128 + H1], BF16,
                                     name="accP")
                if ci < fill_chunks:
                    # warm-up chunks: S builds first (no gather dep), so DVE
                    # starts ~2.5us before the first gather lands
                    Sw = [[build_S(wi, t) for t in range(TPW)]
                          for wi in range(cw)]
                    emit_prev_epilogue()
                    for m0 in range(0, tpc, GATHER_TILES):
                        emit_msg(m0, min(GATHER_TILES, tpc - m0))
                    for wi in range(cw):
                        psum = ppool.tile([128, 128 + H1], F32, space="PSUM",
                                          name="psum")
                        for t, (gl, S) in enumerate(Sw[wi]):
                            nc.tensor.matmul(out=psum[:], lhsT=S[:],
                                             rhs=RHS[:, gl, :],
                                             start=(t == 0),
                                             stop=(t == TPW - 1))
                        nc.scalar.copy(out=accP[:, wi, :], in_=psum[:])
                else:
                    for m0 in range(0, nlo_t, GATHER_TILES):
                        emit_msg(m0, min(GATHER_TILES, nlo_t - m0))
                    for wi in range(cw):
                        Ss = [build_S(wi, t) for t in range(TPW)]
                        if wi == 0:
                            for m0 in range(nlo_t, tpc, GATHER_TILES):
                                emit_msg(m0, min(GATHER_TILES, tpc - m0))
                        psum = ppool.tile([128, 128 + H1], F32, space="PSUM",
                                          name="psum")
                        for t, (gl, S) in enumerate(Ss):
                            nc.tensor.matmul(out=psum[:], lhsT=S[:],
                                             rhs=RHS[:, gl, :],
                                             start=(t == 0),
                                             stop=(t == TPW - 1))
                        nc.scalar.copy(out=accP[:, wi, :], in_=psum[:])
                prev = (ci, accP, cw)
                RHS_cur = RHS_nxt
            pci, paccP, pcw = prev
            for e0 in range(0, pcw, 5):
                epilogue(pci, paccP, e0, min(e0 + 5, pcw))
            # deferred output dumps: pieces ordered by epilogue completion
            cuts = [0, W // 3, 2 * W // 3, geom.w0[-3], geom.w0[-2], W]
            for a, b in zip(cuts, cuts[1:]):
                if b > a:
                    nc.sync.dma_start(
                        out=dump_out[:, a * (FOUT + 2):b * (FOUT + 2)],
                        in_=accAll[:, a:b, :])
    nc.compile()
    return nc


def build_E2(geom):
    W, NTILES = geom.WINS, geom.NTILES
    LO_COLS = W * LOT * 128 // 16
    HI_COLS = W * HIT * 128 // 16
    nc = _new_nc()
    table_in = nc.declare_dram_parameter("table", [N, 256], BF16, isOutput=False)
    ae_in = nc.declare_dram_parameter("ae", [128, NTILES, 1], BF16,
                                      isOutput=False)
    ilo_in = nc.declare_dram_parameter("ilo", [128, LO_COLS], I16,
                                       isOutput=False)
    ihi_in = nc.declare_dram_parameter("ihi", [128, HI_COLS], I16,
                                       isOutput=False)
    drel_in = nc.declare_dram_parameter("drel", [128, NTILES], I16, isOutput=False)

    dump_out = nc.declare_dram_parameter("dump", [128, W * FOUT], BF16,
                                         isOutput=True)

    with tile.TileContext(nc) as tc:
        with (
            tc.tile_pool(name="const", bufs=1) as cpool,
            tc.tile_pool(name="gat", bufs=2) as gpool,
            tc.tile_pool(name="alp", bufs=2) as apool,
            tc.tile_pool(name="sel", bufs=24) as spool,
            tc.tile_pool(name="psw", bufs=7, space="PSUM") as ppool,
            tc.tile_pool(name="agg", bufs=2) as aggpool,
            tc.tile_pool(name="rc", bufs=4) as rcpool,
        ):
            ilo = cpool.tile([128, LO_COLS], I16)
            ihi = cpool.tile([128, HI_COLS], I16)
            drel = cpool.tile([128, NTILES], F32)
            drel16 = cpool.tile([128, NTILES], I16)
            ae = cpool.tile([128, NTILES, 1], BF16)
            accAll = cpool.tile([128, W, FOUT + 2], BF16)
            iota_t = cpool.tile([128, 128], BF16)
            iota = iota_t[:]
            nc.gpsimd.iota(out=iota, pattern=[[1, 128]], base=0,
                           channel_multiplier=0,
                           allow_small_or_imprecise_dtypes=True)

            def load_chunk_inputs(c, cend=None):
                cend = c + 1 if cend is None else cend
                l0, l1 = geom.lo_c0[c], geom.lo_c0[cend]
                h0, h1 = geom.hi_c0[c], geom.hi_c0[cend]
                t0, t1 = geom.t0[c], geom.t0[cend]
                nc.sync.dma_start(out=ilo[:, l0:l1], in_=ilo_in[:, l0:l1])
                nc.sync.dma_start(out=ihi[:, h0:h1], in_=ihi_in[:, h0:h1])
                nc.sync.dma_start(out=drel16[:, t0:t1], in_=drel_in[:, t0:t1])
                nc.sync.dma_start(out=ae[:, t0:t1, :], in_=ae_in[:, t0:t1, :])

            nchunks = len(geom.CHUNKS)
            load_chunk_inputs(0, min(2, nchunks))
            if nchunks > 2:
                load_chunk_inputs(2, nchunks)
            tcv = geom.t0[min(2, nchunks)]
            nc.vector.tensor_copy(out=drel[:, 0:tcv], in_=drel16[:, 0:tcv])

            for ci, cw in enumerate(geom.CHUNKS):
                t0 = geom.t0[ci]
                tpc = cw * TPW
                nlo_t = cw * LOT
                if ci == 1 and len(geom.CHUNKS) > 2:
                    nc.vector.tensor_copy(out=drel[:, geom.t0[2]:],
                                          in_=drel16[:, geom.t0[2]:])
                G = gpool.tile([128, geom.TPC_MAX, 256], BF16, name="G")
                _emit_gathers_il(nc, G, table_in[:, :], table_in[HI_BASE:, :],
                                 ilo, ihi, nlo_t, tpc - nlo_t,
                                 geom.lo_c0[ci], geom.hi_c0[ci])
                A = apool.tile([128, geom.TPC_MAX, 1], BF16, name="A")
                A2 = apool.tile([128, geom.TPC_MAX, 1], F32, name="A2")
                nc.scalar.activation(out=A[:, 0:tpc, :],
                                     in_=ae[:, t0:t0 + tpc, :],
                                     func=ACTF.Prelu, alpha=NEG_SLOPE)
                nc.scalar.activation(out=A2[:, 0:tpc, :], in_=A[:, 0:tpc, :],
                                     func=ACTF.Exp)
                aggN = aggpool.tile([128, max(geom.CHUNKS), FOUT], BF16,
                                    name="aggN")
                for wi in range(cw):
                    Ss = []
                    for t in range(TPW):
                        g = _tile_of(ci, cw, wi, t, 0)
                        S = spool.tile([128, 128], BF16, name="S")
                        nc.vector.tensor_scalar(
                            out=S[:], in0=iota,
                            scalar1=drel[:, t0 + g:t0 + g + 1],
                            scalar2=A2[:, g, 0:1],
                            op0=ALU.is_equal, op1=ALU.mult)
                        Ss.append((g, S))
                    psum = ppool.tile([128, 129], F32, space="PSUM",
                                      name="psum")
                    for t, (g, S) in enumerate(Ss):
                        nc.tensor.matmul(out=psum[:], lhsT=S[:],
                                         rhs=G[:, g, 0:129],
                                         start=(t == 0), stop=(t == TPW - 1))
                    # out = agg / s: fold 1/s into the ACT psum copy as a
                    # per-partition scale
                    sEps = rcpool.tile([128, 1], F32, name="sEps")
                    nc.scalar.activation(out=sEps[:], in_=psum[:, 128:129],
                                         func=ACTF.Copy, bias=1e-30)
                    rec = rcpool.tile([128, 1], F32, name="rec")
                    nc.vector.reciprocal(out=rec[:], in_=sEps[:])
                    nc.scalar.activation(out=aggN[:, wi, :], in_=psum[:, 0:128],
                                         func=ACTF.Copy, scale=rec[:])
                # b2 is host-folded into the table rows (coefs sum to 1)
                c0 = geom.w0[ci] * FOUT
                c1 = geom.w0[ci + 1] * FOUT
                nc.sync.dma_start(out=dump_out[:, c0:c1],
                                  in_=aggN[:, 0:cw, :])
    nc.compile()
    return nc


# ----------------------------------------------------------------------------
# Host orchestration
# ----------------------------------------------------------------------------

def _run(nc, in_maps, tag):
    trace = os.environ.get("KERNEL_TRACE", "0") == "1"
    res = run_bass_kernel_spmd(nc, in_maps, list(range(NCORES)), trace=trace)
    if trace:
        _CACHE.setdefault("profiles", {})[tag] = res
    return res.results


def _expand_ae(cores, a_src, a_dst):
    """Host-expanded per-slot alpha = a_src[src] + a_dst[dst] per core."""
    a_src = a_src.astype(np.float32)
    a_dst = a_dst.astype(np.float32)
    return [np.ascontiguousarray(
        (a_src[cd["slot_src"]] + a_dst[cd["slot_dst"]]).astype(BF))
        for cd in cores]


def kernel(x, src, dst, W1, att_src1, att_dst1, b1, W2, att_src2, att_dst2, b2):
    x = np.asarray(x, np.float32)
    src = np.asarray(src, np.int64)
    dst = np.asarray(dst, np.int64)
    W1 = np.asarray(W1, np.float32)
    W2 = np.asarray(W2, np.float32)
    att_src1 = np.asarray(att_src1, np.float32)
    att_dst1 = np.asarray(att_dst1, np.float32)
    att_src2 = np.asarray(att_src2, np.float32)
    att_dst2 = np.asarray(att_dst2, np.float32)
    b1 = np.asarray(b1, np.float32)
    b2 = np.asarray(b2, np.float32)

    ekey = ("edges", hash(src.tobytes()), hash(dst.tobytes()))
    if ekey not in _CACHE:
        plan = _plan_all(src, dst)
        W = plan[4]
        geom1 = Geom(W, _e1_chunks(W))
        geom2 = Geom(W)
        _CACHE[ekey] = (geom1, geom2, _fill_cores(plan, geom1),
                        _fill_cores(plan, geom2))
    geom1, geom2, cores1, cores2 = _CACHE[ekey]

    pkey = ("progs_geom", geom1.WINS, tuple(geom1.CHUNKS),
            tuple(geom2.CHUNKS))
    if pkey not in _CACHE:
        _CACHE[pkey] = (build_T(), build_E1(geom1), build_E2(geom2))
        _CACHE["progs"] = _CACHE[pkey]
    ncT, ncE1, ncE2 = _CACHE[pkey]

    perm = _perm_cmajor()
    W1P = np.ascontiguousarray(W1[:, perm])
    W1A_src = np.einsum("fhc,hc->fh", W1.reshape(FIN, H1, C1), att_src1)
    W1A_dst = np.einsum("fhc,hc->fh", W1.reshape(FIN, H1, C1), att_dst1)
    WT = np.concatenate([W1P, W1A_src, W1A_dst], axis=1).astype(BF)  # [128,144]
    b1P = b1[perm].astype(np.float32)
    W2P = np.ascontiguousarray(W2[perm, :])
    att2cat = np.stack([att_src2[0], att_dst2[0]], axis=1).astype(np.float32)
    W2A = (W2P @ att2cat).astype(np.float32)  # [128, 2] in permuted row space
    W2C = np.concatenate([W2P, W2A], axis=1).astype(BF)  # [128, 130]

    ident = np.eye(128, dtype=np.float32).astype(BF)
    iota = np.tile(np.arange(128, dtype=np.float32), (128, 1)).astype(BF)
    b1rep = np.tile(b1P, (128, 1)).astype(BF)
    b2rep = np.tile(b2, (128, 1)).astype(BF)
    cst1 = np.ascontiguousarray(
        np.concatenate([ident, W2C], axis=1))               # [128, 258]

    # ---- Launch T: per-core [xh | a_src | a_dst] -------------------------
    xtpad = np.zeros((NCORES, 128, NPC_PAD), BF)
    for c in range(NCORES):
        xtpad[c, :, :NPC] = x[c * NPC:(c + 1) * NPC].T.astype(BF)
    in_maps = [{"xt": xtpad[c], "w": WT} for c in range(NCORES)]
    resT = _run(ncT, in_maps, "T")
    parts = []
    for c in range(NCORES):
        d = resT[c]["dump"].reshape(128, NT_T, TCOLS)
        parts.append(d.transpose(1, 0, 2).reshape(NPC_PAD, TCOLS)[:NPC])
    ta = np.concatenate(parts)                      # [N, 144] bf16
    # fold b1 into the rows: (sum e*(xh+b1))/s = (sum e*xh)/s + b1
    table1 = np.ascontiguousarray(
        (ta[:, 0:FMID].astype(np.float32) + b1P).astype(BF))  # [N, 128]
    a1_src = np.ascontiguousarray(ta[:, FMID:FMID + H1])
    a1_dst = np.ascontiguousarray(ta[:, FMID + H1:FMID + 2 * H1])
    ae1 = _expand_ae(cores1, a1_src, a1_dst)

    # ---- Launch E1 --------------------------------------------------------
    in_maps = [{"table": table1, "ae": ae1[c], "ilo": cores1[c]["idx_lo"],
                "ihi": cores1[c]["idx_hi"], "drel": cores1[c]["drel"],
                "cst": cst1}
               for c in range(NCORES)]
    resE1 = _run(ncE1, in_maps, "E1")
    ha = np.zeros((N, FOUT + 2), BF)
    for c in range(NCORES):
        d = resE1[c]["dump"].reshape(128, geom1.WINS, FOUT + 2)
        for w, (base, nd) in enumerate(cores1[c]["wmap"]):
            if nd:
                ha[base:base + nd] = d[0:nd, w, :]
    table2 = np.zeros((N, 256), BF)                 # [xh2+b2 | 1.0 | pad]
    table2[:, 0:FOUT] = (ha[:, 0:FOUT].astype(np.float32) + b2).astype(BF)
    table2[:, FOUT] = BF(1.0)
    a2_src = np.ascontiguousarray(ha[:, FOUT:FOUT + 1])
    a2_dst = np.ascontiguousarray(ha[:, FOUT + 1:FOUT + 2])
    ae2 = _expand_ae(cores2, a2_src, a2_dst)

    # ---- Launch E2 --------------------------------------------------------
    in_maps = [{"table": table2, "ae": ae2[c], "ilo": cores2[c]["idx_lo"],
                "ihi": cores2[c]["idx_hi"], "drel": cores2[c]["drel"]}
               for c in range(NCORES)]
    resE2 = _run(ncE2, in_maps, "E2")
    out = np.zeros((N, FOUT), np.float32)
    for c in range(NCORES):
        d = resE2[c]["dump"].reshape(128, geom2.WINS, FOUT).astype(np.float32)
        for w, (base, nd) in enumerate(cores2[c]["wmap"]):
            if nd:
                out[base:base + nd] = d[0:nd, w, :]
    return np.ascontiguousarray(out)


# revision 33
# speedup vs baseline: 1.0066x; 1.0025x over previous
"""GAT 2-layer kernel for 8 Trainium2 NeuronCores (bf16 pipeline).

Strategy (edge-parallel over dst-sorted edges, node-range sharded): host
appends self-loops, sorts edges by dst, gives each core a contiguous 6250-dst
range. Dsts are greedily packed into windows of <=128 dsts whose edges fit a
fixed 9-tile budget (4 "lo" + 5 "hi" tiles of 128 slots, split by src index so
int16 dma_gather indices reach the whole node table). The per-window dst
RANGES vary per core (host data) while the tile geometry is shared, so one
SPMD program serves all 8 cores with ~6% fewer gathered slots than a fixed
10-tile layout. Per-slot attention logits alpha = a_src[src] + a_dst[dst] are
host-expanded (bf16), like all index prep.

  - Launch T: [xh | a_src | a_dst] = x^T-tiles @ [W1P | W1A] per core from a
    host-pretransposed bf16 xT; psums grouped 3 tiles per bank, psum->SBUF
    copies alternate ACT/DVE, one DMA in / six piece DMAs out.
  - Launch E1 (heads=8): small chunks (2-4 windows, DVE-paced); per chunk,
    dma_gather of bf16 xh rows (256B); e = exp(leaky(alpha)) on ACT;
    msg = xh[src] * e on
    DVE (2x, c-major head broadcast); one-hot S per tile (tensor_scalar
    is_equal, 4x); segment sums via S^T @ [msg | e] matmuls accumulated in
    PSUM; ACT copies psums to a bf16 chunk buffer; the normalize + bias + ELU
    epilogue runs batched (in <=5-window halves for PSUM pressure) one chunk
    behind (software pipelining); batched PE transposes + [h@W2 | h@W2A]
    matmuls; per-chunk output DMAs. Tapered tail chunks keep the pipeline
    drain after the last gather short. Host reassembles the layer-2 table
    between launches.
  - Launch E2 (heads=1): e2 is folded into the selection matrix (S_e = e2 *
    one_hot via fused is_equal+mult), the gathered 512B rows carry a trailing
    1.0 so one matmul yields [agg | s]; divide-by-s is fused into the ACT psum
    copy as a per-partition scale; + b2; window-major dump, host scatters rows
    back to node order.

Sharding note (vs the edge-parallel hint): edges are sharded by dst range so
all segment reductions stay core-local in PSUM - no cross-core all-reduce is
needed; the small weights are folded/replicated on the host side.
"""

import os
import sys

sys.path.insert(0, "/opt/trn_rl_repo")

import numpy as np
import ml_dtypes

import concourse.bass as bass
import concourse.bacc as bacc
import concourse.mybir as mybir
import concourse.tile as tile
from concourse.bass_utils import run_bass_kernel_spmd

F32 = mybir.dt.float32
BF16 = mybir.dt.bfloat16
I16 = mybir.dt.int16
ALU = mybir.AluOpType
ACTF = mybir.ActivationFunctionType
BF = ml_dtypes.bfloat16

# Problem constants (hardcoded per harness contract).
N = 50000
E = 400000
FIN = 128
H1, C1 = 8, 16          # layer-1 heads / channels
FMID = H1 * C1          # 128
FOUT = 128
NEG_SLOPE = 0.2

NCORES = 8
NPC = N // NCORES       # 6250 nodes per core
LOT = 4                 # lo tiles per window (src < 32768 reachable)
HIT = 5                 # hi tiles per window (src >= HI_BASE reachable)
TPW = LOT + HIT         # 9 tiles of 128 slots per window
SENT = -1               # sentinel dst_rel for padding slots
HI_BASE = N - 32768     # 17232: hi gather covers rows [HI_BASE, N)
NT_T = (NPC + 127) // 128  # x tiles per core in launch T (49)
NPC_PAD = NT_T * 128
TCOLS = FMID + 2 * H1   # 144: [xh | a_src | a_dst] in launch T

GATHER_TILES = 8        # tiles (128 idxs each) per dma_gather call
                        # (1024 idxs = 64 descs/engine packet, HW limit)

_CACHE = {}


# ----------------------------------------------------------------------------
# Host-side graph preprocessing
# ----------------------------------------------------------------------------

def _wrap16(idx):
    """int16 index array [n] -> dma_gather wrapped layout [16, n//16]."""
    n = idx.shape[0]
    return np.ascontiguousarray(idx.reshape(n // 16, 16).T.astype(np.int16))


class Geom:
    """Shared launch geometry: W windows of TPW tiles, chunk window counts."""

    def __init__(self, wins, chunks=None):
        self.WINS = wins
        if chunks is None:
            full, rem = divmod(wins, 10)
            chunks = [10] * full + ([rem] if rem else [])
            if chunks[-1] > 4:                  # short drain after last gather
                chunks = chunks[:-1] + [chunks[-1] - 3, 3]
        assert sum(chunks) == wins
        self.CHUNKS = chunks
        self.NTILES = wins * TPW
        self.TPC_MAX = max(chunks) * TPW
        # cumulative offsets per chunk (windows / tiles / lo+hi idx columns)
        w0 = [0]
        for cw in chunks:
            w0.append(w0[-1] + cw)
        self.w0 = w0
        self.t0 = [w * TPW for w in w0]
        self.lo_c0 = [w * LOT * 128 // 16 for w in w0]
        self.hi_c0 = [w * HIT * 128 // 16 for w in w0]


def _plan_windows(counts_core, ml_core, mh_core):
    """Greedy dst packing: <=128 dsts, <=LOT*128 lo slots, <=HIT*128 hi
    slots, <=TPW*128 total edges per window. Returns [(dst0, ndst)]."""
    wins = []
    n = counts_core.shape[0]
    d = 0
    cap_t, cap_l, cap_h = TPW * 128, LOT * 128, HIT * 128
    while d < n:
        d0 = d
        tot = ml = mh = 0
        while d < n and d - d0 < 128:
            k, l, h = counts_core[d], ml_core[d], mh_core[d]
            if tot + k > cap_t or ml + l > cap_l or mh + h > cap_h:
                break
            tot += k
            ml += l
            mh += h
            d += 1
        assert d > d0, "single dst exceeds window caps"
        wins.append((d0, d - d0))
    return wins


def _e1_chunks(wins):
    """Fine-grained chunks (4 windows) with small warm-up and a single-window
    final chunk: E1's post-gather drain is the last chunk's compute plus its
    epilogue chain, so the smallest possible tail wins."""
    rem = wins - 10
    assert rem > 0
    return [2, 3] + [4] * (rem // 4) + ([rem % 4] if rem % 4 else []) + [4, 1]


def _plan_all(src, dst):
    """Sort edges by dst, plan shared windows. Returns the plan tuple."""
    s_all = np.concatenate([src, np.arange(N, dtype=np.int64)])
    d_all = np.concatenate([dst, np.arange(N, dtype=np.int64)])
    order = np.argsort(d_all, kind="stable")
    s_all = s_all[order]
    d_all = d_all[order]
    counts = np.bincount(d_all, minlength=N)
    starts = np.concatenate([[0], np.cumsum(counts)])
    # per-dst mandatory-lo / mandatory-hi counts
    ml_all = np.bincount(d_all[s_all < HI_BASE], minlength=N)
    mh_all = np.bincount(d_all[s_all >= 32768], minlength=N)

    core_wins = []
    for c in range(NCORES):
        r = slice(c * NPC, (c + 1) * NPC)
        wins = _plan_windows(counts[r], ml_all[r], mh_all[r])
        core_wins.append([(c * NPC + d0, nd) for d0, nd in wins])
    W = max(len(w) for w in core_wins)
    return (s_all, d_all, starts, core_wins, W)


def _fill_cores(plan, geom):
    """Per-core device index arrays + host slot maps for one chunking."""
    s_all, d_all, starts, core_wins, W = plan
    cores = []
    for c in range(NCORES):
        wmap = core_wins[c] + [(c * NPC, 0)] * (W - len(core_wins[c]))
        slot_src = np.zeros((geom.NTILES, 128), np.int64)
        slot_dst = np.zeros((geom.NTILES, 128), np.int64)
        slot_rel = np.full((geom.NTILES, 128), SENT, np.int64)
        ilo_cols = []
        ihi_cols = []
        for ci, cw in enumerate(geom.CHUNKS):
            lo_flat = np.zeros(cw * LOT * 128, np.int64)
            hi_flat = np.zeros(cw * HIT * 128, np.int64)
            for wi in range(cw):
                w = geom.w0[ci] + wi
                base, nd = wmap[w]
                e0, e1 = starts[base], starts[base + nd]
                ss, dd = s_all[e0:e1], d_all[e0:e1]
                must_lo = ss < HI_BASE
                must_hi = ss >= 32768
                free = ~must_lo & ~must_hi
                cap_lo = LOT * 128
                n_lo = min(cap_lo, int(e1 - e0) - int(must_hi.sum()))
                sel_lo = must_lo.copy()
                free_idx = np.where(free)[0]
                sel_lo[free_idx[:n_lo - int(must_lo.sum())]] = True
                sel_hi = ~sel_lo
                nl, nh = int(sel_lo.sum()), int(sel_hi.sum())
                assert nl <= cap_lo and nh <= HIT * 128, (nl, nh)
                # lo block
                ls = np.zeros(cap_lo, np.int64)
                ld = np.full(cap_lo, base, np.int64)
                lr = np.full(cap_lo, SENT, np.int64)
                ls[:nl] = ss[sel_lo]
                ld[:nl] = dd[sel_lo]
                lr[:nl] = dd[sel_lo] - base
                lo_flat[wi * cap_lo:(wi + 1) * cap_lo] = ls
                g0 = geom.t0[ci] + wi * LOT
                slot_src[g0:g0 + LOT] = ls.reshape(LOT, 128)
                slot_dst[g0:g0 + LOT] = ld.reshape(LOT, 128)
                slot_rel[g0:g0 + LOT] = lr.reshape(LOT, 128)
                # hi block
                cap_hi = HIT * 128
                hs = np.full(cap_hi, HI_BASE, np.int64)
                hd = np.full(cap_hi, base, np.int64)
                hr = np.full(cap_hi, SENT, np.int64)
                hs[:nh] = ss[sel_hi]
                hd[:nh] = dd[sel_hi]
                hr[:nh] = dd[sel_hi] - base
                hi_flat[wi * cap_hi:(wi + 1) * cap_hi] = hs
                g1 = geom.t0[ci] + cw * LOT + wi * HIT
                slot_src[g1:g1 + HIT] = hs.reshape(HIT, 128)
                slot_dst[g1:g1 + HIT] = hd.reshape(HIT, 128)
                slot_rel[g1:g1 + HIT] = hr.reshape(HIT, 128)
            ilo_cols.append(_wrap16(lo_flat))
            ihi_cols.append(_wrap16(hi_flat - HI_BASE))
        idx_lo = np.concatenate(ilo_cols, axis=1)
        idx_hi = np.concatenate(ihi_cols, axis=1)
        cores.append({
            "idx_lo": np.ascontiguousarray(np.tile(idx_lo, (8, 1))),
            "idx_hi": np.ascontiguousarray(np.tile(idx_hi, (8, 1))),
            "slot_src": np.ascontiguousarray(slot_src.T),   # [128, n_tiles]
            "slot_dst": np.ascontiguousarray(slot_dst.T),
            "drel": np.ascontiguousarray(slot_rel.T.astype(np.int16)),
            "wmap": wmap,
        })
    return cores


def _perm_cmajor():
    """Column permutation h*16+c -> c*8+h for layer-1 features."""
    p = np.zeros(FMID, np.int64)
    for h in range(H1):
        for c in range(C1):
            p[c * H1 + h] = h * C1 + c
    return p


# ----------------------------------------------------------------------------
# Bass program builders
# ----------------------------------------------------------------------------

def _new_nc():
    return bacc.Bacc("TRN2", target_bir_lowering=False, debug=False,
                     num_devices=NCORES)


def build_T():
    """Table launch: [xh | a_src | a_dst] = xT^T @ [W1P | W1A] per core."""
    nc = _new_nc()
    xt_in = nc.declare_dram_parameter("xt", [128, NPC_PAD], BF16, isOutput=False)
    w_in = nc.declare_dram_parameter("w", [FIN, TCOLS], BF16, isOutput=False)
    dump_out = nc.declare_dram_parameter("dump", [128, NT_T * TCOLS], BF16,
                                         isOutput=True)

    with tile.TileContext(nc) as tc:
        with (
            tc.tile_pool(name="const", bufs=1) as cpool,
            tc.tile_pool(name="ps", bufs=6, space="PSUM") as pspool,
        ):
            xt = cpool.tile([128, NPC_PAD], BF16)
            w = cpool.tile([FIN, TCOLS], BF16)
            acc = cpool.tile([128, NT_T, TCOLS], BF16)
            # split xT load so tile-0 compute starts early
            nc.sync.dma_start(out=w[:], in_=w_in[:, :])
            q = [0, 6 * 128, 18 * 128, 34 * 128, NPC_PAD]
            for i in range(4):
                nc.sync.dma_start(out=xt[:, q[i]:q[i + 1]],
                                  in_=xt_in[:, q[i]:q[i + 1]])
            for g0 in range(0, NT_T, 3):
                gn = min(3, NT_T - g0)
                ps = pspool.tile([128, 3, TCOLS], F32, space="PSUM")
                for j in range(gn):
                    t = g0 + j
                    nc.tensor.matmul(out=ps[:, j, :],
                                     lhsT=xt[:, t * 128:(t + 1) * 128],
                                     rhs=w[:], start=True, stop=True)
                if (g0 // 3) % 2 == 0:
                    nc.scalar.copy(out=acc[:, g0:g0 + gn, :], in_=ps[:, 0:gn, :])
                else:
                    nc.vector.tensor_copy(out=acc[:, g0:g0 + gn, :],
                                          in_=ps[:, 0:gn, :])
                if g0 + gn in (9, 18, 27, 36, 42, NT_T):
                    marks = [0, 9, 18, 27, 36, 42, NT_T]
                    d0 = marks[marks.index(g0 + gn) - 1] * TCOLS
                    d1 = (g0 + gn) * TCOLS
                    nc.sync.dma_start(out=dump_out[:, d0:d1],
                                      in_=acc[:, d0 // TCOLS:(g0 + gn), :])
    nc.compile()
    return nc


def _emit_gathers(nc, G, table_in, idx, base_tile, n_tiles, idx_col0):
    done = 0
    while done < n_tiles:
        piece = min(GATHER_TILES, n_tiles - done)
        nidx = piece * 128
        c0 = idx_col0 + done * 8
        nc.gpsimd.dma_gather(
            out_ap=G[:, base_tile + done:base_tile + done + piece, :],
            in_ap=table_in, idxs_ap=idx[:, c0:c0 + nidx // 16],
            num_idxs=nidx, num_idxs_reg=nidx,
            elem_size=table_in.shape[-1])
        done += piece


def _emit_gathers_il(nc, G, lo_ap, hi_ap, ilo, ihi, nlo_t, nhi_t,
                     lo_c0, hi_c0):
    """Interleave lo/hi gather pieces so each window's full tile set (its lo
    AND hi block) lands as early as possible."""
    lo_done = hi_done = 0
    while lo_done < nlo_t or hi_done < nhi_t:
        for ap, idx, done, n_t, c0, base in (
                (lo_ap, ilo, lo_done, nlo_t, lo_c0, 0),
                (hi_ap, ihi, hi_done, nhi_t, hi_c0, nlo_t)):
            if done >= n_t:
                continue
            piece = min(GATHER_TILES, n_t - done)
            nidx = piece * 128
            cc = c0 + done * 8
            nc.gpsimd.dma_gather(
                out_ap=G[:, base + done:base + done + piece, :],
                in_ap=ap, idxs_ap=idx[:, cc:cc + nidx // 16],
                num_idxs=nidx, num_idxs_reg=nidx,
                elem_size=ap.shape[-1])
        lo_done = min(nlo_t, lo_done + GATHER_TILES)
        hi_done = min(nhi_t, hi_done + GATHER_TILES)


def _tile_of(ci, cw, wi, t, t0):
    """Global tile id for tile t of window wi in chunk ci (lo block first)."""
    if t < LOT:
        return t0 + wi * LOT + t
    return t0 + cw * LOT + wi * HIT + (t - LOT)


def build_E1(geom, deep_bufs=False, pool_s=0, fill_chunks=6):
    W, NTILES = geom.WINS, geom.NTILES
    LO_COLS = W * LOT * 128 // 16
    HI_COLS = W * HIT * 128 // 16
    nc = _new_nc()
    table_in = nc.declare_dram_parameter("table", [N, 128], BF16, isOutput=False)
    ae_in = nc.declare_dram_parameter("ae", [128, NTILES, H1], BF16,
                                      isOutput=False)
    ilo_in = nc.declare_dram_parameter("ilo", [128, LO_COLS], I16,
                                       isOutput=False)
    ihi_in = nc.declare_dram_parameter("ihi", [128, HI_COLS], I16,
                                       isOutput=False)
    drel_in = nc.declare_dram_parameter("drel", [128, NTILES], I16, isOutput=False)
    cst_in = nc.declare_dram_parameter("cst", [128, 258], BF16, isOutput=False)
    dump_out = nc.declare_dram_parameter("dump", [128, W * (FOUT + 2)], BF16,
                                         isOutput=True)

    with tile.TileContext(nc) as tc:
        with (
            tc.tile_pool(name="const", bufs=1) as cpool,
            tc.tile_pool(name="gat", bufs=3 if deep_bufs else 2) as gpool,
            tc.tile_pool(name="alp", bufs=3 if deep_bufs else 2) as apool,
            tc.tile_pool(name="rhs", bufs=3 if deep_bufs else 2) as rpool,
            tc.tile_pool(name="sel", bufs=48) as spool,
            tc.tile_pool(name="psw", bufs=2, space="PSUM") as ppool,
            tc.tile_pool(name="accp", bufs=2) as accppool,
            tc.tile_pool(name="acca", bufs=2) as accapool,
            tc.tile_pool(name="epi", bufs=2) as epool,
            tc.tile_pool(name="hel", bufs=2) as hpool,
            tc.tile_pool(name="ht", bufs=2) as htpool,
            tc.tile_pool(name="psep", bufs=1, space="PSUM") as peppool,
        ):
            ilo = cpool.tile([128, LO_COLS], I16)
            ihi = cpool.tile([128, HI_COLS], I16)
            drel = cpool.tile([128, NTILES], F32)
            drel16 = cpool.tile([128, NTILES], I16)
            ae = cpool.tile([128, NTILES, H1], BF16)
            cst = cpool.tile([128, 258], BF16)
            ident = cst[:, 0:128]
            w2c = cst[:, 128:258]
            accAll = cpool.tile([128, W, FOUT + 2], BF16)
            iota_t = cpool.tile([128, 128], BF16)
            iota = iota_t[:]
            # iota built on-device (Pool, ~0.2us): the one-hot S builds then
            # depend only on drel, starting ~1us earlier in the fill
            nc.gpsimd.iota(out=iota, pattern=[[1, 128]], base=0,
                           channel_multiplier=0,
                           allow_small_or_imprecise_dtypes=True)

            # per-chunk JIT input loads: chunks 0/1 up front, chunk ch+1
            # during chunk ch, remainder all at once
            def load_chunk_inputs(c, cend=None):
                cend = c + 1 if cend is None else cend
                l0, l1 = geom.lo_c0[c], geom.lo_c0[cend]
                h0, h1 = geom.hi_c0[c], geom.hi_c0[cend]
                t0, t1 = geom.t0[c], geom.t0[cend]
                nc.sync.dma_start(out=ilo[:, l0:l1], in_=ilo_in[:, l0:l1])
                nc.sync.dma_start(out=ihi[:, h0:h1], in_=ihi_in[:, h0:h1])
                nc.sync.dma_start(out=drel16[:, t0:t1], in_=drel_in[:, t0:t1])
                nc.sync.dma_start(out=ae[:, t0:t1, :], in_=ae_in[:, t0:t1, :])

            nchunks = len(geom.CHUNKS)
            load_chunk_inputs(0, min(2, nchunks))
            nc.sync.dma_start(out=cst[:], in_=cst_in[:, :])
            if nchunks > 2:
                load_chunk_inputs(2, nchunks)
            tcv = geom.t0[min(2, nchunks)]
            nc.vector.tensor_copy(out=drel[:, 0:tcv], in_=drel16[:, 0:tcv])

            def epilogue(ci, accP, w0, w1):
                # batched normalize + bias + ELU over windows [w0, w1)
                # (fixed 5-window tile shapes so pool tags stay unified)
                nw = w1 - w0
                sEps = epool.tile([128, 5, H1], F32, name="sEps")
                nc.scalar.activation(out=sEps[:, 0:nw, :],
                                     in_=accP[:, w0:w1, 128:128 + H1],
                                     func=ACTF.Copy, bias=1e-30)
                rec = epool.tile([128, 5, H1], BF16, name="rec")
                with nc.allow_low_precision(reason="coef normalize in bf16"):
                    nc.vector.reciprocal(out=rec[:, 0:nw, :],
                                         in_=sEps[:, 0:nw, :])
                # b1 is host-folded into the table rows: since coefs sum
                # to 1, (sum e*(xh+b1))/s = (sum e*xh)/s + b1, landing the
                # bias before the ELU exactly as the reference does
                hB = epool.tile([128, 5, 128], BF16, name="hB")
                nc.vector.tensor_tensor(
                    out=hB[:, 0:nw, :].rearrange("p w (c h) -> p w c h", h=H1),
                    in0=accP[:, w0:w1, 0:128].rearrange(
                        "p w (c h) -> p w c h", h=H1),
                    in1=rec[:, 0:nw, :].unsqueeze(2).broadcast_to(
                        [128, nw, C1, H1]),
                    op=ALU.mult)
                # exp(min(x,0)) = exp(-relu(-x)): both steps on ACT
                tmp = epool.tile([128, 5, 128], BF16, name="tmp")
                nc.scalar.activation(out=tmp[:, 0:nw, :], in_=hB[:, 0:nw, :],
                                     func=ACTF.Relu, scale=-1.0)
                nc.scalar.activation(out=tmp[:, 0:nw, :], in_=tmp[:, 0:nw, :],
                                     func=ACTF.Exp, scale=-1.0)
                helu = hpool.tile([128, 5, 128], BF16, name="helu")
                nc.vector.tensor_scalar(out=helu[:, 0:nw, :],
                                        in0=tmp[:, 0:nw, :],
                                        scalar1=-1.0, scalar2=None, op0=ALU.add)
                nc.vector.tensor_tensor(out=helu[:, 0:nw, :],
                                        in0=helu[:, 0:nw, :],
                                        in1=hB[:, 0:nw, :], op=ALU.max)
                # layer-2 features: [h @ W2 | h @ W2A] via batched PE transpose
                gw0 = geom.w0[ci] + w0
                psT = peppool.tile([128, 5, 128], BF16, space="PSUM",
                                   name="psT")
                for wi in range(nw):
                    nc.tensor.transpose(out=psT[:, wi, :], in_=helu[:, wi, :],
                                        identity=ident)
                hT = htpool.tile([128, 5, 128], BF16, name="hT")
                nc.scalar.copy(out=hT[:, 0:nw, :], in_=psT[:, 0:nw, :])
                n1 = (nw + 1) // 2
                psA1 = peppool.tile([128, 3, FOUT + 2], F32, space="PSUM",
                                    name="psA1")
                psA2 = peppool.tile([128, 2, FOUT + 2], F32,
                                    space="PSUM", name="psA2")
                for wi in range(nw):
                    pa = psA1[:, wi, :] if wi < n1 else psA2[:, wi - n1, :]
                    nc.tensor.matmul(out=pa, lhsT=hT[:, wi, :], rhs=w2c,
                                     start=True, stop=True)
                # results land in the persistent accAll; the dumps are
                # deferred past the last gather so they never steal body
                # DMA slots from the gather stream (the launch pacer)
                nc.scalar.copy(out=accAll[:, gw0:gw0 + n1, :],
                               in_=psA1[:, 0:n1, :])
                if nw > n1:
                    nc.scalar.copy(out=accAll[:, gw0 + n1:gw0 + nw, :],
                                   in_=psA2[:, 0:nw - n1, :])

            def emit_exp(ci, cw):
                # e = exp(leaky_relu(alpha)) on ACT (alpha host-preadded);
                # emitted one chunk ahead so the in-order ACT queue never
                # stalls it behind the current chunk's psum copies
                tpc = cw * TPW
                t0 = geom.t0[ci]
                A2 = apool.tile([128, geom.TPC_MAX, H1], BF16, name="A2")
                RHS = rpool.tile([128, geom.TPC_MAX, 128 + H1], BF16,
                                 name="RHS")
                nc.scalar.activation(out=A2[:, 0:tpc, :],
                                     in_=ae[:, t0:t0 + tpc, :],
                                     func=ACTF.Prelu, alpha=NEG_SLOPE)
                nc.scalar.activation(out=RHS[:, 0:tpc, 128:128 + H1],
                                     in_=A2[:, 0:tpc, :], func=ACTF.Exp)
                return RHS

            prev = None
            RHS_cur = None
            for ci, cw in enumerate(geom.CHUNKS):
                t0 = geom.t0[ci]
                tpc = cw * TPW
                nlo_t = cw * LOT
                G = gpool.tile([128, geom.TPC_MAX, 128], BF16, name="G")
                _emit_gathers(nc, G, table_in[:, :], ilo, 0, nlo_t,
                              geom.lo_c0[ci])
                _emit_gathers(nc, G, table_in[HI_BASE:, :], ihi, nlo_t,
                              tpc - nlo_t, geom.hi_c0[ci])
                if ci == 0:
                    RHS_cur = emit_exp(0, cw)
                RHS_nxt = (emit_exp(ci + 1, geom.CHUNKS[ci + 1])
                           if ci + 1 < len(geom.CHUNKS) else None)
                RHS = RHS_cur
                if ci == 1 and len(geom.CHUNKS) > 2:
                    # bulk drel int16->f32 (its DMA landed during chunk 0)
                    nc.vector.tensor_copy(out=drel[:, geom.t0[2]:],
                                          in_=drel16[:, geom.t0[2]:])

                def emit_prev_epilogue():
                    if prev is not None:
                        pci, paccP, pcw = prev
                        for e0 in range(0, pcw, 5):
                            epilogue(pci, paccP, e0, min(e0 + 5, pcw))

                # steady state: previous chunk's epilogue first (its deps are
                # long done, so the in-order DVE queue never stalls on it and
                # it fills DVE while this chunk's gathers land). During the
                # fill (ci < 4) deps complete in order S -> epilogue -> msgs,
                # so emit in that order instead to avoid head-of-line blocks.
                if ci >= fill_chunks:
                    emit_prev_epilogue()

                def emit_msg(lo0, n):
                    # msg = xh[src] * e (broadcast over channels; c-major)
                    in0 = G[:, lo0:lo0 + n, :].rearrange(
                        "p t (c h) -> p t c h", h=H1)
                    in1 = RHS[:, lo0:lo0 + n, 128:128 + H1].unsqueeze(
                        2).broadcast_to([128, n, C1, H1])
                    out0 = RHS[:, lo0:lo0 + n, 0:128].rearrange(
                        "p t (c h) -> p t c h", h=H1)
                    nc.vector.tensor_tensor(out=out0, in0=in0, in1=in1,
                                            op=ALU.mult)

                def build_S(wi, t):
                    gl = _tile_of(ci, cw, wi, t, 0)
                    S = spool.tile([128, 128], BF16, name="S")
                    eng = (nc.gpsimd if (wi == cw - 1 and t < pool_s)
                           else nc.vector)
                    eng.tensor_scalar(
                        out=S[:], in0=iota,
                        scalar1=drel[:, t0 + gl:t0 + gl + 1], scalar2=None,
                        op0=ALU.is_equal)
                    return (gl, S)

                accP = accppool.tile([128, max(geom.CHUNKS), 128 + H1], BF16,
                                     name="accP")
                if ci < fill_chunks:
                    # warm-up chunks: S builds first (no gather dep), so DVE
                    # starts ~2.5us before the first gather lands
                    Sw = [[build_S(wi, t) for t in range(TPW)]
                          for wi in range(cw)]
                    emit_prev_epilogue()
                    for m0 in range(0, tpc, GATHER_TILES):
                        emit_msg(m0, min(GATHER_TILES, tpc - m0))
                    for wi in range(cw):
                        psum = ppool.tile([128, 128 + H1], F32, space="PSUM",
                                          name="psum")
                        for t, (gl, S) in enumerate(Sw[wi]):
                            nc.tensor.matmul(out=psum[:], lhsT=S[:],
                                             rhs=RHS[:, gl, :],
                                             start=(t == 0),
                                             stop=(t == TPW - 1))
                        nc.scalar.copy(out=accP[:, wi, :], in_=psum[:])
                else:
                    for m0 in range(0, nlo_t, GATHER_TILES):
                        emit_msg(m0, min(GATHER_TILES, nlo_t - m0))
                    for wi in range(cw):
                        Ss = [build_S(wi, t) for t in range(TPW)]
                        if wi == 0:
                            for m0 in range(nlo_t, tpc, GATHER_TILES):
                                emit_msg(m0, min(GATHER_TILES, tpc - m0))
                        psum = ppool.tile([128, 128 + H1], F32, space="PSUM",
                                          name="psum")
                        for t, (gl, S) in enumerate(Ss):
                            nc.tensor.matmul(out=psum[:], lhsT=S[:],
                                             rhs=RHS[:, gl, :],
                                             start=(t == 0),
                                             stop=(t == TPW - 1))
                        nc.scalar.copy(out=accP[:, wi, :], in_=psum[:])
                prev = (ci, accP, cw)
                RHS_cur = RHS_nxt
            pci, paccP, pcw = prev
            for e0 in range(0, pcw, 5):
                epilogue(pci, paccP, e0, min(e0 + 5, pcw))
            # deferred output dumps: pieces ordered by epilogue completion
            cuts = [0, W // 3, 2 * W // 3, geom.w0[-3], geom.w0[-2], W]
            for a, b in zip(cuts, cuts[1:]):
                if b > a:
                    nc.sync.dma_start(
                        out=dump_out[:, a * (FOUT + 2):b * (FOUT + 2)],
                        in_=accAll[:, a:b, :])
    nc.compile()
    return nc


def build_E2(geom):
    W, NTILES = geom.WINS, geom.NTILES
    LO_COLS = W * LOT * 128 // 16
    HI_COLS = W * HIT * 128 // 16
    nc = _new_nc()
    table_in = nc.declare_dram_parameter("table", [N, 256], BF16, isOutput=False)
    ae_in = nc.declare_dram_parameter("ae", [128, NTILES, 1], BF16,
                                      isOutput=False)
    ilo_in = nc.declare_dram_parameter("ilo", [128, LO_COLS], I16,
                                       isOutput=False)
    ihi_in = nc.declare_dram_parameter("ihi", [128, HI_COLS], I16,
                                       isOutput=False)
    drel_in = nc.declare_dram_parameter("drel", [128, NTILES], I16, isOutput=False)

    dump_out = nc.declare_dram_parameter("dump", [128, W * FOUT], BF16,
                                         isOutput=True)

    with tile.TileContext(nc) as tc:
        with (
            tc.tile_pool(name="const", bufs=1) as cpool,
            tc.tile_pool(name="gat", bufs=2) as gpool,
            tc.tile_pool(name="alp", bufs=2) as apool,
            tc.tile_pool(name="sel", bufs=24) as spool,
            tc.tile_pool(name="psw", bufs=7, space="PSUM") as ppool,
            tc.tile_pool(name="agg", bufs=2) as aggpool,
            tc.tile_pool(name="rc", bufs=4) as rcpool,
        ):
            ilo = cpool.tile([128, LO_COLS], I16)
            ihi = cpool.tile([128, HI_COLS], I16)
            drel = cpool.tile([128, NTILES], F32)
            drel16 = cpool.tile([128, NTILES], I16)
            ae = cpool.tile([128, NTILES, 1], BF16)
            accAll = cpool.tile([128, W, FOUT + 2], BF16)
            iota_t = cpool.tile([128, 128], BF16)
            iota = iota_t[:]
            nc.gpsimd.iota(out=iota, pattern=[[1, 128]], base=0,
                           channel_multiplier=0,
                           allow_small_or_imprecise_dtypes=True)

            def load_chunk_inputs(c, cend=None):
                cend = c + 1 if cend is None else cend
                l0, l1 = geom.lo_c0[c], geom.lo_c0[cend]
                h0, h1 = geom.hi_c0[c], geom.hi_c0[cend]
                t0, t1 = geom.t0[c], geom.t0[cend]
                nc.sync.dma_start(out=ilo[:, l0:l1], in_=ilo_in[:, l0:l1])
                nc.sync.dma_start(out=ihi[:, h0:h1], in_=ihi_in[:, h0:h1])
                nc.sync.dma_start(out=drel16[:, t0:t1], in_=drel_in[:, t0:t1])
                nc.sync.dma_start(out=ae[:, t0:t1, :], in_=ae_in[:, t0:t1, :])

            nchunks = len(geom.CHUNKS)
            load_chunk_inputs(0, min(2, nchunks))
            if nchunks > 2:
                load_chunk_inputs(2, nchunks)
            tcv = geom.t0[min(2, nchunks)]
            nc.vector.tensor_copy(out=drel[:, 0:tcv], in_=drel16[:, 0:tcv])

            for ci, cw in enumerate(geom.CHUNKS):
                t0 = geom.t0[ci]
                tpc = cw * TPW
                nlo_t = cw * LOT
                if ci == 1 and len(geom.CHUNKS) > 2:
                    nc.vector.tensor_copy(out=drel[:, geom.t0[2]:],
                                          in_=drel16[:, geom.t0[2]:])
                G = gpool.tile([128, geom.TPC_MAX, 256], BF16, name="G")
                _emit_gathers_il(nc, G, table_in[:, :], table_in[HI_BASE:, :],
                                 ilo, ihi, nlo_t, tpc - nlo_t,
                                 geom.lo_c0[ci], geom.hi_c0[ci])
                A = apool.tile([128, geom.TPC_MAX, 1], BF16, name="A")
                A2 = apool.tile([128, geom.TPC_MAX, 1], F32, name="A2")
                nc.scalar.activation(out=A[:, 0:tpc, :],
                                     in_=ae[:, t0:t0 + tpc, :],
                                     func=ACTF.Prelu, alpha=NEG_SLOPE)
                nc.scalar.activation(out=A2[:, 0:tpc, :], in_=A[:, 0:tpc, :],
                                     func=ACTF.Exp)
                aggN = aggpool.tile([128, max(geom.CHUNKS), FOUT], BF16,
                                    name="aggN")
                for wi in range(cw):
                    Ss = []
                    for t in range(TPW):
                        g = _tile_of(ci, cw, wi, t, 0)
                        S = spool.tile([128, 128], BF16, name="S")
                        nc.vector.tensor_scalar(
                            out=S[:], in0=iota,
                            scalar1=drel[:, t0 + g:t0 + g + 1],
                            scalar2=A2[:, g, 0:1],
                            op0=ALU.is_equal, op1=ALU.mult)
                        Ss.append((g, S))
                    psum = ppool.tile([128, 129], F32, space="PSUM",
                                      name="psum")
                    for t, (g, S) in enumerate(Ss):
                        nc.tensor.matmul(out=psum[:], lhsT=S[:],
                                         rhs=G[:, g, 0:129],
                                         start=(t == 0), stop=(t == TPW - 1))
                    # out = agg / s: fold 1/s into the ACT psum copy as a
                    # per-partition scale
                    sEps = rcpool.tile([128, 1], F32, name="sEps")
                    nc.scalar.activation(out=sEps[:], in_=psum[:, 128:129],
                                         func=ACTF.Copy, bias=1e-30)
                    rec = rcpool.tile([128, 1], F32, name="rec")
                    nc.vector.reciprocal(out=rec[:], in_=sEps[:])
                    nc.scalar.activation(out=aggN[:, wi, :], in_=psum[:, 0:128],
                                         func=ACTF.Copy, scale=rec[:])
                # b2 is host-folded into the table rows (coefs sum to 1)
                c0 = geom.w0[ci] * FOUT
                c1 = geom.w0[ci + 1] * FOUT
                nc.sync.dma_start(out=dump_out[:, c0:c1],
                                  in_=aggN[:, 0:cw, :])
    nc.compile()
    return nc


# ----------------------------------------------------------------------------
# Host orchestration
# ----------------------------------------------------------------------------

def _run(nc, in_maps, tag):
    trace = os.environ.get("KERNEL_TRACE", "0") == "1"
    res = run_bass_kernel_spmd(nc, in_maps, list(range(NCORES)), trace=trace)
    if trace:
        _CACHE.setdefault("profiles", {})[tag] = res
    return res.results


def _expand_ae(cores, a_src, a_dst):
    """Host-expanded per-slot alpha = a_src[src] + a_dst[dst] per core."""
    a_src = a_src.astype(np.float32)
    a_dst = a_dst.astype(np.float32)
    return [np.ascontiguousarray(
        (a_src[cd["slot_src"]] + a_dst[cd["slot_dst"]]).astype(BF))
        for cd in cores]


def kernel(x, src, dst, W1, att_src1, att_dst1, b1, W2, att_src2, att_dst2, b2):
    x = np.asarray(x, np.float32)
    src = np.asarray(src, np.int64)
    dst = np.asarray(dst, np.int64)
    W1 = np.asarray(W1, np.float32)
    W2 = np.asarray(W2, np.float32)
    att_src1 = np.asarray(att_src1, np.float32)
    att_dst1 = np.asarray(att_dst1, np.float32)
    att_src2 = np.asarray(att_src2, np.float32)
    att_dst2 = np.asarray(att_dst2, np.float32)
    b1 = np.asarray(b1, np.float32)
    b2 = np.asarray(b2, np.float32)

    ekey = ("edges", hash(src.tobytes()), hash(dst.tobytes()))
    if ekey not in _CACHE:
        plan = _plan_all(src, dst)
        W = plan[4]
        geom1 = Geom(W, _e1_chunks(W))
        geom2 = Geom(W)
        _CACHE[ekey] = (geom1, geom2, _fill_cores(plan, geom1),
                        _fill_cores(plan, geom2))
    geom1, geom2, cores1, cores2 = _CACHE[ekey]

    pkey = ("progs_geom", geom1.WINS, tuple(geom1.CHUNKS),
            tuple(geom2.CHUNKS))
    if pkey not in _CACHE:
        _CACHE[pkey] = (build_T(), build_E1(geom1), build_E2(geom2))
        _CACHE["progs"] = _CACHE[pkey]
    ncT, ncE1, ncE2 = _CACHE[pkey]

    perm = _perm_cmajor()
    W1P = np.ascontiguousarray(W1[:, perm])
    W1A_src = np.einsum("fhc,hc->fh", W1.reshape(FIN, H1, C1), att_src1)
    W1A_dst = np.einsum("fhc,hc->fh", W1.reshape(FIN, H1, C1), att_dst1)
    WT = np.concatenate([W1P, W1A_src, W1A_dst], axis=1).astype(BF)  # [128,144]
    b1P = b1[perm].astype(np.float32)
    W2P = np.ascontiguousarray(W2[perm, :])
    att2cat = np.stack([att_src2[0], att_dst2[0]], axis=1).astype(np.float32)
    W2A = (W2P @ att2cat).astype(np.float32)  # [128, 2] in permuted row space
    W2C = np.concatenate([W2P, W2A], axis=1).astype(BF)  # [128, 130]

    ident = np.eye(128, dtype=np.float32).astype(BF)
    iota = np.tile(np.arange(128, dtype=np.float32), (128, 1)).astype(BF)
    b1rep = np.tile(b1P, (128, 1)).astype(BF)
    b2rep = np.tile(b2, (128, 1)).astype(BF)
    cst1 = np.ascontiguousarray(
        np.concatenate([ident, W2C], axis=1))               # [128, 258]

    # ---- Launch T: per-core [xh | a_src | a_dst] -------------------------
    xtpad = np.zeros((NCORES, 128, NPC_PAD), BF)
    for c in range(NCORES):
        xtpad[c, :, :NPC] = x[c * NPC:(c + 1) * NPC].T.astype(BF)
    in_maps = [{"xt": xtpad[c], "w": WT} for c in range(NCORES)]
    resT = _run(ncT, in_maps, "T")
    parts = []
    for c in range(NCORES):
        d = resT[c]["dump"].reshape(128, NT_T, TCOLS)
        parts.append(d.transpose(1, 0, 2).reshape(NPC_PAD, TCOLS)[:NPC])
    ta = np.concatenate(parts)                      # [N, 144] bf16
    # fold b1 into the rows: (sum e*(xh+b1))/s = (sum e*xh)/s + b1
    table1 = np.ascontiguousarray(
        (ta[:, 0:FMID].astype(np.float32) + b1P).astype(BF))  # [N, 128]
    a1_src = np.ascontiguousarray(ta[:, FMID:FMID + H1])
    a1_dst = np.ascontiguousarray(ta[:, FMID + H1:FMID + 2 * H1])
    ae1 = _expand_ae(cores1, a1_src, a1_dst)

    # ---- Launch E1 --------------------------------------------------------
    in_maps = [{"table": table1, "ae": ae1[c], "ilo": cores1[c]["idx_lo"],
                "ihi": cores1[c]["idx_hi"], "drel": cores1[c]["drel"],
                "cst": cst1}
               for c in range(NCORES)]
    resE1 = _run(ncE1, in_maps, "E1")
    ha = np.zeros((N, FOUT + 2), BF)
    for c in range(NCORES):
        d = resE1[c]["dump"].reshape(128, geom1.WINS, FOUT + 2)
        for w, (base, nd) in enumerate(cores1[c]["wmap"]):
            if nd:
                ha[base:base + nd] = d[0:nd, w, :]
    table2 = np.zeros((N, 256), BF)                 # [xh2+b2 | 1.0 | pad]
    table2[:, 0:FOUT] = (ha[:, 0:FOUT].astype(np.float32) + b2).astype(BF)
    table2[:, FOUT] = BF(1.0)
    a2_src = np.ascontiguousarray(ha[:, FOUT:FOUT + 1])
    a2_dst = np.ascontiguousarray(ha[:, FOUT + 1:FOUT + 2])
    ae2 = _expand_ae(cores2, a2_src, a2_dst)

    # ---- Launch E2 --------------------------------------------------------
    in_maps = [{"table": table2, "ae": ae2[c], "ilo": cores2[c]["idx_lo"],
                "ihi": cores2[c]["idx_hi"], "drel": cores2[c]["drel"]}
               for c in range(NCORES)]
    resE2 = _run(ncE2, in_maps, "E2")
    out = np.zeros((N, FOUT), np.float32)
    for c in range(NCORES):
        d = resE2[c]["dump"].reshape(128, geom2.WINS, FOUT).astype(np.float32)
        for w, (base, nd) in enumerate(cores2[c]["wmap"]):
            if nd:
                out[base:base + nd] = d[0:nd, w, :]
    return np.ascontiguousarray(out)


# revision 34
# speedup vs baseline: 1.0091x; 1.0025x over previous
"""GAT 2-layer kernel for 8 Trainium2 NeuronCores (bf16 pipeline).

Strategy (edge-parallel over dst-sorted edges, node-range sharded): host
appends self-loops, sorts edges by dst, gives each core a contiguous 6250-dst
range. Dsts are greedily packed into windows of <=128 dsts whose edges fit a
fixed 9-tile budget (4 "lo" + 5 "hi" tiles of 128 slots, split by src index so
int16 dma_gather indices reach the whole node table). The per-window dst
RANGES vary per core (host data) while the tile geometry is shared, so one
SPMD program serves all 8 cores with ~6% fewer gathered slots than a fixed
10-tile layout. Per-slot attention logits alpha = a_src[src] + a_dst[dst] are
host-expanded (bf16), like all index prep.

  - Launch T: [xh | a_src | a_dst] = x^T-tiles @ [W1P | W1A] per core from a
    host-pretransposed bf16 xT; psums grouped 3 tiles per bank, psum->SBUF
    copies alternate ACT/DVE, one DMA in / six piece DMAs out.
  - Launch E1 (heads=8): small chunks (2-4 windows, DVE-paced); per chunk,
    dma_gather of bf16 xh rows (256B); e = exp(leaky(alpha)) on ACT;
    msg = xh[src] * e on
    DVE (2x, c-major head broadcast); one-hot S per tile (tensor_scalar
    is_equal, 4x); segment sums via S^T @ [msg | e] matmuls accumulated in
    PSUM; ACT copies psums to a bf16 chunk buffer; the normalize + bias + ELU
    epilogue runs batched (in <=5-window halves for PSUM pressure) one chunk
    behind (software pipelining); batched PE transposes + [h@W2 | h@W2A]
    matmuls; per-chunk output DMAs. Tapered tail chunks keep the pipeline
    drain after the last gather short. Host reassembles the layer-2 table
    between launches.
  - Launch E2 (heads=1): e2 is folded into the selection matrix (S_e = e2 *
    one_hot via fused is_equal+mult), the gathered 512B rows carry a trailing
    1.0 so one matmul yields [agg | s]; divide-by-s is fused into the ACT psum
    copy as a per-partition scale; + b2; window-major dump, host scatters rows
    back to node order.

Sharding note (vs the edge-parallel hint): edges are sharded by dst range so
all segment reductions stay core-local in PSUM - no cross-core all-reduce is
needed; the small weights are folded/replicated on the host side.
"""

import os
import sys

sys.path.insert(0, "/opt/trn_rl_repo")

import numpy as np
import ml_dtypes

import concourse.bass as bass
import concourse.bacc as bacc
import concourse.mybir as mybir
import concourse.tile as tile
from concourse.bass_utils import run_bass_kernel_spmd

F32 = mybir.dt.float32
BF16 = mybir.dt.bfloat16
I16 = mybir.dt.int16
ALU = mybir.AluOpType
ACTF = mybir.ActivationFunctionType
BF = ml_dtypes.bfloat16

# Problem constants (hardcoded per harness contract).
N = 50000
E = 400000
FIN = 128
H1, C1 = 8, 16          # layer-1 heads / channels
FMID = H1 * C1          # 128
FOUT = 128
NEG_SLOPE = 0.2

NCORES = 8
NPC = N // NCORES       # 6250 nodes per core
LOT = 4                 # lo tiles per window (src < 32768 reachable)
HIT = 5                 # hi tiles per window (src >= HI_BASE reachable)
TPW = LOT + HIT         # 9 tiles of 128 slots per window
SENT = -1               # sentinel dst_rel for padding slots
HI_BASE = N - 32768     # 17232: hi gather covers rows [HI_BASE, N)
NT_T = (NPC + 127) // 128  # x tiles per core in launch T (49)
NPC_PAD = NT_T * 128
TCOLS = FMID + 2 * H1   # 144: [xh | a_src | a_dst] in launch T

GATHER_TILES = 8        # tiles (128 idxs each) per dma_gather call
                        # (1024 idxs = 64 descs/engine packet, HW limit)

_CACHE = {}


# ----------------------------------------------------------------------------
# Host-side graph preprocessing
# ----------------------------------------------------------------------------

def _wrap16(idx):
    """int16 index array [n] -> dma_gather wrapped layout [16, n//16]."""
    n = idx.shape[0]
    return np.ascontiguousarray(idx.reshape(n // 16, 16).T.astype(np.int16))


class Geom:
    """Shared launch geometry: W windows of TPW tiles, chunk window counts."""

    def __init__(self, wins, chunks=None):
        self.WINS = wins
        if chunks is None:
            full, rem = divmod(wins, 10)
            chunks = [10] * full + ([rem] if rem else [])
            if chunks[-1] > 1:  # single-window tail: minimal post-gather drain
                chunks = chunks[:-1] + [chunks[-1] - 1, 1]
        assert sum(chunks) == wins
        self.CHUNKS = chunks
        self.NTILES = wins * TPW
        self.TPC_MAX = max(chunks) * TPW
        # cumulative offsets per chunk (windows / tiles / lo+hi idx columns)
        w0 = [0]
        for cw in chunks:
            w0.append(w0[-1] + cw)
        self.w0 = w0
        self.t0 = [w * TPW for w in w0]
        self.lo_c0 = [w * LOT * 128 // 16 for w in w0]
        self.hi_c0 = [w * HIT * 128 // 16 for w in w0]


def _plan_windows(counts_core, ml_core, mh_core):
    """Greedy dst packing: <=128 dsts, <=LOT*128 lo slots, <=HIT*128 hi
    slots, <=TPW*128 total edges per window. Returns [(dst0, ndst)]."""
    wins = []
    n = counts_core.shape[0]
    d = 0
    cap_t, cap_l, cap_h = TPW * 128, LOT * 128, HIT * 128
    while d < n:
        d0 = d
        tot = ml = mh = 0
        while d < n and d - d0 < 128:
            k, l, h = counts_core[d], ml_core[d], mh_core[d]
            if tot + k > cap_t or ml + l > cap_l or mh + h > cap_h:
                break
            tot += k
            ml += l
            mh += h
            d += 1
        assert d > d0, "single dst exceeds window caps"
        wins.append((d0, d - d0))
    return wins


def _e1_chunks(wins):
    """Fine-grained chunks (4 windows) with small warm-up and a single-window
    final chunk: E1's post-gather drain is the last chunk's compute plus its
    epilogue chain, so the smallest possible tail wins."""
    rem = wins - 10
    assert rem > 0
    return [2, 3] + [4] * (rem // 4) + ([rem % 4] if rem % 4 else []) + [4, 1]


def _plan_all(src, dst):
    """Sort edges by dst, plan shared windows. Returns the plan tuple."""
    s_all = np.concatenate([src, np.arange(N, dtype=np.int64)])
    d_all = np.concatenate([dst, np.arange(N, dtype=np.int64)])
    order = np.argsort(d_all, kind="stable")
    s_all = s_all[order]
    d_all = d_all[order]
    counts = np.bincount(d_all, minlength=N)
    starts = np.concatenate([[0], np.cumsum(counts)])
    # per-dst mandatory-lo / mandatory-hi counts
    ml_all = np.bincount(d_all[s_all < HI_BASE], minlength=N)
    mh_all = np.bincount(d_all[s_all >= 32768], minlength=N)

    core_wins = []
    for c in range(NCORES):
        r = slice(c * NPC, (c + 1) * NPC)
        wins = _plan_windows(counts[r], ml_all[r], mh_all[r])
        core_wins.append([(c * NPC + d0, nd) for d0, nd in wins])
    W = max(len(w) for w in core_wins)
    return (s_all, d_all, starts, core_wins, W)


def _fill_cores(plan, geom):
    """Per-core device index arrays + host slot maps for one chunking."""
    s_all, d_all, starts, core_wins, W = plan
    cores = []
    for c in range(NCORES):
        wmap = core_wins[c] + [(c * NPC, 0)] * (W - len(core_wins[c]))
        slot_src = np.zeros((geom.NTILES, 128), np.int64)
        slot_dst = np.zeros((geom.NTILES, 128), np.int64)
        slot_rel = np.full((geom.NTILES, 128), SENT, np.int64)
        ilo_cols = []
        ihi_cols = []
        for ci, cw in enumerate(geom.CHUNKS):
            lo_flat = np.zeros(cw * LOT * 128, np.int64)
            hi_flat = np.zeros(cw * HIT * 128, np.int64)
            for wi in range(cw):
                w = geom.w0[ci] + wi
                base, nd = wmap[w]
                e0, e1 = starts[base], starts[base + nd]
                ss, dd = s_all[e0:e1], d_all[e0:e1]
                must_lo = ss < HI_BASE
                must_hi = ss >= 32768
                free = ~must_lo & ~must_hi
                cap_lo = LOT * 128
                n_lo = min(cap_lo, int(e1 - e0) - int(must_hi.sum()))
                sel_lo = must_lo.copy()
                free_idx = np.where(free)[0]
                sel_lo[free_idx[:n_lo - int(must_lo.sum())]] = True
                sel_hi = ~sel_lo
                nl, nh = int(sel_lo.sum()), int(sel_hi.sum())
                assert nl <= cap_lo and nh <= HIT * 128, (nl, nh)
                # lo block
                ls = np.zeros(cap_lo, np.int64)
                ld = np.full(cap_lo, base, np.int64)
                lr = np.full(cap_lo, SENT, np.int64)
                ls[:nl] = ss[sel_lo]
                ld[:nl] = dd[sel_lo]
                lr[:nl] = dd[sel_lo] - base
                lo_flat[wi * cap_lo:(wi + 1) * cap_lo] = ls
                g0 = geom.t0[ci] + wi * LOT
                slot_src[g0:g0 + LOT] = ls.reshape(LOT, 128)
                slot_dst[g0:g0 + LOT] = ld.reshape(LOT, 128)
                slot_rel[g0:g0 + LOT] = lr.reshape(LOT, 128)
                # hi block
                cap_hi = HIT * 128
                hs = np.full(cap_hi, HI_BASE, np.int64)
                hd = np.full(cap_hi, base, np.int64)
                hr = np.full(cap_hi, SENT, np.int64)
                hs[:nh] = ss[sel_hi]
                hd[:nh] = dd[sel_hi]
                hr[:nh] = dd[sel_hi] - base
                hi_flat[wi * cap_hi:(wi + 1) * cap_hi] = hs
                g1 = geom.t0[ci] + cw * LOT + wi * HIT
                slot_src[g1:g1 + HIT] = hs.reshape(HIT, 128)
                slot_dst[g1:g1 + HIT] = hd.reshape(HIT, 128)
                slot_rel[g1:g1 + HIT] = hr.reshape(HIT, 128)
            ilo_cols.append(_wrap16(lo_flat))
            ihi_cols.append(_wrap16(hi_flat - HI_BASE))
        idx_lo = np.concatenate(ilo_cols, axis=1)
        idx_hi = np.concatenate(ihi_cols, axis=1)
        cores.append({
            "idx_lo": np.ascontiguousarray(np.tile(idx_lo, (8, 1))),
            "idx_hi": np.ascontiguousarray(np.tile(idx_hi, (8, 1))),
            "slot_src": np.ascontiguousarray(slot_src.T),   # [128, n_tiles]
            "slot_dst": np.ascontiguousarray(slot_dst.T),
            "drel": np.ascontiguousarray(slot_rel.T.astype(np.int16)),
            "wmap": wmap,
        })
    return cores


def _perm_cmajor():
    """Column permutation h*16+c -> c*8+h for layer-1 features."""
    p = np.zeros(FMID, np.int64)
    for h in range(H1):
        for c in range(C1):
            p[c * H1 + h] = h * C1 + c
    return p


# ----------------------------------------------------------------------------
# Bass program builders
# ----------------------------------------------------------------------------

def _new_nc():
    return bacc.Bacc("TRN2", target_bir_lowering=False, debug=False,
                     num_devices=NCORES)


def build_T():
    """Table launch: [xh | a_src | a_dst] = xT^T @ [W1P | W1A] per core."""
    nc = _new_nc()
    xt_in = nc.declare_dram_parameter("xt", [128, NPC_PAD], BF16, isOutput=False)
    w_in = nc.declare_dram_parameter("w", [FIN, TCOLS], BF16, isOutput=False)
    dump_out = nc.declare_dram_parameter("dump", [128, NT_T * TCOLS], BF16,
                                         isOutput=True)

    with tile.TileContext(nc) as tc:
        with (
            tc.tile_pool(name="const", bufs=1) as cpool,
            tc.tile_pool(name="ps", bufs=6, space="PSUM") as pspool,
        ):
            xt = cpool.tile([128, NPC_PAD], BF16)
            w = cpool.tile([FIN, TCOLS], BF16)
            acc = cpool.tile([128, NT_T, TCOLS], BF16)
            # split xT load so tile-0 compute starts early
            nc.sync.dma_start(out=w[:], in_=w_in[:, :])
            q = [0, 6 * 128, 18 * 128, 34 * 128, NPC_PAD]
            for i in range(4):
                nc.sync.dma_start(out=xt[:, q[i]:q[i + 1]],
                                  in_=xt_in[:, q[i]:q[i + 1]])
            for g0 in range(0, NT_T, 3):
                gn = min(3, NT_T - g0)
                ps = pspool.tile([128, 3, TCOLS], F32, space="PSUM")
                for j in range(gn):
                    t = g0 + j
                    nc.tensor.matmul(out=ps[:, j, :],
                                     lhsT=xt[:, t * 128:(t + 1) * 128],
                                     rhs=w[:], start=True, stop=True)
                if (g0 // 3) % 2 == 0:
                    nc.scalar.copy(out=acc[:, g0:g0 + gn, :], in_=ps[:, 0:gn, :])
                else:
                    nc.vector.tensor_copy(out=acc[:, g0:g0 + gn, :],
                                          in_=ps[:, 0:gn, :])
                if g0 + gn in (9, 18, 27, 36, 42, NT_T):
                    marks = [0, 9, 18, 27, 36, 42, NT_T]
                    d0 = marks[marks.index(g0 + gn) - 1] * TCOLS
                    d1 = (g0 + gn) * TCOLS
                    nc.sync.dma_start(out=dump_out[:, d0:d1],
                                      in_=acc[:, d0 // TCOLS:(g0 + gn), :])
    nc.compile()
    return nc


def _emit_gathers(nc, G, table_in, idx, base_tile, n_tiles, idx_col0):
    done = 0
    while done < n_tiles:
        piece = min(GATHER_TILES, n_tiles - done)
        nidx = piece * 128
        c0 = idx_col0 + done * 8
        nc.gpsimd.dma_gather(
            out_ap=G[:, base_tile + done:base_tile + done + piece, :],
            in_ap=table_in, idxs_ap=idx[:, c0:c0 + nidx // 16],
            num_idxs=nidx, num_idxs_reg=nidx,
            elem_size=table_in.shape[-1])
        done += piece


def _emit_gathers_il(nc, G, lo_ap, hi_ap, ilo, ihi, nlo_t, nhi_t,
                     lo_c0, hi_c0):
    """Interleave lo/hi gather pieces so each window's full tile set (its lo
    AND hi block) lands as early as possible."""
    lo_done = hi_done = 0
    while lo_done < nlo_t or hi_done < nhi_t:
        for ap, idx, done, n_t, c0, base in (
                (lo_ap, ilo, lo_done, nlo_t, lo_c0, 0),
                (hi_ap, ihi, hi_done, nhi_t, hi_c0, nlo_t)):
            if done >= n_t:
                continue
            piece = min(GATHER_TILES, n_t - done)
            nidx = piece * 128
            cc = c0 + done * 8
            nc.gpsimd.dma_gather(
                out_ap=G[:, base + done:base + done + piece, :],
                in_ap=ap, idxs_ap=idx[:, cc:cc + nidx // 16],
                num_idxs=nidx, num_idxs_reg=nidx,
                elem_size=ap.shape[-1])
        lo_done = min(nlo_t, lo_done + GATHER_TILES)
        hi_done = min(nhi_t, hi_done + GATHER_TILES)


def _tile_of(ci, cw, wi, t, t0):
    """Global tile id for tile t of window wi in chunk ci (lo block first)."""
    if t < LOT:
        return t0 + wi * LOT + t
    return t0 + cw * LOT + wi * HIT + (t - LOT)


def build_E1(geom, deep_bufs=False, pool_s=0, fill_chunks=6):
    W, NTILES = geom.WINS, geom.NTILES
    LO_COLS = W * LOT * 128 // 16
    HI_COLS = W * HIT * 128 // 16
    nc = _new_nc()
    table_in = nc.declare_dram_parameter("table", [N, 128], BF16, isOutput=False)
    ae_in = nc.declare_dram_parameter("ae", [128, NTILES, H1], BF16,
                                      isOutput=False)
    ilo_in = nc.declare_dram_parameter("ilo", [128, LO_COLS], I16,
                                       isOutput=False)
    ihi_in = nc.declare_dram_parameter("ihi", [128, HI_COLS], I16,
                                       isOutput=False)
    drel_in = nc.declare_dram_parameter("drel", [128, NTILES], I16, isOutput=False)
    cst_in = nc.declare_dram_parameter("cst", [128, 258], BF16, isOutput=False)
    dump_out = nc.declare_dram_parameter("dump", [128, W * (FOUT + 2)], BF16,
                                         isOutput=True)

    with tile.TileContext(nc) as tc:
        with (
            tc.tile_pool(name="const", bufs=1) as cpool,
            tc.tile_pool(name="gat", bufs=3 if deep_bufs else 2) as gpool,
            tc.tile_pool(name="alp", bufs=3 if deep_bufs else 2) as apool,
            tc.tile_pool(name="rhs", bufs=3 if deep_bufs else 2) as rpool,
            tc.tile_pool(name="sel", bufs=48) as spool,
            tc.tile_pool(name="psw", bufs=2, space="PSUM") as ppool,
            tc.tile_pool(name="accp", bufs=2) as accppool,
            tc.tile_pool(name="acca", bufs=2) as accapool,
            tc.tile_pool(name="epi", bufs=2) as epool,
            tc.tile_pool(name="hel", bufs=2) as hpool,
            tc.tile_pool(name="ht", bufs=2) as htpool,
            tc.tile_pool(name="psep", bufs=1, space="PSUM") as peppool,
        ):
            ilo = cpool.tile([128, LO_COLS], I16)
            ihi = cpool.tile([128, HI_COLS], I16)
            drel = cpool.tile([128, NTILES], F32)
            drel16 = cpool.tile([128, NTILES], I16)
            ae = cpool.tile([128, NTILES, H1], BF16)
            cst = cpool.tile([128, 258], BF16)
            ident = cst[:, 0:128]
            w2c = cst[:, 128:258]
            accAll = cpool.tile([128, W, FOUT + 2], BF16)
            iota_t = cpool.tile([128, 128], BF16)
            iota = iota_t[:]
            # iota built on-device (Pool, ~0.2us): the one-hot S builds then
            # depend only on drel, starting ~1us earlier in the fill
            nc.gpsimd.iota(out=iota, pattern=[[1, 128]], base=0,
                           channel_multiplier=0,
                           allow_small_or_imprecise_dtypes=True)

            # per-chunk JIT input loads: chunks 0/1 up front, chunk ch+1
            # during chunk ch, remainder all at once
            def load_chunk_inputs(c, cend=None):
                cend = c + 1 if cend is None else cend
                l0, l1 = geom.lo_c0[c], geom.lo_c0[cend]
                h0, h1 = geom.hi_c0[c], geom.hi_c0[cend]
                t0, t1 = geom.t0[c], geom.t0[cend]
                nc.sync.dma_start(out=ilo[:, l0:l1], in_=ilo_in[:, l0:l1])
                nc.sync.dma_start(out=ihi[:, h0:h1], in_=ihi_in[:, h0:h1])
                nc.sync.dma_start(out=drel16[:, t0:t1], in_=drel_in[:, t0:t1])
                nc.sync.dma_start(out=ae[:, t0:t1, :], in_=ae_in[:, t0:t1, :])

            nchunks = len(geom.CHUNKS)
            load_chunk_inputs(0, min(2, nchunks))
            nc.sync.dma_start(out=cst[:], in_=cst_in[:, :])
            if nchunks > 2:
                load_chunk_inputs(2, nchunks)
            tcv = geom.t0[min(2, nchunks)]
            nc.vector.tensor_copy(out=drel[:, 0:tcv], in_=drel16[:, 0:tcv])

            def epilogue(ci, accP, w0, w1):
                # batched normalize + bias + ELU over windows [w0, w1)
                # (fixed 5-window tile shapes so pool tags stay unified)
                nw = w1 - w0
                sEps = epool.tile([128, 5, H1], F32, name="sEps")
                nc.scalar.activation(out=sEps[:, 0:nw, :],
                                     in_=accP[:, w0:w1, 128:128 + H1],
                                     func=ACTF.Copy, bias=1e-30)
                rec = epool.tile([128, 5, H1], BF16, name="rec")
                with nc.allow_low_precision(reason="coef normalize in bf16"):
                    nc.vector.reciprocal(out=rec[:, 0:nw, :],
                                         in_=sEps[:, 0:nw, :])
                # b1 is host-folded into the table rows: since coefs sum
                # to 1, (sum e*(xh+b1))/s = (sum e*xh)/s + b1, landing the
                # bias before the ELU exactly as the reference does
                hB = epool.tile([128, 5, 128], BF16, name="hB")
                nc.vector.tensor_tensor(
                    out=hB[:, 0:nw, :].rearrange("p w (c h) -> p w c h", h=H1),
                    in0=accP[:, w0:w1, 0:128].rearrange(
                        "p w (c h) -> p w c h", h=H1),
                    in1=rec[:, 0:nw, :].unsqueeze(2).broadcast_to(
                        [128, nw, C1, H1]),
                    op=ALU.mult)
                # exp(min(x,0)) = exp(-relu(-x)): both steps on ACT
                tmp = epool.tile([128, 5, 128], BF16, name="tmp")
                nc.scalar.activation(out=tmp[:, 0:nw, :], in_=hB[:, 0:nw, :],
                                     func=ACTF.Relu, scale=-1.0)
                nc.scalar.activation(out=tmp[:, 0:nw, :], in_=tmp[:, 0:nw, :],
                                     func=ACTF.Exp, scale=-1.0)
                helu = hpool.tile([128, 5, 128], BF16, name="helu")
                nc.vector.tensor_scalar(out=helu[:, 0:nw, :],
                                        in0=tmp[:, 0:nw, :],
                                        scalar1=-1.0, scalar2=None, op0=ALU.add)
                nc.vector.tensor_tensor(out=helu[:, 0:nw, :],
                                        in0=helu[:, 0:nw, :],
                                        in1=hB[:, 0:nw, :], op=ALU.max)
                # layer-2 features: [h @ W2 | h @ W2A] via batched PE transpose
                gw0 = geom.w0[ci] + w0
                psT = peppool.tile([128, 5, 128], BF16, space="PSUM",
                                   name="psT")
                for wi in range(nw):
                    nc.tensor.transpose(out=psT[:, wi, :], in_=helu[:, wi, :],
                                        identity=ident)
                hT = htpool.tile([128, 5, 128], BF16, name="hT")
                nc.scalar.copy(out=hT[:, 0:nw, :], in_=psT[:, 0:nw, :])
                n1 = (nw + 1) // 2
                psA1 = peppool.tile([128, 3, FOUT + 2], F32, space="PSUM",
                                    name="psA1")
                psA2 = peppool.tile([128, 2, FOUT + 2], F32,
                                    space="PSUM", name="psA2")
                for wi in range(nw):
                    pa = psA1[:, wi, :] if wi < n1 else psA2[:, wi - n1, :]
                    nc.tensor.matmul(out=pa, lhsT=hT[:, wi, :], rhs=w2c,
                                     start=True, stop=True)
                # results land in the persistent accAll; the dumps are
                # deferred past the last gather so they never steal body
                # DMA slots from the gather stream (the launch pacer)
                nc.scalar.copy(out=accAll[:, gw0:gw0 + n1, :],
                               in_=psA1[:, 0:n1, :])
                if nw > n1:
                    nc.scalar.copy(out=accAll[:, gw0 + n1:gw0 + nw, :],
                                   in_=psA2[:, 0:nw - n1, :])

            def emit_exp(ci, cw):
                # e = exp(leaky_relu(alpha)) on ACT (alpha host-preadded);
                # emitted one chunk ahead so the in-order ACT queue never
                # stalls it behind the current chunk's psum copies
                tpc = cw * TPW
                t0 = geom.t0[ci]
                A2 = apool.tile([128, geom.TPC_MAX, H1], BF16, name="A2")
                RHS = rpool.tile([128, geom.TPC_MAX, 128 + H1], BF16,
                                 name="RHS")
                nc.scalar.activation(out=A2[:, 0:tpc, :],
                                     in_=ae[:, t0:t0 + tpc, :],
                                     func=ACTF.Prelu, alpha=NEG_SLOPE)
                nc.scalar.activation(out=RHS[:, 0:tpc, 128:128 + H1],
                                     in_=A2[:, 0:tpc, :], func=ACTF.Exp)
                return RHS

            prev = None
            RHS_cur = None
            for ci, cw in enumerate(geom.CHUNKS):
                t0 = geom.t0[ci]
                tpc = cw * TPW
                nlo_t = cw * LOT
                G = gpool.tile([128, geom.TPC_MAX, 128], BF16, name="G")
                _emit_gathers(nc, G, table_in[:, :], ilo, 0, nlo_t,
                              geom.lo_c0[ci])
                _emit_gathers(nc, G, table_in[HI_BASE:, :], ihi, nlo_t,
                              tpc - nlo_t, geom.hi_c0[ci])
                if ci == 0:
                    RHS_cur = emit_exp(0, cw)
                RHS_nxt = (emit_exp(ci + 1, geom.CHUNKS[ci + 1])
                           if ci + 1 < len(geom.CHUNKS) else None)
                RHS = RHS_cur
                if ci == 1 and len(geom.CHUNKS) > 2:
                    # bulk drel int16->f32 (its DMA landed during chunk 0)
                    nc.vector.tensor_copy(out=drel[:, geom.t0[2]:],
                                          in_=drel16[:, geom.t0[2]:])

                def emit_prev_epilogue():
                    if prev is not None:
                        pci, paccP, pcw = prev
                        for e0 in range(0, pcw, 5):
                            epilogue(pci, paccP, e0, min(e0 + 5, pcw))

                # steady state: previous chunk's epilogue first (its deps are
                # long done, so the in-order DVE queue never stalls on it and
                # it fills DVE while this chunk's gathers land). During the
                # fill (ci < 4) deps complete in order S -> epilogue -> msgs,
                # so emit in that order instead to avoid head-of-line blocks.
                if ci >= fill_chunks:
                    emit_prev_epilogue()

                def emit_msg(lo0, n):
                    # msg = xh[src] * e (broadcast over channels; c-major)
                    in0 = G[:, lo0:lo0 + n, :].rearrange(
                        "p t (c h) -> p t c h", h=H1)
                    in1 = RHS[:, lo0:lo0 + n, 128:128 + H1].unsqueeze(
                        2).broadcast_to([128, n, C1, H1])
                    out0 = RHS[:, lo0:lo0 + n, 0:128].rearrange(
                        "p t (c h) -> p t c h", h=H1)
                    nc.vector.tensor_tensor(out=out0, in0=in0, in1=in1,
                                            op=ALU.mult)

                def build_S(wi, t):
                    gl = _tile_of(ci, cw, wi, t, 0)
                    S = spool.tile([128, 128], BF16, name="S")
                    eng = (nc.gpsimd if (wi == cw - 1 and t < pool_s)
                           else nc.vector)
                    eng.tensor_scalar(
                        out=S[:], in0=iota,
                        scalar1=drel[:, t0 + gl:t0 + gl + 1], scalar2=None,
                        op0=ALU.is_equal)
                    return (gl, S)

                accP = accppool.tile([128, max(geom.CHUNKS), 128 + H1], BF16,
                                     name="accP")
                if ci < fill_chunks:
                    # warm-up chunks: S builds first (no gather dep), so DVE
                    # starts ~2.5us before the first gather lands
                    Sw = [[build_S(wi, t) for t in range(TPW)]
                          for wi in range(cw)]
                    emit_prev_epilogue()
                    for m0 in range(0, tpc, GATHER_TILES):
                        emit_msg(m0, min(GATHER_TILES, tpc - m0))
                    for wi in range(cw):
                        psum = ppool.tile([128, 128 + H1], F32, space="PSUM",
                                          name="psum")
                        for t, (gl, S) in enumerate(Sw[wi]):
                            nc.tensor.matmul(out=psum[:], lhsT=S[:],
                                             rhs=RHS[:, gl, :],
                                             start=(t == 0),
                                             stop=(t == TPW - 1))
                        nc.scalar.copy(out=accP[:, wi, :], in_=psum[:])
                else:
                    for m0 in range(0, nlo_t, GATHER_TILES):
                        emit_msg(m0, min(GATHER_TILES, nlo_t - m0))
                    for wi in range(cw):
                        Ss = [build_S(wi, t) for t in range(TPW)]
                        if wi == 0:
                            for m0 in range(nlo_t, tpc, GATHER_TILES):
                                emit_msg(m0, min(GATHER_TILES, tpc - m0))
                        psum = ppool.tile([128, 128 + H1], F32, space="PSUM",
                                          name="psum")
                        for t, (gl, S) in enumerate(Ss):
                            nc.tensor.matmul(out=psum[:], lhsT=S[:],
                                             rhs=RHS[:, gl, :],
                                             start=(t == 0),
                                             stop=(t == TPW - 1))
                        nc.scalar.copy(out=accP[:, wi, :], in_=psum[:])
                prev = (ci, accP, cw)
                RHS_cur = RHS_nxt
            pci, paccP, pcw = prev
            for e0 in range(0, pcw, 5):
                epilogue(pci, paccP, e0, min(e0 + 5, pcw))
            # deferred output dumps: pieces ordered by epilogue completion
            cuts = [0, W // 3, 2 * W // 3, geom.w0[-3], geom.w0[-2], W]
            for a, b in zip(cuts, cuts[1:]):
                if b > a:
                    nc.sync.dma_start(
                        out=dump_out[:, a * (FOUT + 2):b * (FOUT + 2)],
                        in_=accAll[:, a:b, :])
    nc.compile()
    return nc


def build_E2(geom):
    W, NTILES = geom.WINS, geom.NTILES
    LO_COLS = W * LOT * 128 // 16
    HI_COLS = W * HIT * 128 // 16
    nc = _new_nc()
    table_in = nc.declare_dram_parameter("table", [N, 256], BF16, isOutput=False)
    ae_in = nc.declare_dram_parameter("ae", [128, NTILES, 1], BF16,
                                      isOutput=False)
    ilo_in = nc.declare_dram_parameter("ilo", [128, LO_COLS], I16,
                                       isOutput=False)
    ihi_in = nc.declare_dram_parameter("ihi", [128, HI_COLS], I16,
                                       isOutput=False)
    drel_in = nc.declare_dram_parameter("drel", [128, NTILES], I16, isOutput=False)

    dump_out = nc.declare_dram_parameter("dump", [128, W * FOUT], BF16,
                                         isOutput=True)

    with tile.TileContext(nc) as tc:
        with (
            tc.tile_pool(name="const", bufs=1) as cpool,
            tc.tile_pool(name="gat", bufs=2) as gpool,
            tc.tile_pool(name="alp", bufs=2) as apool,
            tc.tile_pool(name="sel", bufs=24) as spool,
            tc.tile_pool(name="psw", bufs=7, space="PSUM") as ppool,
            tc.tile_pool(name="agg", bufs=2) as aggpool,
            tc.tile_pool(name="rc", bufs=4) as rcpool,
        ):
            ilo = cpool.tile([128, LO_COLS], I16)
            ihi = cpool.tile([128, HI_COLS], I16)
            drel = cpool.tile([128, NTILES], F32)
            drel16 = cpool.tile([128, NTILES], I16)
            ae = cpool.tile([128, NTILES, 1], BF16)
            accAll = cpool.tile([128, W, FOUT + 2], BF16)
            iota_t = cpool.tile([128, 128], BF16)
            iota = iota_t[:]
            nc.gpsimd.iota(out=iota, pattern=[[1, 128]], base=0,
                           channel_multiplier=0,
                           allow_small_or_imprecise_dtypes=True)

            def load_chunk_inputs(c, cend=None):
                cend = c + 1 if cend is None else cend
                l0, l1 = geom.lo_c0[c], geom.lo_c0[cend]
                h0, h1 = geom.hi_c0[c], geom.hi_c0[cend]
                t0, t1 = geom.t0[c], geom.t0[cend]
                nc.sync.dma_start(out=ilo[:, l0:l1], in_=ilo_in[:, l0:l1])
                nc.sync.dma_start(out=ihi[:, h0:h1], in_=ihi_in[:, h0:h1])
                nc.sync.dma_start(out=drel16[:, t0:t1], in_=drel_in[:, t0:t1])
                nc.sync.dma_start(out=ae[:, t0:t1, :], in_=ae_in[:, t0:t1, :])

            nchunks = len(geom.CHUNKS)
            load_chunk_inputs(0, min(2, nchunks))
            if nchunks > 2:
                load_chunk_inputs(2, nchunks)
            tcv = geom.t0[min(2, nchunks)]
            nc.vector.tensor_copy(out=drel[:, 0:tcv], in_=drel16[:, 0:tcv])

            for ci, cw in enumerate(geom.CHUNKS):
                t0 = geom.t0[ci]
                tpc = cw * TPW
                nlo_t = cw * LOT
                if ci == 1 and len(geom.CHUNKS) > 2:
                    nc.vector.tensor_copy(out=drel[:, geom.t0[2]:],
                                          in_=drel16[:, geom.t0[2]:])
                G = gpool.tile([128, geom.TPC_MAX, 256], BF16, name="G")
                _emit_gathers_il(nc, G, table_in[:, :], table_in[HI_BASE:, :],
                                 ilo, ihi, nlo_t, tpc - nlo_t,
                                 geom.lo_c0[ci], geom.hi_c0[ci])
                A = apool.tile([128, geom.TPC_MAX, 1], BF16, name="A")
                A2 = apool.tile([128, geom.TPC_MAX, 1], F32, name="A2")
                nc.scalar.activation(out=A[:, 0:tpc, :],
                                     in_=ae[:, t0:t0 + tpc, :],
                                     func=ACTF.Prelu, alpha=NEG_SLOPE)
                nc.scalar.activation(out=A2[:, 0:tpc, :], in_=A[:, 0:tpc, :],
                                     func=ACTF.Exp)
                aggN = aggpool.tile([128, max(geom.CHUNKS), FOUT], BF16,
                                    name="aggN")
                for wi in range(cw):
                    Ss = []
                    for t in range(TPW):
                        g = _tile_of(ci, cw, wi, t, 0)
                        S = spool.tile([128, 128], BF16, name="S")
                        nc.vector.tensor_scalar(
                            out=S[:], in0=iota,
                            scalar1=drel[:, t0 + g:t0 + g + 1],
                            scalar2=A2[:, g, 0:1],
                            op0=ALU.is_equal, op1=ALU.mult)
                        Ss.append((g, S))
                    psum = ppool.tile([128, 129], F32, space="PSUM",
                                      name="psum")
                    for t, (g, S) in enumerate(Ss):
                        nc.tensor.matmul(out=psum[:], lhsT=S[:],
                                         rhs=G[:, g, 0:129],
                                         start=(t == 0), stop=(t == TPW - 1))
                    # out = agg / s: fold 1/s into the ACT psum copy as a
                    # per-partition scale
                    sEps = rcpool.tile([128, 1], F32, name="sEps")
                    nc.scalar.activation(out=sEps[:], in_=psum[:, 128:129],
                                         func=ACTF.Copy, bias=1e-30)
                    rec = rcpool.tile([128, 1], F32, name="rec")
                    nc.vector.reciprocal(out=rec[:], in_=sEps[:])
                    nc.scalar.activation(out=aggN[:, wi, :], in_=psum[:, 0:128],
                                         func=ACTF.Copy, scale=rec[:])
                # b2 is host-folded into the table rows (coefs sum to 1)
                c0 = geom.w0[ci] * FOUT
                c1 = geom.w0[ci + 1] * FOUT
                nc.sync.dma_start(out=dump_out[:, c0:c1],
                                  in_=aggN[:, 0:cw, :])
    nc.compile()
    return nc


# ----------------------------------------------------------------------------
# Host orchestration
# ----------------------------------------------------------------------------

def _run(nc, in_maps, tag):
    trace = os.environ.get("KERNEL_TRACE", "0") == "1"
    res = run_bass_kernel_spmd(nc, in_maps, list(range(NCORES)), trace=trace)
    if trace:
        _CACHE.setdefault("profiles", {})[tag] = res
    return res.results


def _expand_ae(cores, a_src, a_dst):
    """Host-expanded per-slot alpha = a_src[src] + a_dst[dst] per core."""
    a_src = a_src.astype(np.float32)
    a_dst = a_dst.astype(np.float32)
    return [np.ascontiguousarray(
        (a_src[cd["slot_src"]] + a_dst[cd["slot_dst"]]).astype(BF))
        for cd in cores]


def kernel(x, src, dst, W1, att_src1, att_dst1, b1, W2, att_src2, att_dst2, b2):
    x = np.asarray(x, np.float32)
    src = np.asarray(src, np.int64)
    dst = np.asarray(dst, np.int64)
    W1 = np.asarray(W1, np.float32)
    W2 = np.asarray(W2, np.float32)
    att_src1 = np.asarray(att_src1, np.float32)
    att_dst1 = np.asarray(att_dst1, np.float32)
    att_src2 = np.asarray(att_src2, np.float32)
    att_dst2 = np.asarray(att_dst2, np.float32)
    b1 = np.asarray(b1, np.float32)
    b2 = np.asarray(b2, np.float32)

    ekey = ("edges", hash(src.tobytes()), hash(dst.tobytes()))
    if ekey not in _CACHE:
        plan = _plan_all(src, dst)
        W = plan[4]
        geom1 = Geom(W, _e1_chunks(W))
        geom2 = Geom(W)
        _CACHE[ekey] = (geom1, geom2, _fill_cores(plan, geom1),
                        _fill_cores(plan, geom2))
    geom1, geom2, cores1, cores2 = _CACHE[ekey]

    pkey = ("progs_geom", geom1.WINS, tuple(geom1.CHUNKS),
            tuple(geom2.CHUNKS))
    if pkey not in _CACHE:
        _CACHE[pkey] = (build_T(), build_E1(geom1), build_E2(geom2))
        _CACHE["progs"] = _CACHE[pkey]
    ncT, ncE1, ncE2 = _CACHE[pkey]

    perm = _perm_cmajor()
    W1P = np.ascontiguousarray(W1[:, perm])
    W1A_src = np.einsum("fhc,hc->fh", W1.reshape(FIN, H1, C1), att_src1)
    W1A_dst = np.einsum("fhc,hc->fh", W1.reshape(FIN, H1, C1), att_dst1)
    WT = np.concatenate([W1P, W1A_src, W1A_dst], axis=1).astype(BF)  # [128,144]
    b1P = b1[perm].astype(np.float32)
    W2P = np.ascontiguousarray(W2[perm, :])
    att2cat = np.stack([att_src2[0], att_dst2[0]], axis=1).astype(np.float32)
    W2A = (W2P @ att2cat).astype(np.float32)  # [128, 2] in permuted row space
    W2C = np.concatenate([W2P, W2A], axis=1).astype(BF)  # [128, 130]

    ident = np.eye(128, dtype=np.float32).astype(BF)
    iota = np.tile(np.arange(128, dtype=np.float32), (128, 1)).astype(BF)
    b1rep = np.tile(b1P, (128, 1)).astype(BF)
    b2rep = np.tile(b2, (128, 1)).astype(BF)
    cst1 = np.ascontiguousarray(
        np.concatenate([ident, W2C], axis=1))               # [128, 258]

    # ---- Launch T: per-core [xh | a_src | a_dst] -------------------------
    xtpad = np.zeros((NCORES, 128, NPC_PAD), BF)
    for c in range(NCORES):
        xtpad[c, :, :NPC] = x[c * NPC:(c + 1) * NPC].T.astype(BF)
    in_maps = [{"xt": xtpad[c], "w": WT} for c in range(NCORES)]
    resT = _run(ncT, in_maps, "T")
    parts = []
    for c in range(NCORES):
        d = resT[c]["dump"].reshape(128, NT_T, TCOLS)
        parts.append(d.transpose(1, 0, 2).reshape(NPC_PAD, TCOLS)[:NPC])
    ta = np.concatenate(parts)                      # [N, 144] bf16
    # fold b1 into the rows: (sum e*(xh+b1))/s = (sum e*xh)/s + b1
    table1 = np.ascontiguousarray(
        (ta[:, 0:FMID].astype(np.float32) + b1P).astype(BF))  # [N, 128]
    a1_src = np.ascontiguousarray(ta[:, FMID:FMID + H1])
    a1_dst = np.ascontiguousarray(ta[:, FMID + H1:FMID + 2 * H1])
    ae1 = _expand_ae(cores1, a1_src, a1_dst)

    # ---- Launch E1 --------------------------------------------------------
    in_maps = [{"table": table1, "ae": ae1[c], "ilo": cores1[c]["idx_lo"],
                "ihi": cores1[c]["idx_hi"], "drel": cores1[c]["drel"],
                "cst": cst1}
               for c in range(NCORES)]
    resE1 = _run(ncE1, in_maps, "E1")
    ha = np.zeros((N, FOUT + 2), BF)
    for c in range(NCORES):
        d = resE1[c]["dump"].reshape(128, geom1.WINS, FOUT + 2)
        for w, (base, nd) in enumerate(cores1[c]["wmap"]):
            if nd:
                ha[base:base + nd] = d[0:nd, w, :]
    table2 = np.zeros((N, 256), BF)                 # [xh2+b2 | 1.0 | pad]
    table2[:, 0:FOUT] = (ha[:, 0:FOUT].astype(np.float32) + b2).astype(BF)
    table2[:, FOUT] = BF(1.0)
    a2_src = np.ascontiguousarray(ha[:, FOUT:FOUT + 1])
    a2_dst = np.ascontiguousarray(ha[:, FOUT + 1:FOUT + 2])
    ae2 = _expand_ae(cores2, a2_src, a2_dst)

    # ---- Launch E2 --------------------------------------------------------
    in_maps = [{"table": table2, "ae": ae2[c], "ilo": cores2[c]["idx_lo"],
                "ihi": cores2[c]["idx_hi"], "drel": cores2[c]["drel"]}
               for c in range(NCORES)]
    resE2 = _run(ncE2, in_maps, "E2")
    out = np.zeros((N, FOUT), np.float32)
    for c in range(NCORES):
        d = resE2[c]["dump"].reshape(128, geom2.WINS, FOUT).astype(np.float32)
        for w, (base, nd) in enumerate(cores2[c]["wmap"]):
            if nd:
                out[base:base + nd] = d[0:nd, w, :]
    return np.ascontiguousarray(out)


# revision 35
# speedup vs baseline: 1.0169x; 1.0077x over previous
"""GAT 2-layer kernel for 8 Trainium2 NeuronCores (bf16 pipeline).

Strategy (edge-parallel over dst-sorted edges, node-range sharded): host
appends self-loops, sorts edges by dst, gives each core a contiguous 6250-dst
range. Dsts are greedily packed into windows of <=128 dsts whose edges fit a
fixed 9-tile budget (4 "lo" + 5 "hi" tiles of 128 slots, split by src index so
int16 dma_gather indices reach the whole node table). The per-window dst
RANGES vary per core (host data) while the tile geometry is shared, so one
SPMD program serves all 8 cores with ~6% fewer gathered slots than a fixed
10-tile layout. Per-slot attention logits alpha = a_src[src] + a_dst[dst] are
host-expanded (bf16), like all index prep.

  - Launch T: [xh | a_src | a_dst] = x^T-tiles @ [W1P | W1A] per core from a
    host-pretransposed bf16 xT; psums grouped 3 tiles per bank, psum->SBUF
    copies alternate ACT/DVE, one DMA in / six piece DMAs out.
  - Launch E1 (heads=8): small chunks (2-4 windows, DVE-paced); per chunk,
    dma_gather of bf16 xh rows (256B); e = exp(leaky(alpha)) on ACT;
    msg = xh[src] * e on
    DVE (2x, c-major head broadcast); one-hot S per tile (tensor_scalar
    is_equal, 4x); segment sums via S^T @ [msg | e] matmuls accumulated in
    PSUM; ACT copies psums to a bf16 chunk buffer; the normalize + bias + ELU
    epilogue runs batched (in <=5-window halves for PSUM pressure) one chunk
    behind (software pipelining); batched PE transposes + [h@W2 | h@W2A]
    matmuls; per-chunk output DMAs. Tapered tail chunks keep the pipeline
    drain after the last gather short. Host reassembles the layer-2 table
    between launches.
  - Launch E2 (heads=1): e2 is folded into the selection matrix (S_e = e2 *
    one_hot via fused is_equal+mult), the gathered 512B rows carry a trailing
    1.0 so one matmul yields [agg | s]; divide-by-s is fused into the ACT psum
    copy as a per-partition scale; + b2; window-major dump, host scatters rows
    back to node order.

Sharding note (vs the edge-parallel hint): edges are sharded by dst range so
all segment reductions stay core-local in PSUM - no cross-core all-reduce is
needed; the small weights are folded/replicated on the host side.
"""

import os
import sys

sys.path.insert(0, "/opt/trn_rl_repo")

import numpy as np
import ml_dtypes

import concourse.bass as bass
import concourse.bacc as bacc
import concourse.mybir as mybir
import concourse.tile as tile
from concourse.bass_utils import run_bass_kernel_spmd

F32 = mybir.dt.float32
BF16 = mybir.dt.bfloat16
I16 = mybir.dt.int16
ALU = mybir.AluOpType
ACTF = mybir.ActivationFunctionType
BF = ml_dtypes.bfloat16

# Problem constants (hardcoded per harness contract).
N = 50000
E = 400000
FIN = 128
H1, C1 = 8, 16          # layer-1 heads / channels
FMID = H1 * C1          # 128
FOUT = 128
NEG_SLOPE = 0.2

NCORES = 8
NPC = N // NCORES       # 6250 nodes per core
LOT = 4                 # lo tiles per window (src < 32768 reachable)
HIT = 5                 # hi tiles per window (src >= HI_BASE reachable)
TPW = LOT + HIT         # 9 tiles of 128 slots per window
SENT = -1               # sentinel dst_rel for padding slots
HI_BASE = N - 32768     # 17232: hi gather covers rows [HI_BASE, N)
NT_T = (NPC + 127) // 128  # x tiles per core in launch T (49)
NPC_PAD = NT_T * 128
TCOLS = FMID + 2 * H1   # 144: [xh | a_src | a_dst] in launch T

GATHER_TILES = 8        # tiles (128 idxs each) per dma_gather call
                        # (1024 idxs = 64 descs/engine packet, HW limit)

_CACHE = {}


# ----------------------------------------------------------------------------
# Host-side graph preprocessing
# ----------------------------------------------------------------------------

def _wrap16(idx):
    """int16 index array [n] -> dma_gather wrapped layout [16, n//16]."""
    n = idx.shape[0]
    return np.ascontiguousarray(idx.reshape(n // 16, 16).T.astype(np.int16))


class Geom:
    """Shared launch geometry: W windows of TPW tiles, chunk window counts."""

    def __init__(self, wins, chunks=None):
        self.WINS = wins
        if chunks is None:
            full, rem = divmod(wins, 10)
            chunks = [10] * full + ([rem] if rem else [])
            if chunks[-1] > 1:  # single-window tail: minimal post-gather drain
                chunks = chunks[:-1] + [chunks[-1] - 1, 1]
        assert sum(chunks) == wins
        self.CHUNKS = chunks
        self.NTILES = wins * TPW
        self.TPC_MAX = max(chunks) * TPW
        # cumulative offsets per chunk (windows / tiles / lo+hi idx columns)
        w0 = [0]
        for cw in chunks:
            w0.append(w0[-1] + cw)
        self.w0 = w0
        self.t0 = [w * TPW for w in w0]
        self.lo_c0 = [w * LOT * 128 // 16 for w in w0]
        self.hi_c0 = [w * HIT * 128 // 16 for w in w0]


def _plan_windows(counts_core, ml_core, mh_core):
    """Greedy dst packing: <=128 dsts, <=LOT*128 lo slots, <=HIT*128 hi
    slots, <=TPW*128 total edges per window. Returns [(dst0, ndst)]."""
    wins = []
    n = counts_core.shape[0]
    d = 0
    cap_t, cap_l, cap_h = TPW * 128, LOT * 128, HIT * 128
    while d < n:
        d0 = d
        tot = ml = mh = 0
        while d < n and d - d0 < 128:
            k, l, h = counts_core[d], ml_core[d], mh_core[d]
            if tot + k > cap_t or ml + l > cap_l or mh + h > cap_h:
                break
            tot += k
            ml += l
            mh += h
            d += 1
        assert d > d0, "single dst exceeds window caps"
        wins.append((d0, d - d0))
    return wins


def _e1_chunks(wins):
    """Fine-grained chunks (4 windows) with small warm-up and a single-window
    final chunk: E1's post-gather drain is the last chunk's compute plus its
    epilogue chain, so the smallest possible tail wins."""
    rem = wins - 10
    assert rem > 0
    return [2, 3] + [4] * (rem // 4) + ([rem % 4] if rem % 4 else []) + [4, 1]


def _plan_all(src, dst):
    """Sort edges by dst, plan shared windows. Returns the plan tuple."""
    s_all = np.concatenate([src, np.arange(N, dtype=np.int64)])
    d_all = np.concatenate([dst, np.arange(N, dtype=np.int64)])
    order = np.argsort(d_all, kind="stable")
    s_all = s_all[order]
    d_all = d_all[order]
    counts = np.bincount(d_all, minlength=N)
    starts = np.concatenate([[0], np.cumsum(counts)])
    # per-dst mandatory-lo / mandatory-hi counts
    ml_all = np.bincount(d_all[s_all < HI_BASE], minlength=N)
    mh_all = np.bincount(d_all[s_all >= 32768], minlength=N)

    core_wins = []
    for c in range(NCORES):
        r = slice(c * NPC, (c + 1) * NPC)
        wins = _plan_windows(counts[r], ml_all[r], mh_all[r])
        core_wins.append([(c * NPC + d0, nd) for d0, nd in wins])
    W = max(len(w) for w in core_wins)
    return (s_all, d_all, starts, core_wins, W)


def _fill_cores(plan, geom):
    """Per-core device index arrays + host slot maps for one chunking."""
    s_all, d_all, starts, core_wins, W = plan
    cores = []
    for c in range(NCORES):
        wmap = core_wins[c] + [(c * NPC, 0)] * (W - len(core_wins[c]))
        slot_src = np.zeros((geom.NTILES, 128), np.int64)
        slot_dst = np.zeros((geom.NTILES, 128), np.int64)
        slot_rel = np.full((geom.NTILES, 128), SENT, np.int64)
        ilo_cols = []
        ihi_cols = []
        for ci, cw in enumerate(geom.CHUNKS):
            lo_flat = np.zeros(cw * LOT * 128, np.int64)
            hi_flat = np.zeros(cw * HIT * 128, np.int64)
            for wi in range(cw):
                w = geom.w0[ci] + wi
                base, nd = wmap[w]
                e0, e1 = starts[base], starts[base + nd]
                ss, dd = s_all[e0:e1], d_all[e0:e1]
                must_lo = ss < HI_BASE
                must_hi = ss >= 32768
                free = ~must_lo & ~must_hi
                cap_lo = LOT * 128
                n_lo = min(cap_lo, int(e1 - e0) - int(must_hi.sum()))
                sel_lo = must_lo.copy()
                free_idx = np.where(free)[0]
                sel_lo[free_idx[:n_lo - int(must_lo.sum())]] = True
                sel_hi = ~sel_lo
                nl, nh = int(sel_lo.sum()), int(sel_hi.sum())
                assert nl <= cap_lo and nh <= HIT * 128, (nl, nh)
                # lo block
                ls = np.zeros(cap_lo, np.int64)
                ld = np.full(cap_lo, base, np.int64)
                lr = np.full(cap_lo, SENT, np.int64)
                ls[:nl] = ss[sel_lo]
                ld[:nl] = dd[sel_lo]
                lr[:nl] = dd[sel_lo] - base
                lo_flat[wi * cap_lo:(wi + 1) * cap_lo] = ls
                g0 = geom.t0[ci] + wi * LOT
                slot_src[g0:g0 + LOT] = ls.reshape(LOT, 128)
                slot_dst[g0:g0 + LOT] = ld.reshape(LOT, 128)
                slot_rel[g0:g0 + LOT] = lr.reshape(LOT, 128)
                # hi block
                cap_hi = HIT * 128
                hs = np.full(cap_hi, HI_BASE, np.int64)
                hd = np.full(cap_hi, base, np.int64)
                hr = np.full(cap_hi, SENT, np.int64)
                hs[:nh] = ss[sel_hi]
                hd[:nh] = dd[sel_hi]
                hr[:nh] = dd[sel_hi] - base
                hi_flat[wi * cap_hi:(wi + 1) * cap_hi] = hs
                g1 = geom.t0[ci] + cw * LOT + wi * HIT
                slot_src[g1:g1 + HIT] = hs.reshape(HIT, 128)
                slot_dst[g1:g1 + HIT] = hd.reshape(HIT, 128)
                slot_rel[g1:g1 + HIT] = hr.reshape(HIT, 128)
            ilo_cols.append(_wrap16(lo_flat))
            ihi_cols.append(_wrap16(hi_flat - HI_BASE))
        idx_lo = np.concatenate(ilo_cols, axis=1)
        idx_hi = np.concatenate(ihi_cols, axis=1)
        cores.append({
            "idx_lo": np.ascontiguousarray(np.tile(idx_lo, (8, 1))),
            "idx_hi": np.ascontiguousarray(np.tile(idx_hi, (8, 1))),
            "slot_src": np.ascontiguousarray(slot_src.T),   # [128, n_tiles]
            "slot_dst": np.ascontiguousarray(slot_dst.T),
            "drel": np.ascontiguousarray(slot_rel.T.astype(np.int16)),
            "wmap": wmap,
        })
    return cores


def _perm_cmajor():
    """Column permutation h*16+c -> c*8+h for layer-1 features."""
    p = np.zeros(FMID, np.int64)
    for h in range(H1):
        for c in range(C1):
            p[c * H1 + h] = h * C1 + c
    return p


# ----------------------------------------------------------------------------
# Bass program builders
# ----------------------------------------------------------------------------

def _new_nc():
    return bacc.Bacc("TRN2", target_bir_lowering=False, debug=False,
                     num_devices=NCORES)


def build_T():
    """Table launch: [xh | a_src | a_dst] = xT^T @ [W1P | W1A] per core."""
    nc = _new_nc()
    xt_in = nc.declare_dram_parameter("xt", [128, NPC_PAD], BF16, isOutput=False)
    w_in = nc.declare_dram_parameter("w", [FIN, TCOLS], BF16, isOutput=False)
    dump_out = nc.declare_dram_parameter("dump", [128, NT_T * TCOLS], BF16,
                                         isOutput=True)

    with tile.TileContext(nc) as tc:
        with (
            tc.tile_pool(name="const", bufs=1) as cpool,
            tc.tile_pool(name="ps", bufs=6, space="PSUM") as pspool,
        ):
            xt = cpool.tile([128, NPC_PAD], BF16)
            w = cpool.tile([FIN, TCOLS], BF16)
            acc = cpool.tile([128, NT_T, TCOLS], BF16)
            # split xT load so tile-0 compute starts early
            nc.sync.dma_start(out=w[:], in_=w_in[:, :])
            q = [0, 6 * 128, 18 * 128, 34 * 128, NPC_PAD]
            for i in range(4):
                nc.sync.dma_start(out=xt[:, q[i]:q[i + 1]],
                                  in_=xt_in[:, q[i]:q[i + 1]])
            for g0 in range(0, NT_T, 3):
                gn = min(3, NT_T - g0)
                ps = pspool.tile([128, 3, TCOLS], F32, space="PSUM")
                for j in range(gn):
                    t = g0 + j
                    nc.tensor.matmul(out=ps[:, j, :],
                                     lhsT=xt[:, t * 128:(t + 1) * 128],
                                     rhs=w[:], start=True, stop=True)
                if (g0 // 3) % 2 == 0:
                    nc.scalar.copy(out=acc[:, g0:g0 + gn, :], in_=ps[:, 0:gn, :])
                else:
                    nc.vector.tensor_copy(out=acc[:, g0:g0 + gn, :],
                                          in_=ps[:, 0:gn, :])
                if g0 + gn in (9, 18, 27, 36, 42, NT_T):
                    marks = [0, 9, 18, 27, 36, 42, NT_T]
                    d0 = marks[marks.index(g0 + gn) - 1] * TCOLS
                    d1 = (g0 + gn) * TCOLS
                    nc.sync.dma_start(out=dump_out[:, d0:d1],
                                      in_=acc[:, d0 // TCOLS:(g0 + gn), :])
    nc.compile()
    return nc


def _emit_gathers(nc, G, table_in, idx, base_tile, n_tiles, idx_col0):
    done = 0
    while done < n_tiles:
        piece = min(GATHER_TILES, n_tiles - done)
        nidx = piece * 128
        c0 = idx_col0 + done * 8
        nc.gpsimd.dma_gather(
            out_ap=G[:, base_tile + done:base_tile + done + piece, :],
            in_ap=table_in, idxs_ap=idx[:, c0:c0 + nidx // 16],
            num_idxs=nidx, num_idxs_reg=nidx,
            elem_size=table_in.shape[-1])
        done += piece


def _emit_gathers_il(nc, G, lo_ap, hi_ap, ilo, ihi, nlo_t, nhi_t,
                     lo_c0, hi_c0):
    """Interleave lo/hi gather pieces so each window's full tile set (its lo
    AND hi block) lands as early as possible."""
    lo_done = hi_done = 0
    while lo_done < nlo_t or hi_done < nhi_t:
        for ap, idx, done, n_t, c0, base in (
                (lo_ap, ilo, lo_done, nlo_t, lo_c0, 0),
                (hi_ap, ihi, hi_done, nhi_t, hi_c0, nlo_t)):
            if done >= n_t:
                continue
            piece = min(GATHER_TILES, n_t - done)
            nidx = piece * 128
            cc = c0 + done * 8
            nc.gpsimd.dma_gather(
                out_ap=G[:, base + done:base + done + piece, :],
                in_ap=ap, idxs_ap=idx[:, cc:cc + nidx // 16],
                num_idxs=nidx, num_idxs_reg=nidx,
                elem_size=ap.shape[-1])
        lo_done = min(nlo_t, lo_done + GATHER_TILES)
        hi_done = min(nhi_t, hi_done + GATHER_TILES)


def _tile_of(ci, cw, wi, t, t0):
    """Global tile id for tile t of window wi in chunk ci (lo block first)."""
    if t < LOT:
        return t0 + wi * LOT + t
    return t0 + cw * LOT + wi * HIT + (t - LOT)


def build_E1(geom, deep_bufs=False, pool_s=0, fill_chunks=None):
    W, NTILES = geom.WINS, geom.NTILES
    LO_COLS = W * LOT * 128 // 16
    HI_COLS = W * HIT * 128 // 16
    if fill_chunks is None:
        # dependency-ordered emission wins everywhere except the last two
        # chunks, where the steady-state order drains better
        fill_chunks = max(0, len(geom.CHUNKS) - 2)
    nc = _new_nc()
    table_in = nc.declare_dram_parameter("table", [N, 128], BF16, isOutput=False)
    ae_in = nc.declare_dram_parameter("ae", [128, NTILES, H1], BF16,
                                      isOutput=False)
    ilo_in = nc.declare_dram_parameter("ilo", [128, LO_COLS], I16,
                                       isOutput=False)
    ihi_in = nc.declare_dram_parameter("ihi", [128, HI_COLS], I16,
                                       isOutput=False)
    drel_in = nc.declare_dram_parameter("drel", [128, NTILES], I16, isOutput=False)
    cst_in = nc.declare_dram_parameter("cst", [128, 258], BF16, isOutput=False)
    dump_out = nc.declare_dram_parameter("dump", [128, W * (FOUT + 2)], BF16,
                                         isOutput=True)

    with tile.TileContext(nc) as tc:
        with (
            tc.tile_pool(name="const", bufs=1) as cpool,
            tc.tile_pool(name="gat", bufs=3 if deep_bufs else 2) as gpool,
            tc.tile_pool(name="alp", bufs=3 if deep_bufs else 2) as apool,
            tc.tile_pool(name="rhs", bufs=3 if deep_bufs else 2) as rpool,
            tc.tile_pool(name="sel", bufs=48) as spool,
            tc.tile_pool(name="psw", bufs=2, space="PSUM") as ppool,
            tc.tile_pool(name="accp", bufs=2) as accppool,
            tc.tile_pool(name="acca", bufs=2) as accapool,
            tc.tile_pool(name="epi", bufs=2) as epool,
            tc.tile_pool(name="hel", bufs=2) as hpool,
            tc.tile_pool(name="ht", bufs=2) as htpool,
            tc.tile_pool(name="psep", bufs=1, space="PSUM") as peppool,
        ):
            ilo = cpool.tile([128, LO_COLS], I16)
            ihi = cpool.tile([128, HI_COLS], I16)
            drel = cpool.tile([128, NTILES], F32)
            drel16 = cpool.tile([128, NTILES], I16)
            ae = cpool.tile([128, NTILES, H1], BF16)
            cst = cpool.tile([128, 258], BF16)
            ident = cst[:, 0:128]
            w2c = cst[:, 128:258]
            accAll = cpool.tile([128, W, FOUT + 2], BF16)
            iota_t = cpool.tile([128, 128], BF16)
            iota = iota_t[:]
            # iota built on-device (Pool, ~0.2us): the one-hot S builds then
            # depend only on drel, starting ~1us earlier in the fill
            nc.gpsimd.iota(out=iota, pattern=[[1, 128]], base=0,
                           channel_multiplier=0,
                           allow_small_or_imprecise_dtypes=True)

            # per-chunk JIT input loads: chunks 0/1 up front, chunk ch+1
            # during chunk ch, remainder all at once
            def load_chunk_inputs(c, cend=None):
                cend = c + 1 if cend is None else cend
                l0, l1 = geom.lo_c0[c], geom.lo_c0[cend]
                h0, h1 = geom.hi_c0[c], geom.hi_c0[cend]
                t0, t1 = geom.t0[c], geom.t0[cend]
                nc.sync.dma_start(out=ilo[:, l0:l1], in_=ilo_in[:, l0:l1])
                nc.sync.dma_start(out=ihi[:, h0:h1], in_=ihi_in[:, h0:h1])
                nc.sync.dma_start(out=drel16[:, t0:t1], in_=drel_in[:, t0:t1])
                nc.sync.dma_start(out=ae[:, t0:t1, :], in_=ae_in[:, t0:t1, :])

            nchunks = len(geom.CHUNKS)
            load_chunk_inputs(0, min(2, nchunks))
            nc.sync.dma_start(out=cst[:], in_=cst_in[:, :])
            if nchunks > 2:
                load_chunk_inputs(2, nchunks)
            tcv = geom.t0[min(2, nchunks)]
            nc.vector.tensor_copy(out=drel[:, 0:tcv], in_=drel16[:, 0:tcv])

            def epilogue(ci, accP, w0, w1):
                # batched normalize + bias + ELU over windows [w0, w1)
                # (fixed 5-window tile shapes so pool tags stay unified)
                nw = w1 - w0
                sEps = epool.tile([128, 5, H1], F32, name="sEps")
                nc.scalar.activation(out=sEps[:, 0:nw, :],
                                     in_=accP[:, w0:w1, 128:128 + H1],
                                     func=ACTF.Copy, bias=1e-30)
                rec = epool.tile([128, 5, H1], BF16, name="rec")
                with nc.allow_low_precision(reason="coef normalize in bf16"):
                    nc.vector.reciprocal(out=rec[:, 0:nw, :],
                                         in_=sEps[:, 0:nw, :])
                # b1 is host-folded into the table rows: since coefs sum
                # to 1, (sum e*(xh+b1))/s = (sum e*xh)/s + b1, landing the
                # bias before the ELU exactly as the reference does
                hB = epool.tile([128, 5, 128], BF16, name="hB")
                nc.vector.tensor_tensor(
                    out=hB[:, 0:nw, :].rearrange("p w (c h) -> p w c h", h=H1),
                    in0=accP[:, w0:w1, 0:128].rearrange(
                        "p w (c h) -> p w c h", h=H1),
                    in1=rec[:, 0:nw, :].unsqueeze(2).broadcast_to(
                        [128, nw, C1, H1]),
                    op=ALU.mult)
                # exp(min(x,0)) = exp(-relu(-x)): both steps on ACT
                tmp = epool.tile([128, 5, 128], BF16, name="tmp")
                nc.scalar.activation(out=tmp[:, 0:nw, :], in_=hB[:, 0:nw, :],
                                     func=ACTF.Relu, scale=-1.0)
                nc.scalar.activation(out=tmp[:, 0:nw, :], in_=tmp[:, 0:nw, :],
                                     func=ACTF.Exp, scale=-1.0)
                helu = hpool.tile([128, 5, 128], BF16, name="helu")
                nc.vector.tensor_scalar(out=helu[:, 0:nw, :],
                                        in0=tmp[:, 0:nw, :],
                                        scalar1=-1.0, scalar2=None, op0=ALU.add)
                nc.vector.tensor_tensor(out=helu[:, 0:nw, :],
                                        in0=helu[:, 0:nw, :],
                                        in1=hB[:, 0:nw, :], op=ALU.max)
                # layer-2 features: [h @ W2 | h @ W2A] via batched PE transpose
                gw0 = geom.w0[ci] + w0
                psT = peppool.tile([128, 5, 128], BF16, space="PSUM",
                                   name="psT")
                for wi in range(nw):
                    nc.tensor.transpose(out=psT[:, wi, :], in_=helu[:, wi, :],
                                        identity=ident)
                hT = htpool.tile([128, 5, 128], BF16, name="hT")
                nc.scalar.copy(out=hT[:, 0:nw, :], in_=psT[:, 0:nw, :])
                n1 = (nw + 1) // 2
                psA1 = peppool.tile([128, 3, FOUT + 2], F32, space="PSUM",
                                    name="psA1")
                psA2 = peppool.tile([128, 2, FOUT + 2], F32,
                                    space="PSUM", name="psA2")
                for wi in range(nw):
                    pa = psA1[:, wi, :] if wi < n1 else psA2[:, wi - n1, :]
                    nc.tensor.matmul(out=pa, lhsT=hT[:, wi, :], rhs=w2c,
                                     start=True, stop=True)
                # results land in the persistent accAll; the dumps are
                # deferred past the last gather so they never steal body
                # DMA slots from the gather stream (the launch pacer)
                nc.scalar.copy(out=accAll[:, gw0:gw0 + n1, :],
                               in_=psA1[:, 0:n1, :])
                if nw > n1:
                    nc.scalar.copy(out=accAll[:, gw0 + n1:gw0 + nw, :],
                                   in_=psA2[:, 0:nw - n1, :])

            def emit_exp(ci, cw):
                # e = exp(leaky_relu(alpha)) on ACT (alpha host-preadded);
                # emitted one chunk ahead so the in-order ACT queue never
                # stalls it behind the current chunk's psum copies
                tpc = cw * TPW
                t0 = geom.t0[ci]
                A2 = apool.tile([128, geom.TPC_MAX, H1], BF16, name="A2")
                RHS = rpool.tile([128, geom.TPC_MAX, 128 + H1], BF16,
                                 name="RHS")
                nc.scalar.activation(out=A2[:, 0:tpc, :],
                                     in_=ae[:, t0:t0 + tpc, :],
                                     func=ACTF.Prelu, alpha=NEG_SLOPE)
                nc.scalar.activation(out=RHS[:, 0:tpc, 128:128 + H1],
                                     in_=A2[:, 0:tpc, :], func=ACTF.Exp)
                return RHS

            prev = None
            RHS_cur = None
            for ci, cw in enumerate(geom.CHUNKS):
                t0 = geom.t0[ci]
                tpc = cw * TPW
                nlo_t = cw * LOT
                G = gpool.tile([128, geom.TPC_MAX, 128], BF16, name="G")
                _emit_gathers(nc, G, table_in[:, :], ilo, 0, nlo_t,
                              geom.lo_c0[ci])
                _emit_gathers(nc, G, table_in[HI_BASE:, :], ihi, nlo_t,
                              tpc - nlo_t, geom.hi_c0[ci])
                if ci == 0:
                    RHS_cur = emit_exp(0, cw)
                RHS_nxt = (emit_exp(ci + 1, geom.CHUNKS[ci + 1])
                           if ci + 1 < len(geom.CHUNKS) else None)
                RHS = RHS_cur
                if ci == 1 and len(geom.CHUNKS) > 2:
                    # bulk drel int16->f32 (its DMA landed during chunk 0)
                    nc.vector.tensor_copy(out=drel[:, geom.t0[2]:],
                                          in_=drel16[:, geom.t0[2]:])

                def emit_prev_epilogue():
                    if prev is not None:
                        pci, paccP, pcw = prev
                        for e0 in range(0, pcw, 5):
                            epilogue(pci, paccP, e0, min(e0 + 5, pcw))

                # steady state: previous chunk's epilogue first (its deps are
                # long done, so the in-order DVE queue never stalls on it and
                # it fills DVE while this chunk's gathers land). During the
                # fill (ci < 4) deps complete in order S -> epilogue -> msgs,
                # so emit in that order instead to avoid head-of-line blocks.
                if ci >= fill_chunks:
                    emit_prev_epilogue()

                def emit_msg(lo0, n):
                    # msg = xh[src] * e (broadcast over channels; c-major)
                    in0 = G[:, lo0:lo0 + n, :].rearrange(
                        "p t (c h) -> p t c h", h=H1)
                    in1 = RHS[:, lo0:lo0 + n, 128:128 + H1].unsqueeze(
                        2).broadcast_to([128, n, C1, H1])
                    out0 = RHS[:, lo0:lo0 + n, 0:128].rearrange(
                        "p t (c h) -> p t c h", h=H1)
                    nc.vector.tensor_tensor(out=out0, in0=in0, in1=in1,
                                            op=ALU.mult)

                def build_S(wi, t):
                    gl = _tile_of(ci, cw, wi, t, 0)
                    S = spool.tile([128, 128], BF16, name="S")
                    eng = (nc.gpsimd if (wi == cw - 1 and t < pool_s)
                           else nc.vector)
                    eng.tensor_scalar(
                        out=S[:], in0=iota,
                        scalar1=drel[:, t0 + gl:t0 + gl + 1], scalar2=None,
                        op0=ALU.is_equal)
                    return (gl, S)

                accP = accppool.tile([128, max(geom.CHUNKS), 128 + H1], BF16,
                                     name="accP")
                if ci < fill_chunks:
                    # warm-up chunks: S builds first (no gather dep), so DVE
                    # starts ~2.5us before the first gather lands
                    Sw = [[build_S(wi, t) for t in range(TPW)]
                          for wi in range(cw)]
                    emit_prev_epilogue()
                    for m0 in range(0, tpc, GATHER_TILES):
                        emit_msg(m0, min(GATHER_TILES, tpc - m0))
                    for wi in range(cw):
                        psum = ppool.tile([128, 128 + H1], F32, space="PSUM",
                                          name="psum")
                        for t, (gl, S) in enumerate(Sw[wi]):
                            nc.tensor.matmul(out=psum[:], lhsT=S[:],
                                             rhs=RHS[:, gl, :],
                                             start=(t == 0),
                                             stop=(t == TPW - 1))
                        nc.scalar.copy(out=accP[:, wi, :], in_=psum[:])
                else:
                    for m0 in range(0, nlo_t, GATHER_TILES):
                        emit_msg(m0, min(GATHER_TILES, nlo_t - m0))
                    for wi in range(cw):
                        Ss = [build_S(wi, t) for t in range(TPW)]
                        if wi == 0:
                            for m0 in range(nlo_t, tpc, GATHER_TILES):
                                emit_msg(m0, min(GATHER_TILES, tpc - m0))
                        psum = ppool.tile([128, 128 + H1], F32, space="PSUM",
                                          name="psum")
                        for t, (gl, S) in enumerate(Ss):
                            nc.tensor.matmul(out=psum[:], lhsT=S[:],
                                             rhs=RHS[:, gl, :],
                                             start=(t == 0),
                                             stop=(t == TPW - 1))
                        nc.scalar.copy(out=accP[:, wi, :], in_=psum[:])
                prev = (ci, accP, cw)
                RHS_cur = RHS_nxt
            pci, paccP, pcw = prev
            for e0 in range(0, pcw, 5):
                epilogue(pci, paccP, e0, min(e0 + 5, pcw))
            # deferred output dumps: pieces ordered by epilogue completion
            cuts = [0, W // 3, 2 * W // 3, geom.w0[-3], geom.w0[-2], W]
            for a, b in zip(cuts, cuts[1:]):
                if b > a:
                    nc.sync.dma_start(
                        out=dump_out[:, a * (FOUT + 2):b * (FOUT + 2)],
                        in_=accAll[:, a:b, :])
    nc.compile()
    return nc


def build_E2(geom):
    W, NTILES = geom.WINS, geom.NTILES
    LO_COLS = W * LOT * 128 // 16
    HI_COLS = W * HIT * 128 // 16
    nc = _new_nc()
    table_in = nc.declare_dram_parameter("table", [N, 256], BF16, isOutput=False)
    ae_in = nc.declare_dram_parameter("ae", [128, NTILES, 1], BF16,
                                      isOutput=False)
    ilo_in = nc.declare_dram_parameter("ilo", [128, LO_COLS], I16,
                                       isOutput=False)
    ihi_in = nc.declare_dram_parameter("ihi", [128, HI_COLS], I16,
                                       isOutput=False)
    drel_in = nc.declare_dram_parameter("drel", [128, NTILES], I16, isOutput=False)

    dump_out = nc.declare_dram_parameter("dump", [128, W * FOUT], BF16,
                                         isOutput=True)

    with tile.TileContext(nc) as tc:
        with (
            tc.tile_pool(name="const", bufs=1) as cpool,
            tc.tile_pool(name="gat", bufs=2) as gpool,
            tc.tile_pool(name="alp", bufs=2) as apool,
            tc.tile_pool(name="sel", bufs=24) as spool,
            tc.tile_pool(name="psw", bufs=7, space="PSUM") as ppool,
            tc.tile_pool(name="agg", bufs=2) as aggpool,
            tc.tile_pool(name="rc", bufs=4) as rcpool,
        ):
            ilo = cpool.tile([128, LO_COLS], I16)
            ihi = cpool.tile([128, HI_COLS], I16)
            drel = cpool.tile([128, NTILES], F32)
            drel16 = cpool.tile([128, NTILES], I16)
            ae = cpool.tile([128, NTILES, 1], BF16)
            accAll = cpool.tile([128, W, FOUT + 2], BF16)
            iota_t = cpool.tile([128, 128], BF16)
            iota = iota_t[:]
            nc.gpsimd.iota(out=iota, pattern=[[1, 128]], base=0,
                           channel_multiplier=0,
                           allow_small_or_imprecise_dtypes=True)

            def load_chunk_inputs(c, cend=None):
                cend = c + 1 if cend is None else cend
                l0, l1 = geom.lo_c0[c], geom.lo_c0[cend]
                h0, h1 = geom.hi_c0[c], geom.hi_c0[cend]
                t0, t1 = geom.t0[c], geom.t0[cend]
                nc.sync.dma_start(out=ilo[:, l0:l1], in_=ilo_in[:, l0:l1])
                nc.sync.dma_start(out=ihi[:, h0:h1], in_=ihi_in[:, h0:h1])
                nc.sync.dma_start(out=drel16[:, t0:t1], in_=drel_in[:, t0:t1])
                nc.sync.dma_start(out=ae[:, t0:t1, :], in_=ae_in[:, t0:t1, :])

            nchunks = len(geom.CHUNKS)
            load_chunk_inputs(0, min(2, nchunks))
            if nchunks > 2:
                load_chunk_inputs(2, nchunks)
            tcv = geom.t0[min(2, nchunks)]
            nc.vector.tensor_copy(out=drel[:, 0:tcv], in_=drel16[:, 0:tcv])

            for ci, cw in enumerate(geom.CHUNKS):
                t0 = geom.t0[ci]
                tpc = cw * TPW
                nlo_t = cw * LOT
                if ci == 1 and len(geom.CHUNKS) > 2:
                    nc.vector.tensor_copy(out=drel[:, geom.t0[2]:],
                                          in_=drel16[:, geom.t0[2]:])
                G = gpool.tile([128, geom.TPC_MAX, 256], BF16, name="G")
                _emit_gathers_il(nc, G, table_in[:, :], table_in[HI_BASE:, :],
                                 ilo, ihi, nlo_t, tpc - nlo_t,
                                 geom.lo_c0[ci], geom.hi_c0[ci])
                A = apool.tile([128, geom.TPC_MAX, 1], BF16, name="A")
                A2 = apool.tile([128, geom.TPC_MAX, 1], F32, name="A2")
                nc.scalar.activation(out=A[:, 0:tpc, :],
                                     in_=ae[:, t0:t0 + tpc, :],
                                     func=ACTF.Prelu, alpha=NEG_SLOPE)
                nc.scalar.activation(out=A2[:, 0:tpc, :], in_=A[:, 0:tpc, :],
                                     func=ACTF.Exp)
                aggN = aggpool.tile([128, max(geom.CHUNKS), FOUT], BF16,
                                    name="aggN")
                for wi in range(cw):
                    Ss = []
                    for t in range(TPW):
                        g = _tile_of(ci, cw, wi, t, 0)
                        S = spool.tile([128, 128], BF16, name="S")
                        nc.vector.tensor_scalar(
                            out=S[:], in0=iota,
                            scalar1=drel[:, t0 + g:t0 + g + 1],
                            scalar2=A2[:, g, 0:1],
                            op0=ALU.is_equal, op1=ALU.mult)
                        Ss.append((g, S))
                    psum = ppool.tile([128, 129], F32, space="PSUM",
                                      name="psum")
                    for t, (g, S) in enumerate(Ss):
                        nc.tensor.matmul(out=psum[:], lhsT=S[:],
                                         rhs=G[:, g, 0:129],
                                         start=(t == 0), stop=(t == TPW - 1))
                    # out = agg / s: fold 1/s into the ACT psum copy as a
                    # per-partition scale
                    sEps = rcpool.tile([128, 1], F32, name="sEps")
                    nc.scalar.activation(out=sEps[:], in_=psum[:, 128:129],
                                         func=ACTF.Copy, bias=1e-30)
                    rec = rcpool.tile([128, 1], F32, name="rec")
                    nc.vector.reciprocal(out=rec[:], in_=sEps[:])
                    nc.scalar.activation(out=aggN[:, wi, :], in_=psum[:, 0:128],
                                         func=ACTF.Copy, scale=rec[:])
                # b2 is host-folded into the table rows (coefs sum to 1)
                c0 = geom.w0[ci] * FOUT
                c1 = geom.w0[ci + 1] * FOUT
                nc.sync.dma_start(out=dump_out[:, c0:c1],
                                  in_=aggN[:, 0:cw, :])
    nc.compile()
    return nc


# ----------------------------------------------------------------------------
# Host orchestration
# ----------------------------------------------------------------------------

def _run(nc, in_maps, tag):
    trace = os.environ.get("KERNEL_TRACE", "0") == "1"
    res = run_bass_kernel_spmd(nc, in_maps, list(range(NCORES)), trace=trace)
    if trace:
        _CACHE.setdefault("profiles", {})[tag] = res
    return res.results


def _expand_ae(cores, a_src, a_dst):
    """Host-expanded per-slot alpha = a_src[src] + a_dst[dst] per core."""
    a_src = a_src.astype(np.float32)
    a_dst = a_dst.astype(np.float32)
    return [np.ascontiguousarray(
        (a_src[cd["slot_src"]] + a_dst[cd["slot_dst"]]).astype(BF))
        for cd in cores]


def kernel(x, src, dst, W1, att_src1, att_dst1, b1, W2, att_src2, att_dst2, b2):
    x = np.asarray(x, np.float32)
    src = np.asarray(src, np.int64)
    dst = np.asarray(dst, np.int64)
    W1 = np.asarray(W1, np.float32)
    W2 = np.asarray(W2, np.float32)
    att_src1 = np.asarray(att_src1, np.float32)
    att_dst1 = np.asarray(att_dst1, np.float32)
    att_src2 = np.asarray(att_src2, np.float32)
    att_dst2 = np.asarray(att_dst2, np.float32)
    b1 = np.asarray(b1, np.float32)
    b2 = np.asarray(b2, np.float32)

    ekey = ("edges", hash(src.tobytes()), hash(dst.tobytes()))
    if ekey not in _CACHE:
        plan = _plan_all(src, dst)
        W = plan[4]
        geom1 = Geom(W, _e1_chunks(W))
        geom2 = Geom(W)
        _CACHE[ekey] = (geom1, geom2, _fill_cores(plan, geom1),
                        _fill_cores(plan, geom2))
    geom1, geom2, cores1, cores2 = _CACHE[ekey]

    pkey = ("progs_geom", geom1.WINS, tuple(geom1.CHUNKS),
            tuple(geom2.CHUNKS))
    if pkey not in _CACHE:
        _CACHE[pkey] = (build_T(), build_E1(geom1), build_E2(geom2))
        _CACHE["progs"] = _CACHE[pkey]
    ncT, ncE1, ncE2 = _CACHE[pkey]

    perm = _perm_cmajor()
    W1P = np.ascontiguousarray(W1[:, perm])
    W1A_src = np.einsum("fhc,hc->fh", W1.reshape(FIN, H1, C1), att_src1)
    W1A_dst = np.einsum("fhc,hc->fh", W1.reshape(FIN, H1, C1), att_dst1)
    WT = np.concatenate([W1P, W1A_src, W1A_dst], axis=1).astype(BF)  # [128,144]
    b1P = b1[perm].astype(np.float32)
    W2P = np.ascontiguousarray(W2[perm, :])
    att2cat = np.stack([att_src2[0], att_dst2[0]], axis=1).astype(np.float32)
    W2A = (W2P @ att2cat).astype(np.float32)  # [128, 2] in permuted row space
    W2C = np.concatenate([W2P, W2A], axis=1).astype(BF)  # [128, 130]

    ident = np.eye(128, dtype=np.float32).astype(BF)
    iota = np.tile(np.arange(128, dtype=np.float32), (128, 1)).astype(BF)
    b1rep = np.tile(b1P, (128, 1)).astype(BF)
    b2rep = np.tile(b2, (128, 1)).astype(BF)
    cst1 = np.ascontiguousarray(
        np.concatenate([ident, W2C], axis=1))               # [128, 258]

    # ---- Launch T: per-core [xh | a_src | a_dst] -------------------------
    xtpad = np.zeros((NCORES, 128, NPC_PAD), BF)
    for c in range(NCORES):
        xtpad[c, :, :NPC] = x[c * NPC:(c + 1) * NPC].T.astype(BF)
    in_maps = [{"xt": xtpad[c], "w": WT} for c in range(NCORES)]
    resT = _run(ncT, in_maps, "T")
    parts = []
    for c in range(NCORES):
        d = resT[c]["dump"].reshape(128, NT_T, TCOLS)
        parts.append(d.transpose(1, 0, 2).reshape(NPC_PAD, TCOLS)[:NPC])
    ta = np.concatenate(parts)                      # [N, 144] bf16
    # fold b1 into the rows: (sum e*(xh+b1))/s = (sum e*xh)/s + b1
    table1 = np.ascontiguousarray(
        (ta[:, 0:FMID].astype(np.float32) + b1P).astype(BF))  # [N, 128]
    a1_src = np.ascontiguousarray(ta[:, FMID:FMID + H1])
    a1_dst = np.ascontiguousarray(ta[:, FMID + H1:FMID + 2 * H1])
    ae1 = _expand_ae(cores1, a1_src, a1_dst)

    # ---- Launch E1 --------------------------------------------------------
    in_maps = [{"table": table1, "ae": ae1[c], "ilo": cores1[c]["idx_lo"],
                "ihi": cores1[c]["idx_hi"], "drel": cores1[c]["drel"],
                "cst": cst1}
               for c in range(NCORES)]
    resE1 = _run(ncE1, in_maps, "E1")
    ha = np.zeros((N, FOUT + 2), BF)
    for c in range(NCORES):
        d = resE1[c]["dump"].reshape(128, geom1.WINS, FOUT + 2)
        for w, (base, nd) in enumerate(cores1[c]["wmap"]):
            if nd:
                ha[base:base + nd] = d[0:nd, w, :]
    table2 = np.zeros((N, 256), BF)                 # [xh2+b2 | 1.0 | pad]
    table2[:, 0:FOUT] = (ha[:, 0:FOUT].astype(np.float32) + b2).astype(BF)
    table2[:, FOUT] = BF(1.0)
    a2_src = np.ascontiguousarray(ha[:, FOUT:FOUT + 1])
    a2_dst = np.ascontiguousarray(ha[:, FOUT + 1:FOUT + 2])
    ae2 = _expand_ae(cores2, a2_src, a2_dst)

    # ---- Launch E2 --------------------------------------------------------
    in_maps = [{"table": table2, "ae": ae2[c], "ilo": cores2[c]["idx_lo"],
                "ihi": cores2[c]["idx_hi"], "drel": cores2[c]["drel"]}
               for c in range(NCORES)]
    resE2 = _run(ncE2, in_maps, "E2")
    out = np.zeros((N, FOUT), np.float32)
    for c in range(NCORES):
        d = resE2[c]["dump"].reshape(128, geom2.WINS, FOUT).astype(np.float32)
        for w, (base, nd) in enumerate(cores2[c]["wmap"]):
            if nd:
                out[base:base + nd] = d[0:nd, w, :]
    return np.ascontiguousarray(out)
